# revision 37
# baseline (speedup 1.0000x reference)
"""Bass/Trainium2 kernel for Bahdanau (additive) attention, 8-core data-parallel.

Reference computation (per batch b):
    att1 = enc[b] @ We                    # [N, A]
    att2 = dec[b] @ Wd + bd               # [A]
    att  = tanh(att1 + att2 + be)         # [N, A]
    s    = att @ Wf (+ bf)                # [N]   (bf dropped: softmax-invariant)
    alpha= softmax(s)                     # [N]
    ctx  = sum_n alpha[n] * enc[b, n]     # [E]

Sharding: batch 128 -> 8 cores x 16. Weights replicated. No collectives.

Per-core design (B_loc=16, processed in 8 pairs):
  - enc is loaded f32 (split DMAs across queues), converted to bf16 on
    DVE/ACT, and transposed by the DMA xbar (dma_start transpose=True,
    bf16) straight into SBUF - the PE does no data transposes.
  - att1 computed transposed ([A_chunk=128, n-cols]) in bf16:
    lhsT = We blocks (natural), moving = encT [128, 416] per pair.
  - att2+be add fused into tanh on ScalarE as a per-partition bias.
  - scores via PE: lhsT = Wf chunk [128,1], moving = tanh output.
  - softmax on one partition per pair (reduce_max neg / exp+accum / recip).
  - context via PE with a zero-padded block-diagonal alphaT (each batch's
    alpha column at 256-aligned row offsets) against the bf16 enc tiles,
    all 16 batches accumulated into 4 persistent PSUM banks [16, 512].
  - 3-deep software pipeline: loads(p+2) / convert+xbar(p+1) /
    att1+scores(p) with alpha-placement+context of p-1 emitted mid-att1
    so the PE instruction stream never waits on the softmax chain.
"""

import numpy as np

B, N, E = 128, 196, 2048
D, A = 512, 512
NCORES = 8
BL = B // NCORES            # 16 batches per core
PAIRS = BL // 2             # 8
N0 = 128
N1 = N - N0                 # 68
NP = 80                     # N1 padded to a multiple of XBAR_TILE_SRC_ROWS (16)
PW = N0 + NP                # 208 cols per batch in the transposed layout
W = 2 * PW                  # 416 cols per pair
EC = E // 128               # 16
AC = A // 128               # 4
E4 = E // 512               # 4
DC = D // 128               # 4

_CACHE = {}


def _patch_tile_tail_drain(tile):
    """This walrus build rejects >1 sem-wait per instruction. Split extra
    waits onto single-wait NOPs committed just before the instruction, and
    do the same for the TileContext tail drain."""
    import concourse.mybir as mybir
    from concourse.vector_clock import ScopedClock

    if getattr(tile.TileContext, "_tail_drain_patched", False):
        return

    orig_commit = tile.TileContext._commit_instruction

    def _commit_instruction(self, inst, lazy_reg_writes=True):
        si = getattr(inst, "sync_info", None)
        engine = getattr(inst, "engine", None)
        if (
            si is not None
            and si.on_wait
            and len(si.on_wait) > 1
            and engine is not None
            and engine != mybir.EngineType.Unassigned
            and type(inst).__name__.startswith("Inst")
        ):
            waits = list(si.on_wait)
            for w in waits[:-1]:
                noop = mybir.InstNoOp(
                    name=self.nc.get_next_instruction_name(),
                    sync_info=mybir.SyncInfo(on_wait=[w], on_update=[]),
                    bass_nofuse=True,
                    engine=engine,
                )
                orig_commit(self, noop, lazy_reg_writes=False)
            inst.sync_info = mybir.SyncInfo(
                on_wait=[waits[-1]], on_update=list(si.on_update or [])
            )
        return orig_commit(self, inst, lazy_reg_writes)

    tile.TileContext._commit_instruction = _commit_instruction

    def _drain_and_barrier(self, tick_clock, wait_clock):
        nc = self.nc
        drain_inst = nc.sync.drain()
        wait_clock.add_sem_waits(
            drain_inst.ins, ScopedClock({None: tick_clock.global_clock})
        )
        si = drain_inst.ins.sync_info
        waits = list(si.on_wait or []) if si is not None else []
        if len(waits) > 1:
            drain_inst.ins.sync_info = mybir.SyncInfo(
                on_wait=waits[:1], on_update=list(si.on_update or [])
            )
            for w in waits[1:]:
                d = nc.sync.drain()
                d.ins.sync_info = mybir.SyncInfo(on_wait=[w], on_update=[])
        nc.all_engine_barrier()
        assert self.sems is not None
        popped = nc._tile_sem_poison_stack.pop()
        assert popped is self._sem_poison
        nc.clear_and_free_semaphores(list(self.sems.allocated().values()))
        nc.all_engine_barrier()

    tile.TileContext._drain_and_barrier = _drain_and_barrier
    tile.TileContext._tail_drain_patched = True


def build():
    from contextlib import ExitStack

    import concourse.bass as bass
    import concourse.tile as tile
    from concourse import mybir
    from concourse.bass import ts
    from concourse.masks import make_identity

    _patch_tile_tail_drain(tile)

    f32 = mybir.dt.float32
    bf16 = mybir.dt.bfloat16
    TANH = mybir.ActivationFunctionType.Tanh
    EXP = mybir.ActivationFunctionType.Exp

    nc = bass.Bass(trn_type="TRN2", target_bir_lowering=False, debug=False)
    enc_d = nc.dram_tensor("encoder_out", [BL, N, E], f32, kind="ExternalInput")
    dec_d = nc.dram_tensor("decoder_hidden", [BL, D], f32, kind="ExternalInput")
    we_d = nc.dram_tensor("We", [E, A], f32, kind="ExternalInput")
    be_d = nc.dram_tensor("be", [A], f32, kind="ExternalInput")
    wd_d = nc.dram_tensor("Wd", [D, A], f32, kind="ExternalInput")
    bd_d = nc.dram_tensor("bd", [A], f32, kind="ExternalInput")
    wf_d = nc.dram_tensor("Wf", [A, 1], f32, kind="ExternalInput")
    ctx_d = nc.dram_tensor("context", [BL, E], f32, kind="ExternalOutput")
    alp_d = nc.dram_tensor("alpha", [BL, N, 1], f32, kind="ExternalOutput")

    with tile.TileContext(nc) as tc, ExitStack() as ctx:
        consts = ctx.enter_context(tc.tile_pool(name="consts", bufs=1))

        # ---- constants / weights (preamble-only tensors live in a pool
        # that closes before the main working set opens) ----
        ident = consts.tile([128, 128], f32)
        make_identity(nc, ident[:])
        we_bf = consts.tile([128, EC, A], bf16)
        wf_bf = consts.tile([128, AC], bf16)
        att2T_sb = consts.tile([128, AC, BL], f32)
        alphaT_pad = consts.tile([128, 2 * BL, BL], bf16)

        with tc.tile_pool(name="pre", bufs=1) as pre, tc.tile_pool(
            name="prepsum", bufs=1, space="PSUM"
        ) as prepsum:
            stage = pre.tile([128, EC, A], f32)
            for c in range(EC):
                nc.sync.dma_start(stage[:, c, :], we_d.ap()[ts(c, 128), :])
            for c in range(EC):
                nc.vector.tensor_copy(we_bf[:, c, :], stage[:, c, :])

            wd_sb = pre.tile([128, DC, A], f32)
            for c in range(DC):
                nc.sync.dma_start(wd_sb[:, c, :], wd_d.ap()[ts(c, 128), :])
            wf_f = pre.tile([128, AC], f32)
            nc.sync.dma_start(wf_f[:], wf_d.ap().rearrange("(c p) o -> p (c o)", p=128))
            nc.vector.tensor_copy(wf_bf[:], wf_f[:])
            be_sb = pre.tile([128, AC], f32)
            bd_sb = pre.tile([128, AC], f32)
            nc.sync.dma_start(be_sb[:], be_d.ap().rearrange("(c p) -> p c", p=128))
            nc.sync.dma_start(bd_sb[:], bd_d.ap().rearrange("(c p) -> p c", p=128))
            bdbe = pre.tile([128, AC], f32)
            nc.vector.tensor_add(bdbe[:], be_sb[:], bd_sb[:])

            dec_sb = pre.tile([BL, D], f32)
            nc.sync.dma_start(dec_sb[:], dec_d.ap())

            # decT [d, b] via PE transposes
            decT_sb = pre.tile([128, DC, BL], f32)
            for c in range(DC):
                ps_tp = prepsum.tile([128, BL], f32, tag="tp", bufs=2)
                nc.tensor.transpose(
                    ps_tp[:], dec_sb[:, ts(c, 128)], ident[0:BL, 0:BL]
                )
                nc.vector.tensor_copy(decT_sb[:, c, :], ps_tp[:])

            # att2T [a, b] = Wd.T @ decT (+ bd + be), f32 exact
            for ca in range(AC):
                ps_a2 = prepsum.tile([128, BL], f32, tag="tp", bufs=2)
                for cd in range(DC):
                    nc.tensor.matmul(
                        ps_a2[:],
                        wd_sb[:, cd, ts(ca, 128)],
                        decT_sb[:, cd, :],
                        start=(cd == 0),
                        stop=(cd == DC - 1),
                    )
                nc.scalar.add(att2T_sb[:, ca, :], ps_a2[:], bdbe[:, ca : ca + 1])

            nc.gpsimd.memset(alphaT_pad[:], 0.0)

        work = ctx.enter_context(tc.tile_pool(name="work", bufs=2))
        psum = ctx.enter_context(tc.tile_pool(name="psum", bufs=1, space="PSUM"))

        # persistent context accumulators: 4 banks of [16, 512]
        ctx_ps = psum.tile([BL, E4, 512], f32, tag="ctx", bufs=1)

        def emit_loads(p):
            # SWDGE (gpsimd) spreads each big transfer across all 16 SDMA
            # engines; HWDGE rings would serialize on ~4 queues.
            b0, b1 = 2 * p, 2 * p + 1
            nat0 = work.tile([128, 2, E], f32, tag="nat0", bufs=2, name="nat0")
            nat1 = work.tile([N1, 2, E], f32, tag="nat1", bufs=2, name="nat1")
            for j, b in enumerate((b0, b1)):
                nc.gpsimd.dma_start(nat0[:, j, :], enc_d.ap()[b, 0:N0, :])
                nc.gpsimd.dma_start(nat1[:, j, :], enc_d.ap()[b, N0:N, :])
            return nat0, nat1

        def emit_convert(p, nats):
            nat0, nat1 = nats
            nbf0 = work.tile([128, 2, E], bf16, tag="nbf0", bufs=4, name="nbf0")
            nbf1 = work.tile([NP, 2, E], bf16, tag="nbf1", bufs=4, name="nbf1")
            # engines need 32-aligned base partitions: zero [64:80], the
            # conversion then overwrites the real rows [64:68]
            nc.gpsimd.memset(nbf1[64:NP, :, :], 0.0)
            for j in range(2):
                nc.vector.tensor_copy(nbf0[:, j, :], nat0[:, j, :])
                nc.vector.tensor_copy(nbf1[0:N1, j, :], nat1[:, j, :])
            return nbf0, nbf1

        def emit_xbar(p, bf):
            nbf0, nbf1 = bf
            encT = work.tile([128, EC, W], bf16, tag="encT", bufs=2, name="encT")
            # xbar triggers occupy the issuing engine for the DMA duration:
            # use the otherwise-idle SP engine
            for j in range(2):
                nc.sync.dma_start(
                    encT[:, :, PW * j : PW * j + N0], nbf0[:, j, :], transpose=True
                )
                nc.sync.dma_start(
                    encT[:, :, PW * j + N0 : PW * (j + 1)], nbf1[:, j, :],
                    transpose=True,
                )
            return encT

        def emit_att1(p, encT, ca_list):
            b0, b1 = 2 * p, 2 * p + 1
            att = _CACHE_ATT[p % 2]
            for ca in ca_list:
                ps_a = psum.tile([128, W], f32, tag="att1", bufs=2, name="ps_a")
                for ce in range(EC):
                    nc.tensor.matmul(
                        ps_a[:],
                        we_bf[:, ce, ts(ca, 128)],
                        encT[:, ce, :],
                        start=(ce == 0),
                        stop=(ce == EC - 1),
                    )
                nc.scalar.activation(
                    att[:, ca, 0:PW], ps_a[:, 0:PW], TANH,
                    bias=att2T_sb[:, ca, b0 : b0 + 1],
                )
                nc.scalar.activation(
                    att[:, ca, PW:W], ps_a[:, PW:W], TANH,
                    bias=att2T_sb[:, ca, b1 : b1 + 1],
                )

        def emit_scores_softmax(p):
            b0 = 2 * p
            att = _CACHE_ATT[p % 2]
            ps_s = psum.tile([128, W], f32, tag="att1", bufs=2, name="ps_s")
            for ca in range(AC):
                nc.tensor.matmul(
                    ps_s[0:1, :],
                    wf_bf[:, ca : ca + 1],
                    att[:, ca, :],
                    start=(ca == 0),
                    stop=(ca == AC - 1),
                )
            sc_sb = work.tile([1, W], f32, tag="sc", bufs=2, name="sc")
            nc.vector.tensor_copy(sc_sb[:], ps_s[0:1, :])

            mx = work.tile([1, 2], f32, tag="mx", bufs=2, name="mx")
            sm = work.tile([1, 2], f32, tag="sm", bufs=2, name="sm")
            rs = work.tile([1, 2], f32, tag="rs", bufs=2, name="rs")
            nc.vector.reduce_max(
                mx[:],
                sc_sb[:].rearrange("p (h w) -> p h w", h=2)[:, :, 0:N],
                axis=mybir.AxisListType.X,
                negate=True,
            )
            al = work.tile([1, 2 * N], f32, tag="al", bufs=2, name="al")
            for h in range(2):
                nc.scalar.activation(
                    al[0:1, ts(h, N)], sc_sb[0:1, PW * h : PW * h + N], EXP,
                    bias=mx[0:1, h : h + 1],
                    accum_out=sm[0:1, h : h + 1],
                )
            nc.vector.reciprocal(rs[:], sm[:])
            for h in range(2):
                nc.vector.tensor_scalar_mul(
                    al[0:1, ts(h, N)], al[0:1, ts(h, N)], rs[0:1, h : h + 1]
                )
            nc.gpsimd.dma_start(
                alp_d.ap()[b0 : b0 + 2].rearrange("b n o -> o (b n)"), al[:]
            )
            return al

        def emit_alpha_ctx(p, al, nbf0, nbf1):
            b0, b1 = 2 * p, 2 * p + 1
            ps_al = psum.tile([128, W], f32, tag="att1", bufs=2, name="ps_al")
            for j, b in enumerate((b0, b1)):
                off = N * j
                nc.tensor.transpose(
                    ps_al[:, 2 * j : 2 * j + 1],
                    al[0:1, off : off + N0],
                    ident[0:1, 0:1],
                )
                nc.tensor.transpose(
                    ps_al[0:N1, 2 * j + 1 : 2 * j + 2],
                    al[0:1, off + N0 : off + N],
                    ident[0:1, 0:1],
                )
                nc.vector.tensor_copy(
                    alphaT_pad[:, 2 * b, b : b + 1], ps_al[:, 2 * j : 2 * j + 1]
                )
                nc.vector.tensor_copy(
                    alphaT_pad[0:N1, 2 * b + 1, b : b + 1],
                    ps_al[0:N1, 2 * j + 1 : 2 * j + 2],
                )
            for j, b in enumerate((b0, b1)):
                for e4 in range(E4):
                    nc.tensor.matmul(
                        ctx_ps[:, e4, :],
                        alphaT_pad[:, 2 * b, :],
                        nbf0[:, j, ts(e4, 512)],
                        start=(p == 0 and j == 0),
                        stop=False,
                        skip_group_check=True,
                    )
                    nc.tensor.matmul(
                        ctx_ps[:, e4, :],
                        alphaT_pad[0:N1, 2 * b + 1, :],
                        nbf1[0:N1, j, ts(e4, 512)],
                        start=False,
                        stop=(p == PAIRS - 1 and j == 1),
                        skip_group_check=True,
                    )

        # att tiles persist across the pipeline; double-buffered by parity
        _CACHE_ATT = [
            work.tile([128, AC, W], bf16, tag="att", bufs=2, name=f"att{i}")
            for i in range(2)
        ]

        # ---- 3-deep pipeline ----
        # Per-section emission order matters: it fixes each engine's queue
        # order. att1(p) first so the PE has dense work while softmax(p-1)
        # drains; then the deferred alpha/context of p-1 (its DVE copies go
        # ahead of the next conversions in the DVE queue); then the
        # convert+xbar for p+1 and loads for p+2.
        # Schedule (section p):
        #   top:  loads(p+2) [gpsimd/SWDGE], xbar(p+1) [SP]
        #   mid:  att1(p) ca0,1 ; alphaT+ctx(p-1) ; att1(p) ca2,3
        #   end:  scores+softmax(p) ; convert(p+2) [DVE]
        # nat dies at its conversion (2 bufs); nbf lives conv..ctx (4 bufs);
        # encT lives xbar..att1 (2 bufs).
        nats = {0: emit_loads(0)}
        bfs = {0: emit_convert(0, nats.pop(0))}
        nats[1] = emit_loads(1)
        bfs[1] = emit_convert(1, nats.pop(1))
        encTs = {0: emit_xbar(0, bfs[0])}
        als = {}
        for p in range(PAIRS):
            if p + 2 < PAIRS:
                nats[p + 2] = emit_loads(p + 2)
            if p + 1 < PAIRS:
                encTs[p + 1] = emit_xbar(p + 1, bfs[p + 1])
            emit_att1(p, encTs[p], (0, 1))
            if p >= 1:
                emit_alpha_ctx(p - 1, als.pop(p - 1), bfs[p - 1][0], bfs[p - 1][1])
                del bfs[p - 1]
            emit_att1(p, encTs.pop(p), (2, 3))
            als[p] = emit_scores_softmax(p)
            if p + 2 < PAIRS:
                bfs[p + 2] = emit_convert(p + 2, nats.pop(p + 2))
        emit_alpha_ctx(PAIRS - 1, als.pop(PAIRS - 1), bfs[PAIRS - 1][0], bfs[PAIRS - 1][1])

        # evacuate context and store
        for e4 in range(E4):
            ctx_e = work.tile([BL, 512], f32, tag="ctxe", bufs=2, name="ctxe")
            nc.vector.tensor_copy(ctx_e[:], ctx_ps[:, e4, :])
            nc.gpsimd.dma_start(ctx_d.ap()[:, ts(e4, 512)], ctx_e[:])

    return nc


def _get_nc():
    if "nc" not in _CACHE:
        _CACHE["nc"] = build()
    return _CACHE["nc"]


def _run(in_maps, trace=False, tmpdir=None):
    from concourse.bass_utils import run_bass_kernel_spmd

    nc = _get_nc()
    return run_bass_kernel_spmd(
        nc, in_maps, core_ids=list(range(NCORES)), trace=trace, tmpdir=tmpdir
    )


def make_in_maps(encoder_out, decoder_hidden, We, be, Wd, bd, Wf, bf=None):
    enc = np.ascontiguousarray(np.asarray(encoder_out, dtype=np.float32))
    dec = np.ascontiguousarray(np.asarray(decoder_hidden, dtype=np.float32))
    We = np.ascontiguousarray(np.asarray(We, dtype=np.float32))
    be = np.ascontiguousarray(np.asarray(be, dtype=np.float32))
    Wd = np.ascontiguousarray(np.asarray(Wd, dtype=np.float32))
    bd = np.ascontiguousarray(np.asarray(bd, dtype=np.float32))
    Wf = np.ascontiguousarray(np.asarray(Wf, dtype=np.float32))
    in_maps = []
    for i in range(NCORES):
        sl = slice(i * BL, (i + 1) * BL)
        in_maps.append(
            dict(
                encoder_out=enc[sl],
                decoder_hidden=dec[sl],
                We=We,
                be=be,
                Wd=Wd,
                bd=bd,
                Wf=Wf,
            )
        )
    return in_maps


def gather(results):
    context = np.concatenate([results[i]["context"] for i in range(NCORES)], axis=0)
    alpha = np.concatenate([results[i]["alpha"] for i in range(NCORES)], axis=0)
    return context, alpha


def kernel(encoder_out, decoder_hidden, We, be, Wd, bd, Wf, bf):
    in_maps = make_in_maps(encoder_out, decoder_hidden, We, be, Wd, bd, Wf, bf)
    res = _run(in_maps, trace=False)
    _CACHE["last_results"] = res
    return gather(res.results)


# revision 39
# speedup vs baseline: 1.4700x; 1.4700x over previous
"""Bass/Trainium2 kernel for Bahdanau (additive) attention, 8-core data-parallel.

Reference computation (per batch b):
    att1 = enc[b] @ We                    # [N, A]
    att2 = dec[b] @ Wd + bd               # [A]
    att  = tanh(att1 + att2 + be)         # [N, A]
    s    = att @ Wf (+ bf)                # [N]   (bf dropped: softmax-invariant)
    alpha= softmax(s)                     # [N]
    ctx  = sum_n alpha[n] * enc[b, n]     # [E]

Sharding: batch 128 -> 8 cores x 16. Weights replicated. No collectives.

Per-core design (B_loc=16, processed in 8 pairs):
  - enc is loaded f32 (split DMAs across queues), converted to bf16 on
    DVE/ACT, and transposed by the DMA xbar (dma_start transpose=True,
    bf16) straight into SBUF - the PE does no data transposes.
  - att1 computed transposed ([A_chunk=128, n-cols]) in bf16:
    lhsT = We blocks (natural), moving = encT [128, 416] per pair.
  - att2+be add fused into tanh on ScalarE as a per-partition bias.
  - scores via PE: lhsT = Wf chunk [128,1], moving = tanh output.
  - softmax on one partition per pair (reduce_max neg / exp+accum / recip).
  - context via PE with a zero-padded block-diagonal alphaT (each batch's
    alpha column at 256-aligned row offsets) against the bf16 enc tiles,
    all 16 batches accumulated into 4 persistent PSUM banks [16, 512].
  - 3-deep software pipeline: loads(p+2) / convert+xbar(p+1) /
    att1+scores(p) with alpha-placement+context of p-1 emitted mid-att1
    so the PE instruction stream never waits on the softmax chain.
"""

import numpy as np

B, N, E = 128, 196, 2048
D, A = 512, 512
NCORES = 8
BL = B // NCORES            # 16 batches per core
PAIRS = BL // 2             # 8
N0 = 128
N1 = N - N0                 # 68
NP = 80                     # N1 padded to a multiple of XBAR_TILE_SRC_ROWS (16)
PW = N0 + NP                # 208 cols per batch in the transposed layout
W = 2 * PW                  # 416 cols per pair
EC = E // 128               # 16
AC = A // 128               # 4
E4 = E // 512               # 4
DC = D // 128               # 4

_CACHE = {}


def _patch_tile_tail_drain(tile):
    """This walrus build rejects >1 sem-wait per instruction. Split extra
    waits onto single-wait NOPs committed just before the instruction, and
    do the same for the TileContext tail drain."""
    import concourse.mybir as mybir
    from concourse.vector_clock import ScopedClock

    if getattr(tile.TileContext, "_tail_drain_patched", False):
        return

    orig_commit = tile.TileContext._commit_instruction

    def _commit_instruction(self, inst, lazy_reg_writes=True):
        si = getattr(inst, "sync_info", None)
        engine = getattr(inst, "engine", None)
        if (
            si is not None
            and si.on_wait
            and len(si.on_wait) > 1
            and engine is not None
            and engine != mybir.EngineType.Unassigned
            and type(inst).__name__.startswith("Inst")
        ):
            waits = list(si.on_wait)
            for w in waits[:-1]:
                noop = mybir.InstNoOp(
                    name=self.nc.get_next_instruction_name(),
                    sync_info=mybir.SyncInfo(on_wait=[w], on_update=[]),
                    bass_nofuse=True,
                    engine=engine,
                )
                orig_commit(self, noop, lazy_reg_writes=False)
            inst.sync_info = mybir.SyncInfo(
                on_wait=[waits[-1]], on_update=list(si.on_update or [])
            )
        return orig_commit(self, inst, lazy_reg_writes)

    tile.TileContext._commit_instruction = _commit_instruction

    def _drain_and_barrier(self, tick_clock, wait_clock):
        nc = self.nc
        drain_inst = nc.sync.drain()
        wait_clock.add_sem_waits(
            drain_inst.ins, ScopedClock({None: tick_clock.global_clock})
        )
        si = drain_inst.ins.sync_info
        waits = list(si.on_wait or []) if si is not None else []
        if len(waits) > 1:
            drain_inst.ins.sync_info = mybir.SyncInfo(
                on_wait=waits[:1], on_update=list(si.on_update or [])
            )
            for w in waits[1:]:
                d = nc.sync.drain()
                d.ins.sync_info = mybir.SyncInfo(on_wait=[w], on_update=[])
        nc.all_engine_barrier()
        assert self.sems is not None
        popped = nc._tile_sem_poison_stack.pop()
        assert popped is self._sem_poison
        nc.clear_and_free_semaphores(list(self.sems.allocated().values()))
        nc.all_engine_barrier()

    tile.TileContext._drain_and_barrier = _drain_and_barrier
    tile.TileContext._tail_drain_patched = True


def build():
    from contextlib import ExitStack

    import concourse.bass as bass
    import concourse.tile as tile
    from concourse import mybir
    from concourse.bass import ts
    from concourse.masks import make_identity

    _patch_tile_tail_drain(tile)

    f32 = mybir.dt.float32
    f32r = mybir.dt.float32r
    TANH = mybir.ActivationFunctionType.Tanh
    EXP = mybir.ActivationFunctionType.Exp

    nc = bass.Bass(trn_type="TRN2", target_bir_lowering=False, debug=False)
    enc_d = nc.dram_tensor("encoder_out", [BL, N, E], f32, kind="ExternalInput")
    dec_d = nc.dram_tensor("decoder_hidden", [BL, D], f32, kind="ExternalInput")
    we_d = nc.dram_tensor("We", [E, A], f32, kind="ExternalInput")
    be_d = nc.dram_tensor("be", [A], f32, kind="ExternalInput")
    wd_d = nc.dram_tensor("Wd", [D, A], f32, kind="ExternalInput")
    bd_d = nc.dram_tensor("bd", [A], f32, kind="ExternalInput")
    wf_d = nc.dram_tensor("Wf", [A, 1], f32, kind="ExternalInput")
    ctx_d = nc.dram_tensor("context", [BL, E], f32, kind="ExternalOutput")
    alp_d = nc.dram_tensor("alpha", [BL, N, 1], f32, kind="ExternalOutput")

    WV = 392  # pair width in the transposed layout (2 * N)

    with tile.TileContext(nc) as tc, ExitStack() as ctx:
        consts = ctx.enter_context(tc.tile_pool(name="consts", bufs=1))

        ident = consts.tile([128, 128], f32)
        make_identity(nc, ident[:])
        ident_r = consts.tile([128, 128], f32r)
        nc.vector.tensor_copy(ident_r[:], ident[:])

        we_sb = consts.tile([128, EC, A], f32r)
        for c in range(EC):
            nc.sync.dma_start(we_sb[:, c, :], we_d.ap()[ts(c, 128), :].bitcast(f32r))
        wf_sb = consts.tile([128, AC], f32r)
        nc.sync.dma_start(
            wf_sb[:], wf_d.ap().rearrange("(c p) o -> p (c o)", p=128).bitcast(f32r)
        )
        att2T_sb = consts.tile([128, AC, BL], f32)
        alphaT_pad = consts.tile([128, 2 * BL, BL], f32r)

        with tc.tile_pool(name="pre", bufs=1) as pre, tc.tile_pool(
            name="prepsum", bufs=1, space="PSUM"
        ) as prepsum:
            wd_sb = pre.tile([128, DC, A], f32)
            for c in range(DC):
                nc.sync.dma_start(wd_sb[:, c, :], wd_d.ap()[ts(c, 128), :])
            be_sb = pre.tile([128, AC], f32)
            bd_sb = pre.tile([128, AC], f32)
            nc.sync.dma_start(be_sb[:], be_d.ap().rearrange("(c p) -> p c", p=128))
            nc.sync.dma_start(bd_sb[:], bd_d.ap().rearrange("(c p) -> p c", p=128))
            bdbe = pre.tile([128, AC], f32)
            nc.vector.tensor_add(bdbe[:], be_sb[:], bd_sb[:])

            dec_sb = pre.tile([BL, D], f32)
            nc.sync.dma_start(dec_sb[:], dec_d.ap())
            decT_sb = pre.tile([128, DC, BL], f32)
            for c in range(DC):
                ps_tp = prepsum.tile([128, BL], f32, tag="tp", bufs=2)
                nc.tensor.transpose(
                    ps_tp[:], dec_sb[:, ts(c, 128)], ident[0:BL, 0:BL]
                )
                nc.vector.tensor_copy(decT_sb[:, c, :], ps_tp[:])

            for ca in range(AC):
                ps_a2 = prepsum.tile([128, BL], f32, tag="tp", bufs=2)
                for cd in range(DC):
                    nc.tensor.matmul(
                        ps_a2[:],
                        wd_sb[:, cd, ts(ca, 128)],
                        decT_sb[:, cd, :],
                        start=(cd == 0),
                        stop=(cd == DC - 1),
                    )
                nc.scalar.add(att2T_sb[:, ca, :], ps_a2[:], bdbe[:, ca : ca + 1])

            nc.gpsimd.memset(alphaT_pad[:].bitcast(f32), 0.0)

        work = ctx.enter_context(tc.tile_pool(name="work", bufs=2))
        psum = ctx.enter_context(tc.tile_pool(name="psum", bufs=1, space="PSUM"))

        ctx_ps = psum.tile([BL, E4, 512], f32, tag="ctx", bufs=1)

        def r(ap):
            return ap.bitcast(f32r)

        def emit_loads(p):
            # SWDGE (gpsimd) spreads each big transfer across all 16 SDMA
            # engines; HWDGE rings would serialize on ~4 queues.
            b0, b1 = 2 * p, 2 * p + 1
            nat0 = work.tile([128, 2, E], f32r, tag="nat0", bufs=3, name="nat0")
            nat1 = work.tile([N1, 2, E], f32r, tag="nat1", bufs=3, name="nat1")
            for j, b in enumerate((b0, b1)):
                nc.gpsimd.dma_start(nat0[:, j, :], enc_d.ap()[b, 0:N0, :].bitcast(f32r))
                nc.gpsimd.dma_start(nat1[:, j, :], enc_d.ap()[b, N0:N, :].bitcast(f32r))
            return nat0, nat1

        def copy_op(i, out, in_):
            if i % 2 == 0:
                nc.vector.tensor_copy(out, in_)
            else:
                nc.scalar.copy(out, in_)

        def emit_transposes(p, nats):
            nat0, nat1 = nats
            encT = work.tile([128, EC, WV], f32r, tag="encT", bufs=1, name="encT")
            for ce in range(EC):
                ps_t = psum.tile([128, WV], f32r, tag="tp", bufs=2, name="ps_t")
                for j in range(2):
                    off = N * j
                    nc.tensor.transpose(
                        ps_t[:, off : off + N0],
                        nat0[:, j, ts(ce, 128)],
                        ident_r[:, :],
                    )
                    nc.tensor.transpose(
                        ps_t[:, off + N0 : off + N],
                        nat1[:, j, ts(ce, 128)],
                        ident_r[0:N1, 0:N1],
                    )
                copy_op(ce, encT[:, ce, :], ps_t[:])
            return encT

        def emit_att1(p, encT, ca_list):
            b0, b1 = 2 * p, 2 * p + 1
            att = att_tiles[p % 2]
            for ca in ca_list:
                ps_a = psum.tile([128, WV], f32, tag="att1", bufs=2, name="ps_a")
                for ce in range(EC):
                    nc.tensor.matmul(
                        ps_a[:],
                        we_sb[:, ce, ts(ca, 128)],
                        encT[:, ce, :],
                        start=(ce == 0),
                        stop=(ce == EC - 1),
                    )
                nc.scalar.activation(
                    att[:, ca, 0:N], ps_a[:, 0:N], TANH,
                    bias=att2T_sb[:, ca, b0 : b0 + 1],
                )
                nc.scalar.activation(
                    att[:, ca, N:WV], ps_a[:, N:WV], TANH,
                    bias=att2T_sb[:, ca, b1 : b1 + 1],
                )

        def emit_scores_softmax(p):
            b0 = 2 * p
            att = att_tiles[p % 2]
            ps_s = psum.tile([128, WV], f32, tag="att1", bufs=2, name="ps_s")
            for ca in range(AC):
                nc.tensor.matmul(
                    ps_s[0:1, :],
                    wf_sb[:, ca : ca + 1],
                    att[:, ca, :],
                    start=(ca == 0),
                    stop=(ca == AC - 1),
                )
            sc_sb = work.tile([1, WV], f32, tag="sc", bufs=2, name="sc")
            nc.vector.tensor_copy(sc_sb[:], ps_s[0:1, :])

            mx = work.tile([1, 2], f32, tag="mx", bufs=2, name="mx")
            sm = work.tile([1, 2], f32, tag="sm", bufs=2, name="sm")
            rs = work.tile([1, 2], f32, tag="rs", bufs=2, name="rs")
            nc.vector.reduce_max(
                mx[:],
                sc_sb[:].rearrange("p (h w) -> p h w", h=2),
                axis=mybir.AxisListType.X,
                negate=True,
            )
            al = work.tile([1, WV], f32, tag="al", bufs=2, name="al")
            for h in range(2):
                nc.scalar.activation(
                    al[0:1, ts(h, N)], sc_sb[0:1, ts(h, N)], EXP,
                    bias=mx[0:1, h : h + 1],
                    accum_out=sm[0:1, h : h + 1],
                )
            nc.vector.reciprocal(rs[:], sm[:])
            for h in range(2):
                nc.vector.tensor_scalar_mul(
                    al[0:1, ts(h, N)], al[0:1, ts(h, N)], rs[0:1, h : h + 1]
                )
            nc.gpsimd.dma_start(
                alp_d.ap()[b0 : b0 + 2].rearrange("b n o -> o (b n)"), al[:]
            )
            return al

        def emit_alpha_ctx(p, al, nats):
            nat0, nat1 = nats
            b0, b1 = 2 * p, 2 * p + 1
            ps_al = psum.tile([128, WV], f32, tag="att1", bufs=2, name="ps_al")
            for j, b in enumerate((b0, b1)):
                off = N * j
                nc.tensor.transpose(
                    ps_al[:, 2 * j : 2 * j + 1],
                    al[0:1, off : off + N0],
                    ident[0:1, 0:1],
                )
                nc.tensor.transpose(
                    ps_al[0:N1, 2 * j + 1 : 2 * j + 2],
                    al[0:1, off + N0 : off + N],
                    ident[0:1, 0:1],
                )
                nc.vector.tensor_copy(
                    alphaT_pad[:, 2 * b, b : b + 1], ps_al[:, 2 * j : 2 * j + 1]
                )
                nc.vector.tensor_copy(
                    alphaT_pad[0:N1, 2 * b + 1, b : b + 1],
                    ps_al[0:N1, 2 * j + 1 : 2 * j + 2],
                )
            for j, b in enumerate((b0, b1)):
                for e4 in range(E4):
                    nc.tensor.matmul(
                        ctx_ps[:, e4, :],
                        r(alphaT_pad[:, 2 * b, :]),
                        nat0[:, j, ts(e4, 512)],
                        start=(p == 0 and j == 0),
                        stop=False,
                        skip_group_check=True,
                    )
                    nc.tensor.matmul(
                        ctx_ps[:, e4, :],
                        r(alphaT_pad[0:N1, 2 * b + 1, :]),
                        nat1[:, j, ts(e4, 512)],
                        start=False,
                        stop=(p == PAIRS - 1 and j == 1),
                        skip_group_check=True,
                    )

        att_tiles = [
            work.tile([128, AC, WV], f32r, tag="att", bufs=2, name=f"att{i}")
            for i in range(2)
        ]

        # Pipeline: loads 1 section ahead (SWDGE); per section:
        #   transposes(p) ; alphaT+ctx(p-1) ; att1(p) ; scores+softmax(p)
        # nat(p) lives load(p-1)..ctx(p+1) -> bufs=3. encT written and read
        # within the section (PE in-order) -> bufs=1.
        nats = {0: emit_loads(0)}
        als = {}
        for p in range(PAIRS):
            if p + 1 < PAIRS:
                nats[p + 1] = emit_loads(p + 1)
            encT = emit_transposes(p, nats[p])
            if p >= 1:
                emit_alpha_ctx(p - 1, als.pop(p - 1), nats.pop(p - 1))
            emit_att1(p, encT, (0, 1))
            emit_att1(p, encT, (2, 3))
            als[p] = emit_scores_softmax(p)
        emit_alpha_ctx(PAIRS - 1, als.pop(PAIRS - 1), nats.pop(PAIRS - 1))

        for e4 in range(E4):
            ctx_e = work.tile([BL, 512], f32, tag="ctxe", bufs=2, name="ctxe")
            nc.vector.tensor_copy(ctx_e[:], ctx_ps[:, e4, :])
            nc.gpsimd.dma_start(ctx_d.ap()[:, ts(e4, 512)], ctx_e[:])

    return nc


def _get_nc():
    if "nc" not in _CACHE:
        _CACHE["nc"] = build()
    return _CACHE["nc"]


def _run(in_maps, trace=False, tmpdir=None):
    from concourse.bass_utils import run_bass_kernel_spmd

    nc = _get_nc()
    return run_bass_kernel_spmd(
        nc, in_maps, core_ids=list(range(NCORES)), trace=trace, tmpdir=tmpdir
    )


def make_in_maps(encoder_out, decoder_hidden, We, be, Wd, bd, Wf, bf=None):
    enc = np.ascontiguousarray(np.asarray(encoder_out, dtype=np.float32))
    dec = np.ascontiguousarray(np.asarray(decoder_hidden, dtype=np.float32))
    We = np.ascontiguousarray(np.asarray(We, dtype=np.float32))
    be = np.ascontiguousarray(np.asarray(be, dtype=np.float32))
    Wd = np.ascontiguousarray(np.asarray(Wd, dtype=np.float32))
    bd = np.ascontiguousarray(np.asarray(bd, dtype=np.float32))
    Wf = np.ascontiguousarray(np.asarray(Wf, dtype=np.float32))
    in_maps = []
    for i in range(NCORES):
        sl = slice(i * BL, (i + 1) * BL)
        in_maps.append(
            dict(
                encoder_out=enc[sl],
                decoder_hidden=dec[sl],
                We=We,
                be=be,
                Wd=Wd,
                bd=bd,
                Wf=Wf,
            )
        )
    return in_maps


def gather(results):
    context = np.concatenate([results[i]["context"] for i in range(NCORES)], axis=0)
    alpha = np.concatenate([results[i]["alpha"] for i in range(NCORES)], axis=0)
    return context, alpha


def kernel(encoder_out, decoder_hidden, We, be, Wd, bd, Wf, bf):
    in_maps = make_in_maps(encoder_out, decoder_hidden, We, be, Wd, bd, Wf, bf)
    res = _run(in_maps, trace=False)
    _CACHE["last_results"] = res
    return gather(res.results)


# revision 42
# speedup vs baseline: 1.5710x; 1.0687x over previous
"""Bass/Trainium2 kernel for Bahdanau (additive) attention, 8-core data-parallel.

Reference computation (per batch b):
    att1 = enc[b] @ We                    # [N, A]
    att2 = dec[b] @ Wd + bd               # [A]
    att  = tanh(att1 + att2 + be)         # [N, A]
    s    = att @ Wf (+ bf)                # [N]   (bf dropped: softmax-invariant)
    alpha= softmax(s)                     # [N]
    ctx  = sum_n alpha[n] * enc[b, n]     # [E]

Sharding: batch 128 -> 8 cores x 16. Weights replicated. No collectives.

Per-core design (B_loc=16, processed in 8 pairs):
  - enc is loaded f32 (split DMAs across queues), converted to bf16 on
    DVE/ACT, and transposed by the DMA xbar (dma_start transpose=True,
    bf16) straight into SBUF - the PE does no data transposes.
  - att1 computed transposed ([A_chunk=128, n-cols]) in bf16:
    lhsT = We blocks (natural), moving = encT [128, 416] per pair.
  - att2+be add fused into tanh on ScalarE as a per-partition bias.
  - scores via PE: lhsT = Wf chunk [128,1], moving = tanh output.
  - softmax on one partition per pair (reduce_max neg / exp+accum / recip).
  - context via PE with a zero-padded block-diagonal alphaT (each batch's
    alpha column at 256-aligned row offsets) against the bf16 enc tiles,
    all 16 batches accumulated into 4 persistent PSUM banks [16, 512].
  - 3-deep software pipeline: loads(p+2) / convert+xbar(p+1) /
    att1+scores(p) with alpha-placement+context of p-1 emitted mid-att1
    so the PE instruction stream never waits on the softmax chain.
"""

import numpy as np

B, N, E = 128, 196, 2048
D, A = 512, 512
NCORES = 8
BL = B // NCORES            # 16 batches per core
PAIRS = BL // 2             # 8
N0 = 128
N1 = N - N0                 # 68
NP = 80                     # N1 padded to a multiple of XBAR_TILE_SRC_ROWS (16)
PW = N0 + NP                # 208 cols per batch in the transposed layout
W = 2 * PW                  # 416 cols per pair
EC = E // 128               # 16
AC = A // 128               # 4
E4 = E // 512               # 4
DC = D // 128               # 4

_CACHE = {}


def _patch_tile_tail_drain(tile):
    """This walrus build rejects >1 sem-wait per instruction. Split extra
    waits onto single-wait NOPs committed just before the instruction, and
    do the same for the TileContext tail drain."""
    import concourse.mybir as mybir
    from concourse.vector_clock import ScopedClock

    if getattr(tile.TileContext, "_tail_drain_patched", False):
        return

    orig_commit = tile.TileContext._commit_instruction

    def _commit_instruction(self, inst, lazy_reg_writes=True):
        si = getattr(inst, "sync_info", None)
        engine = getattr(inst, "engine", None)
        if (
            si is not None
            and si.on_wait
            and len(si.on_wait) > 1
            and engine is not None
            and engine != mybir.EngineType.Unassigned
            and type(inst).__name__.startswith("Inst")
        ):
            waits = list(si.on_wait)
            for w in waits[:-1]:
                noop = mybir.InstNoOp(
                    name=self.nc.get_next_instruction_name(),
                    sync_info=mybir.SyncInfo(on_wait=[w], on_update=[]),
                    bass_nofuse=True,
                    engine=engine,
                )
                orig_commit(self, noop, lazy_reg_writes=False)
            inst.sync_info = mybir.SyncInfo(
                on_wait=[waits[-1]], on_update=list(si.on_update or [])
            )
        return orig_commit(self, inst, lazy_reg_writes)

    tile.TileContext._commit_instruction = _commit_instruction

    def _drain_and_barrier(self, tick_clock, wait_clock):
        nc = self.nc
        drain_inst = nc.sync.drain()
        wait_clock.add_sem_waits(
            drain_inst.ins, ScopedClock({None: tick_clock.global_clock})
        )
        si = drain_inst.ins.sync_info
        waits = list(si.on_wait or []) if si is not None else []
        if len(waits) > 1:
            drain_inst.ins.sync_info = mybir.SyncInfo(
                on_wait=waits[:1], on_update=list(si.on_update or [])
            )
            for w in waits[1:]:
                d = nc.sync.drain()
                d.ins.sync_info = mybir.SyncInfo(on_wait=[w], on_update=[])
        nc.all_engine_barrier()
        assert self.sems is not None
        popped = nc._tile_sem_poison_stack.pop()
        assert popped is self._sem_poison
        nc.clear_and_free_semaphores(list(self.sems.allocated().values()))
        nc.all_engine_barrier()

    tile.TileContext._drain_and_barrier = _drain_and_barrier
    tile.TileContext._tail_drain_patched = True


def build():
    from contextlib import ExitStack

    import concourse.bass as bass
    import concourse.tile as tile
    from concourse import mybir
    from concourse.bass import ts
    from concourse.masks import make_identity

    _patch_tile_tail_drain(tile)

    f32 = mybir.dt.float32
    f32r = mybir.dt.float32r
    TANH = mybir.ActivationFunctionType.Tanh
    EXP = mybir.ActivationFunctionType.Exp

    nc = bass.Bass(trn_type="TRN2", target_bir_lowering=False, debug=False)
    enc_d = nc.dram_tensor("encoder_out", [BL, N, E], f32, kind="ExternalInput")
    dec_d = nc.dram_tensor("decoder_hidden", [BL, D], f32, kind="ExternalInput")
    we_d = nc.dram_tensor("We", [E, A], f32, kind="ExternalInput")
    be_d = nc.dram_tensor("be", [A], f32, kind="ExternalInput")
    wd_d = nc.dram_tensor("Wd", [D, A], f32, kind="ExternalInput")
    bd_d = nc.dram_tensor("bd", [A], f32, kind="ExternalInput")
    wf_d = nc.dram_tensor("Wf", [A, 1], f32, kind="ExternalInput")
    ctx_d = nc.dram_tensor("context", [BL, E], f32, kind="ExternalOutput")
    alp_d = nc.dram_tensor("alpha", [BL, N, 1], f32, kind="ExternalOutput")

    WV = 392  # pair width in the transposed layout (2 * N)

    with tile.TileContext(nc) as tc, ExitStack() as ctx:
        consts = ctx.enter_context(tc.tile_pool(name="consts", bufs=1))

        ident = consts.tile([128, 128], f32)
        make_identity(nc, ident[:])
        ident_r = consts.tile([128, 128], f32r)
        nc.vector.tensor_copy(ident_r[:], ident[:])

        we_sb = consts.tile([128, EC, A], f32r)
        for c in range(EC):
            nc.sync.dma_start(we_sb[:, c, :], we_d.ap()[ts(c, 128), :].bitcast(f32r))
        wf_sb = consts.tile([128, AC], f32r)
        nc.sync.dma_start(
            wf_sb[:], wf_d.ap().rearrange("(c p) o -> p (c o)", p=128).bitcast(f32r)
        )
        att2T_sb = consts.tile([128, AC, BL], f32)
        alphaT_pad = consts.tile([128, 2 * BL, BL], f32r)

        with tc.tile_pool(name="pre", bufs=1) as pre, tc.tile_pool(
            name="prepsum", bufs=1, space="PSUM"
        ) as prepsum:
            wd_sb = pre.tile([128, DC, A], f32)
            for c in range(DC):
                nc.sync.dma_start(wd_sb[:, c, :], wd_d.ap()[ts(c, 128), :])
            be_sb = pre.tile([128, AC], f32)
            bd_sb = pre.tile([128, AC], f32)
            nc.sync.dma_start(be_sb[:], be_d.ap().rearrange("(c p) -> p c", p=128))
            nc.sync.dma_start(bd_sb[:], bd_d.ap().rearrange("(c p) -> p c", p=128))
            bdbe = pre.tile([128, AC], f32)
            nc.vector.tensor_add(bdbe[:], be_sb[:], bd_sb[:])

            dec_sb = pre.tile([BL, D], f32)
            nc.sync.dma_start(dec_sb[:], dec_d.ap())
            decT_sb = pre.tile([128, DC, BL], f32)
            for c in range(DC):
                ps_tp = prepsum.tile([128, BL], f32, tag="tp", bufs=2)
                nc.tensor.transpose(
                    ps_tp[:], dec_sb[:, ts(c, 128)], ident[0:BL, 0:BL]
                )
                nc.vector.tensor_copy(decT_sb[:, c, :], ps_tp[:])

            for ca in range(AC):
                ps_a2 = prepsum.tile([128, BL], f32, tag="tp", bufs=2)
                for cd in range(DC):
                    nc.tensor.matmul(
                        ps_a2[:],
                        wd_sb[:, cd, ts(ca, 128)],
                        decT_sb[:, cd, :],
                        start=(cd == 0),
                        stop=(cd == DC - 1),
                    )
                nc.scalar.add(att2T_sb[:, ca, :], ps_a2[:], bdbe[:, ca : ca + 1])

            nc.gpsimd.memset(alphaT_pad[:].bitcast(f32), 0.0)

        work = ctx.enter_context(tc.tile_pool(name="work", bufs=2))
        psum = ctx.enter_context(tc.tile_pool(name="psum", bufs=1, space="PSUM"))

        ctx_ps = psum.tile([BL, E4, 512], f32, tag="ctx", bufs=1)

        def r(ap):
            return ap.bitcast(f32r)

        def emit_loads(p):
            # SWDGE (gpsimd) spreads each big transfer across all 16 SDMA
            # engines; HWDGE rings would serialize on ~4 queues.
            b0, b1 = 2 * p, 2 * p + 1
            nat0 = work.tile([128, 2, E], f32r, tag="nat0", bufs=3, name="nat0")
            nat1 = work.tile([N1, 2, E], f32r, tag="nat1", bufs=3, name="nat1")
            for j, b in enumerate((b0, b1)):
                nc.gpsimd.dma_start(nat0[:, j, :], enc_d.ap()[b, 0:N0, :].bitcast(f32r))
                nc.gpsimd.dma_start(nat1[:, j, :], enc_d.ap()[b, N0:N, :].bitcast(f32r))
            return nat0, nat1

        def copy_op(i, out, in_):
            if i % 2 == 0:
                nc.vector.tensor_copy(out, in_)
            else:
                nc.scalar.copy(out, in_)

        def transpose_jobs(p, nats, encT):
            """Yield closures: 4 PE transposes + 1 evacuation copy per e-chunk.
            Interleaved between att1 matmuls so the PE always has REGULAR
            matmuls in flight (transpose-mode doesn't count as HAM-busy)."""
            nat0, nat1 = nats
            state = {}

            def t_job(ce, j, half):
                def go():
                    if ce not in state:
                        state[ce] = psum.tile(
                            [128, WV], f32r, tag="tp", bufs=2, name="ps_t"
                        )
                    ps_t = state[ce]
                    off = N * j
                    if half == 0:
                        nc.tensor.transpose(
                            ps_t[:, off : off + N0],
                            nat0[:, j, ts(ce, 128)],
                            ident_r[:, :],
                        )
                    else:
                        nc.tensor.transpose(
                            ps_t[:, off + N0 : off + N],
                            nat1[:, j, ts(ce, 128)],
                            ident_r[0:N1, 0:N1],
                        )
                return go

            def c_job(ce):
                def go():
                    copy_op(ce, encT[:, ce, :], state.pop(ce)[:])
                return go

            jobs = []
            for ce in range(EC):
                for j in range(2):
                    for half in range(2):
                        jobs.append(t_job(ce, j, half))
                jobs.append(c_job(ce))
            return jobs

        def emit_transposes(p, nats):
            encT = work.tile([128, EC, WV], f32r, tag="encT", bufs=2, name="encT")
            for job in transpose_jobs(p, nats, encT):
                job()
            return encT

        def emit_att1(p, encT, ca_list, jobs=None):
            b0, b1 = 2 * p, 2 * p + 1
            att = att_tiles[p % 2]
            nmm = len(ca_list) * EC
            k = 0
            jpos = 0
            for ca in ca_list:
                ps_a = psum.tile([128, WV], f32, tag="att1", bufs=2, name="ps_a")
                for ce in range(EC):
                    nc.tensor.matmul(
                        ps_a[:],
                        we_sb[:, ce, ts(ca, 128)],
                        encT[:, ce, :],
                        start=(ce == 0),
                        stop=(ce == EC - 1),
                    )
                    if jobs:
                        k += 1
                        jend = len(jobs) * k // nmm
                        while jpos < jend:
                            jobs[jpos]()
                            jpos += 1
                nc.scalar.activation(
                    att[:, ca, 0:N], ps_a[:, 0:N], TANH,
                    bias=att2T_sb[:, ca, b0 : b0 + 1],
                )
                nc.scalar.activation(
                    att[:, ca, N:WV], ps_a[:, N:WV], TANH,
                    bias=att2T_sb[:, ca, b1 : b1 + 1],
                )
            if jobs:
                while jpos < len(jobs):
                    jobs[jpos]()
                    jpos += 1

        def emit_scores_softmax(p):
            b0 = 2 * p
            att = att_tiles[p % 2]
            ps_s = psum.tile([128, WV], f32, tag="att1", bufs=2, name="ps_s")
            for ca in range(AC):
                nc.tensor.matmul(
                    ps_s[0:1, :],
                    wf_sb[:, ca : ca + 1],
                    att[:, ca, :],
                    start=(ca == 0),
                    stop=(ca == AC - 1),
                )
            sc_sb = work.tile([1, WV], f32, tag="sc", bufs=2, name="sc")
            nc.vector.tensor_copy(sc_sb[:], ps_s[0:1, :])

            mx = work.tile([1, 2], f32, tag="mx", bufs=2, name="mx")
            sm = work.tile([1, 2], f32, tag="sm", bufs=2, name="sm")
            rs = work.tile([1, 2], f32, tag="rs", bufs=2, name="rs")
            nc.vector.reduce_max(
                mx[:],
                sc_sb[:].rearrange("p (h w) -> p h w", h=2),
                axis=mybir.AxisListType.X,
                negate=True,
            )
            al = work.tile([1, WV], f32, tag="al", bufs=2, name="al")
            for h in range(2):
                nc.scalar.activation(
                    al[0:1, ts(h, N)], sc_sb[0:1, ts(h, N)], EXP,
                    bias=mx[0:1, h : h + 1],
                    accum_out=sm[0:1, h : h + 1],
                )
            nc.vector.reciprocal(rs[:], sm[:])
            for h in range(2):
                nc.vector.tensor_scalar_mul(
                    al[0:1, ts(h, N)], al[0:1, ts(h, N)], rs[0:1, h : h + 1]
                )
            nc.gpsimd.dma_start(
                alp_d.ap()[b0 : b0 + 2].rearrange("b n o -> o (b n)"), al[:]
            )
            return al

        def emit_alpha_ctx(p, al, nats):
            nat0, nat1 = nats
            b0, b1 = 2 * p, 2 * p + 1
            ps_al = psum.tile([128, WV], f32, tag="att1", bufs=2, name="ps_al")
            for j, b in enumerate((b0, b1)):
                off = N * j
                nc.tensor.transpose(
                    ps_al[:, 2 * j : 2 * j + 1],
                    al[0:1, off : off + N0],
                    ident[0:1, 0:1],
                )
                nc.tensor.transpose(
                    ps_al[0:N1, 2 * j + 1 : 2 * j + 2],
                    al[0:1, off + N0 : off + N],
                    ident[0:1, 0:1],
                )
                nc.vector.tensor_copy(
                    alphaT_pad[:, 2 * b, b : b + 1], ps_al[:, 2 * j : 2 * j + 1]
                )
                nc.vector.tensor_copy(
                    alphaT_pad[0:N1, 2 * b + 1, b : b + 1],
                    ps_al[0:N1, 2 * j + 1 : 2 * j + 2],
                )
            for j, b in enumerate((b0, b1)):
                for e4 in range(E4):
                    nc.tensor.matmul(
                        ctx_ps[:, e4, :],
                        r(alphaT_pad[:, 2 * b, :]),
                        nat0[:, j, ts(e4, 512)],
                        start=(p == 0 and j == 0),
                        stop=False,
                        skip_group_check=True,
                    )
                    nc.tensor.matmul(
                        ctx_ps[:, e4, :],
                        r(alphaT_pad[0:N1, 2 * b + 1, :]),
                        nat1[:, j, ts(e4, 512)],
                        start=False,
                        stop=(p == PAIRS - 1 and j == 1),
                        skip_group_check=True,
                    )

        att_tiles = [
            work.tile([128, AC, WV], f32r, tag="att", bufs=2, name=f"att{i}")
            for i in range(2)
        ]

        # Pipeline: loads 1 section ahead (SWDGE); per section:
        #   transposes(p) ; alphaT+ctx(p-1) ; att1(p) ; scores+softmax(p)
        # nat(p) lives load(p-1)..ctx(p+1) -> bufs=3. encT written and read
        # within the section (PE in-order) -> bufs=1.
        # Pipeline: loads one section ahead; transposes of pair p+1 are
        # interleaved between the att1 matmuls of pair p (2:1) so the PE
        # stream never has a transpose-only phase (HAM stays warm).
        nats = {0: emit_loads(0)}
        als = {}
        encTs = {0: emit_transposes(0, nats[0])}
        for p in range(PAIRS):
            if p + 1 < PAIRS:
                nats[p + 1] = emit_loads(p + 1)
            emit_att1(p, encTs[p], (0, 1))
            if p >= 1:
                emit_alpha_ctx(p - 1, als.pop(p - 1), nats.pop(p - 1))
            if p + 1 < PAIRS:
                encTs[p + 1] = work.tile(
                    [128, EC, WV], f32r, tag="encT", bufs=2, name="encT"
                )
                jobs = transpose_jobs(p + 1, nats[p + 1], encTs[p + 1])
            else:
                jobs = None
            emit_att1(p, encTs.pop(p), (2, 3), jobs=jobs)
            als[p] = emit_scores_softmax(p)
        emit_alpha_ctx(PAIRS - 1, als.pop(PAIRS - 1), nats.pop(PAIRS - 1))

        for e4 in range(E4):
            ctx_e = work.tile([BL, 512], f32, tag="ctxe", bufs=2, name="ctxe")
            nc.vector.tensor_copy(ctx_e[:], ctx_ps[:, e4, :])
            nc.gpsimd.dma_start(ctx_d.ap()[:, ts(e4, 512)], ctx_e[:])

    return nc


def _get_nc():
    if "nc" not in _CACHE:
        _CACHE["nc"] = build()
    return _CACHE["nc"]


def _run(in_maps, trace=False, tmpdir=None):
    from concourse.bass_utils import run_bass_kernel_spmd

    nc = _get_nc()
    return run_bass_kernel_spmd(
        nc, in_maps, core_ids=list(range(NCORES)), trace=trace, tmpdir=tmpdir
    )


def make_in_maps(encoder_out, decoder_hidden, We, be, Wd, bd, Wf, bf=None):
    enc = np.ascontiguousarray(np.asarray(encoder_out, dtype=np.float32))
    dec = np.ascontiguousarray(np.asarray(decoder_hidden, dtype=np.float32))
    We = np.ascontiguousarray(np.asarray(We, dtype=np.float32))
    be = np.ascontiguousarray(np.asarray(be, dtype=np.float32))
    Wd = np.ascontiguousarray(np.asarray(Wd, dtype=np.float32))
    bd = np.ascontiguousarray(np.asarray(bd, dtype=np.float32))
    Wf = np.ascontiguousarray(np.asarray(Wf, dtype=np.float32))
    in_maps = []
    for i in range(NCORES):
        sl = slice(i * BL, (i + 1) * BL)
        in_maps.append(
            dict(
                encoder_out=enc[sl],
                decoder_hidden=dec[sl],
                We=We,
                be=be,
                Wd=Wd,
                bd=bd,
                Wf=Wf,
            )
        )
    return in_maps


def gather(results):
    context = np.concatenate([results[i]["context"] for i in range(NCORES)], axis=0)
    alpha = np.concatenate([results[i]["alpha"] for i in range(NCORES)], axis=0)
    return context, alpha


def kernel(encoder_out, decoder_hidden, We, be, Wd, bd, Wf, bf):
    in_maps = make_in_maps(encoder_out, decoder_hidden, We, be, Wd, bd, Wf, bf)
    res = _run(in_maps, trace=False)
    _CACHE["last_results"] = res
    return gather(res.results)


# revision 43
# speedup vs baseline: 1.6712x; 1.0638x over previous
"""Bass/Trainium2 kernel for Bahdanau (additive) attention, 8-core data-parallel.

Reference computation (per batch b):
    att1 = enc[b] @ We                    # [N, A]
    att2 = dec[b] @ Wd + bd               # [A]
    att  = tanh(att1 + att2 + be)         # [N, A]
    s    = att @ Wf (+ bf)                # [N]   (bf dropped: softmax-invariant)
    alpha= softmax(s)                     # [N]
    ctx  = sum_n alpha[n] * enc[b, n]     # [E]

Sharding: batch 128 -> 8 cores x 16. Weights replicated. No collectives.

Per-core design (B_loc=16, processed in 8 pairs):
  - enc is loaded f32 (split DMAs across queues), converted to bf16 on
    DVE/ACT, and transposed by the DMA xbar (dma_start transpose=True,
    bf16) straight into SBUF - the PE does no data transposes.
  - att1 computed transposed ([A_chunk=128, n-cols]) in bf16:
    lhsT = We blocks (natural), moving = encT [128, 416] per pair.
  - att2+be add fused into tanh on ScalarE as a per-partition bias.
  - scores via PE: lhsT = Wf chunk [128,1], moving = tanh output.
  - softmax on one partition per pair (reduce_max neg / exp+accum / recip).
  - context via PE with a zero-padded block-diagonal alphaT (each batch's
    alpha column at 256-aligned row offsets) against the bf16 enc tiles,
    all 16 batches accumulated into 4 persistent PSUM banks [16, 512].
  - 3-deep software pipeline: loads(p+2) / convert+xbar(p+1) /
    att1+scores(p) with alpha-placement+context of p-1 emitted mid-att1
    so the PE instruction stream never waits on the softmax chain.
"""

import numpy as np

B, N, E = 128, 196, 2048
D, A = 512, 512
NCORES = 8
BL = B // NCORES            # 16 batches per core
PAIRS = BL // 2             # 8
N0 = 128
N1 = N - N0                 # 68
NP = 80                     # N1 padded to a multiple of XBAR_TILE_SRC_ROWS (16)
PW = N0 + NP                # 208 cols per batch in the transposed layout
W = 2 * PW                  # 416 cols per pair
EC = E // 128               # 16
AC = A // 128               # 4
E4 = E // 512               # 4
DC = D // 128               # 4

_CACHE = {}


def _patch_tile_tail_drain(tile):
    """This walrus build rejects >1 sem-wait per instruction. Split extra
    waits onto single-wait NOPs committed just before the instruction, and
    do the same for the TileContext tail drain."""
    import concourse.mybir as mybir
    from concourse.vector_clock import ScopedClock

    if getattr(tile.TileContext, "_tail_drain_patched", False):
        return

    orig_commit = tile.TileContext._commit_instruction

    def _commit_instruction(self, inst, lazy_reg_writes=True):
        si = getattr(inst, "sync_info", None)
        engine = getattr(inst, "engine", None)
        if (
            si is not None
            and si.on_wait
            and len(si.on_wait) > 1
            and engine is not None
            and engine != mybir.EngineType.Unassigned
            and type(inst).__name__.startswith("Inst")
        ):
            waits = list(si.on_wait)
            for w in waits[:-1]:
                noop = mybir.InstNoOp(
                    name=self.nc.get_next_instruction_name(),
                    sync_info=mybir.SyncInfo(on_wait=[w], on_update=[]),
                    bass_nofuse=True,
                    engine=engine,
                )
                orig_commit(self, noop, lazy_reg_writes=False)
            inst.sync_info = mybir.SyncInfo(
                on_wait=[waits[-1]], on_update=list(si.on_update or [])
            )
        return orig_commit(self, inst, lazy_reg_writes)

    tile.TileContext._commit_instruction = _commit_instruction

    def _drain_and_barrier(self, tick_clock, wait_clock):
        nc = self.nc
        drain_inst = nc.sync.drain()
        wait_clock.add_sem_waits(
            drain_inst.ins, ScopedClock({None: tick_clock.global_clock})
        )
        si = drain_inst.ins.sync_info
        waits = list(si.on_wait or []) if si is not None else []
        if len(waits) > 1:
            drain_inst.ins.sync_info = mybir.SyncInfo(
                on_wait=waits[:1], on_update=list(si.on_update or [])
            )
            for w in waits[1:]:
                d = nc.sync.drain()
                d.ins.sync_info = mybir.SyncInfo(on_wait=[w], on_update=[])
        nc.all_engine_barrier()
        assert self.sems is not None
        popped = nc._tile_sem_poison_stack.pop()
        assert popped is self._sem_poison
        nc.clear_and_free_semaphores(list(self.sems.allocated().values()))
        nc.all_engine_barrier()

    tile.TileContext._drain_and_barrier = _drain_and_barrier
    tile.TileContext._tail_drain_patched = True


def build():
    from contextlib import ExitStack

    import concourse.bass as bass
    import concourse.tile as tile
    from concourse import mybir
    from concourse.bass import ts
    from concourse.masks import make_identity

    _patch_tile_tail_drain(tile)

    f32 = mybir.dt.float32
    f32r = mybir.dt.float32r
    TANH = mybir.ActivationFunctionType.Tanh
    EXP = mybir.ActivationFunctionType.Exp

    nc = bass.Bass(trn_type="TRN2", target_bir_lowering=False, debug=False)
    enc_d = nc.dram_tensor("encoder_out", [BL, N, E], f32, kind="ExternalInput")
    dec_d = nc.dram_tensor("decoder_hidden", [BL, D], f32, kind="ExternalInput")
    we_d = nc.dram_tensor("We", [E, A], f32, kind="ExternalInput")
    be_d = nc.dram_tensor("be", [A], f32, kind="ExternalInput")
    wd_d = nc.dram_tensor("Wd", [D, A], f32, kind="ExternalInput")
    bd_d = nc.dram_tensor("bd", [A], f32, kind="ExternalInput")
    wf_d = nc.dram_tensor("Wf", [A, 1], f32, kind="ExternalInput")
    ctx_d = nc.dram_tensor("context", [BL, E], f32, kind="ExternalOutput")
    alp_d = nc.dram_tensor("alpha", [BL, N, 1], f32, kind="ExternalOutput")

    WV = 392  # pair width in the transposed layout (2 * N)

    with tile.TileContext(nc) as tc, ExitStack() as ctx:
        consts = ctx.enter_context(tc.tile_pool(name="consts", bufs=1))

        ident = consts.tile([128, 128], f32)
        make_identity(nc, ident[:])
        ident_r = consts.tile([128, 128], f32r)
        nc.vector.tensor_copy(ident_r[:], ident[:])

        we_sb = consts.tile([128, EC, A], f32r)
        for c in range(EC):
            nc.sync.dma_start(we_sb[:, c, :], we_d.ap()[ts(c, 128), :].bitcast(f32r))
        wf_sb = consts.tile([128, AC], f32r)
        nc.sync.dma_start(
            wf_sb[:], wf_d.ap().rearrange("(c p) o -> p (c o)", p=128).bitcast(f32r)
        )
        att2T_sb = consts.tile([128, AC, BL], f32)
        alphaT_pad = consts.tile([128, 2 * BL, BL], f32r)

        with tc.tile_pool(name="pre", bufs=1) as pre, tc.tile_pool(
            name="prepsum", bufs=1, space="PSUM"
        ) as prepsum:
            wd_sb = pre.tile([128, DC, A], f32)
            for c in range(DC):
                nc.sync.dma_start(wd_sb[:, c, :], wd_d.ap()[ts(c, 128), :])
            be_sb = pre.tile([128, AC], f32)
            bd_sb = pre.tile([128, AC], f32)
            nc.sync.dma_start(be_sb[:], be_d.ap().rearrange("(c p) -> p c", p=128))
            nc.sync.dma_start(bd_sb[:], bd_d.ap().rearrange("(c p) -> p c", p=128))
            bdbe = pre.tile([128, AC], f32)
            nc.vector.tensor_add(bdbe[:], be_sb[:], bd_sb[:])

            dec_sb = pre.tile([BL, D], f32)
            nc.sync.dma_start(dec_sb[:], dec_d.ap())
            decT_sb = pre.tile([128, DC, BL], f32)
            for c in range(DC):
                ps_tp = prepsum.tile([128, BL], f32, tag="tp", bufs=2)
                nc.tensor.transpose(
                    ps_tp[:], dec_sb[:, ts(c, 128)], ident[0:BL, 0:BL]
                )
                nc.vector.tensor_copy(decT_sb[:, c, :], ps_tp[:])

            for ca in range(AC):
                ps_a2 = prepsum.tile([128, BL], f32, tag="tp", bufs=2)
                for cd in range(DC):
                    nc.tensor.matmul(
                        ps_a2[:],
                        wd_sb[:, cd, ts(ca, 128)],
                        decT_sb[:, cd, :],
                        start=(cd == 0),
                        stop=(cd == DC - 1),
                    )
                nc.scalar.add(att2T_sb[:, ca, :], ps_a2[:], bdbe[:, ca : ca + 1])

            nc.gpsimd.memset(alphaT_pad[:].bitcast(f32), 0.0)

        work = ctx.enter_context(tc.tile_pool(name="work", bufs=2))
        psum = ctx.enter_context(tc.tile_pool(name="psum", bufs=1, space="PSUM"))

        ctx_ps = psum.tile([BL, E4, 512], f32, tag="ctx", bufs=1)

        def r(ap):
            return ap.bitcast(f32r)

        def emit_loads(p):
            # SWDGE (gpsimd) spreads each big transfer across all 16 SDMA
            # engines; HWDGE rings would serialize on ~4 queues.
            b0, b1 = 2 * p, 2 * p + 1
            nat0 = work.tile([128, 2, E], f32r, tag="nat0", bufs=3, name="nat0")
            nat1 = work.tile([N1, 2, E], f32r, tag="nat1", bufs=3, name="nat1")
            for j, b in enumerate((b0, b1)):
                for k in range(4):
                    nc.gpsimd.dma_start(
                        nat0[:, j, ts(k, 512)],
                        enc_d.ap()[b, 0:N0, ts(k, 512)].bitcast(f32r),
                    )
                    nc.gpsimd.dma_start(
                        nat1[:, j, ts(k, 512)],
                        enc_d.ap()[b, N0:N, ts(k, 512)].bitcast(f32r),
                    )
            return nat0, nat1

        def copy_op(i, out, in_):
            if i % 2 == 0:
                nc.vector.tensor_copy(out, in_)
            else:
                nc.scalar.copy(out, in_)

        def transpose_jobs(p, nats, encT):
            """Yield closures: 4 PE transposes + 1 evacuation copy per e-chunk.
            Interleaved between att1 matmuls so the PE always has REGULAR
            matmuls in flight (transpose-mode doesn't count as HAM-busy)."""
            nat0, nat1 = nats
            state = {}

            def t_job(ce, j, half):
                def go():
                    if ce not in state:
                        state[ce] = psum.tile(
                            [128, WV], f32r, tag="tp", bufs=2, name="ps_t"
                        )
                    ps_t = state[ce]
                    off = N * j
                    if half == 0:
                        nc.tensor.transpose(
                            ps_t[:, off : off + N0],
                            nat0[:, j, ts(ce, 128)],
                            ident_r[:, :],
                        )
                    else:
                        nc.tensor.transpose(
                            ps_t[:, off + N0 : off + N],
                            nat1[:, j, ts(ce, 128)],
                            ident_r[0:N1, 0:N1],
                        )
                return go

            def c_job(ce):
                def go():
                    copy_op(ce, encT[:, ce, :], state.pop(ce)[:])
                return go

            jobs = []
            for ce in range(EC):
                for j in range(2):
                    for half in range(2):
                        jobs.append(t_job(ce, j, half))
                jobs.append(c_job(ce))
            return jobs

        def emit_transposes(p, nats):
            encT = work.tile([128, EC, WV], f32r, tag="encT", bufs=2, name="encT")
            for job in transpose_jobs(p, nats, encT):
                job()
            return encT

        def emit_att1(p, encT, ca_list, jobs=None):
            b0, b1 = 2 * p, 2 * p + 1
            att = att_tiles[p % 2]
            nmm = len(ca_list) * EC
            k = 0
            jpos = 0
            for ca in ca_list:
                ps_a = psum.tile([128, WV], f32, tag="att1", bufs=2, name="ps_a")
                for ce in range(EC):
                    nc.tensor.matmul(
                        ps_a[:],
                        we_sb[:, ce, ts(ca, 128)],
                        encT[:, ce, :],
                        start=(ce == 0),
                        stop=(ce == EC - 1),
                    )
                    if jobs:
                        k += 1
                        jend = len(jobs) * k // nmm
                        while jpos < jend:
                            jobs[jpos]()
                            jpos += 1
                nc.scalar.activation(
                    att[:, ca, 0:N], ps_a[:, 0:N], TANH,
                    bias=att2T_sb[:, ca, b0 : b0 + 1],
                )
                nc.scalar.activation(
                    att[:, ca, N:WV], ps_a[:, N:WV], TANH,
                    bias=att2T_sb[:, ca, b1 : b1 + 1],
                )
            if jobs:
                while jpos < len(jobs):
                    jobs[jpos]()
                    jpos += 1

        def emit_scores_softmax(p):
            b0 = 2 * p
            att = att_tiles[p % 2]
            ps_s = psum.tile([128, WV], f32, tag="att1", bufs=2, name="ps_s")
            for ca in range(AC):
                nc.tensor.matmul(
                    ps_s[0:1, :],
                    wf_sb[:, ca : ca + 1],
                    att[:, ca, :],
                    start=(ca == 0),
                    stop=(ca == AC - 1),
                )
            sc_sb = work.tile([1, WV], f32, tag="sc", bufs=2, name="sc")
            nc.vector.tensor_copy(sc_sb[:], ps_s[0:1, :])

            mx = work.tile([1, 2], f32, tag="mx", bufs=2, name="mx")
            sm = work.tile([1, 2], f32, tag="sm", bufs=2, name="sm")
            rs = work.tile([1, 2], f32, tag="rs", bufs=2, name="rs")
            nc.vector.reduce_max(
                mx[:],
                sc_sb[:].rearrange("p (h w) -> p h w", h=2),
                axis=mybir.AxisListType.X,
                negate=True,
            )
            al = work.tile([1, WV], f32, tag="al", bufs=2, name="al")
            for h in range(2):
                nc.scalar.activation(
                    al[0:1, ts(h, N)], sc_sb[0:1, ts(h, N)], EXP,
                    bias=mx[0:1, h : h + 1],
                    accum_out=sm[0:1, h : h + 1],
                )
            nc.vector.reciprocal(rs[:], sm[:])
            for h in range(2):
                nc.vector.tensor_scalar_mul(
                    al[0:1, ts(h, N)], al[0:1, ts(h, N)], rs[0:1, h : h + 1]
                )
            nc.gpsimd.dma_start(
                alp_d.ap()[b0 : b0 + 2].rearrange("b n o -> o (b n)"), al[:]
            )
            return al

        def emit_alpha_ctx(p, al, nats):
            nat0, nat1 = nats
            b0, b1 = 2 * p, 2 * p + 1
            ps_al = psum.tile([128, WV], f32, tag="att1", bufs=2, name="ps_al")
            for j, b in enumerate((b0, b1)):
                off = N * j
                nc.tensor.transpose(
                    ps_al[:, 2 * j : 2 * j + 1],
                    al[0:1, off : off + N0],
                    ident[0:1, 0:1],
                )
                nc.tensor.transpose(
                    ps_al[0:N1, 2 * j + 1 : 2 * j + 2],
                    al[0:1, off + N0 : off + N],
                    ident[0:1, 0:1],
                )
                nc.vector.tensor_copy(
                    alphaT_pad[:, 2 * b, b : b + 1], ps_al[:, 2 * j : 2 * j + 1]
                )
                nc.vector.tensor_copy(
                    alphaT_pad[0:N1, 2 * b + 1, b : b + 1],
                    ps_al[0:N1, 2 * j + 1 : 2 * j + 2],
                )
            for j, b in enumerate((b0, b1)):
                for e4 in range(E4):
                    nc.tensor.matmul(
                        ctx_ps[:, e4, :],
                        r(alphaT_pad[:, 2 * b, :]),
                        nat0[:, j, ts(e4, 512)],
                        start=(p == 0 and j == 0),
                        stop=False,
                        skip_group_check=True,
                    )
                    nc.tensor.matmul(
                        ctx_ps[:, e4, :],
                        r(alphaT_pad[0:N1, 2 * b + 1, :]),
                        nat1[:, j, ts(e4, 512)],
                        start=False,
                        stop=(p == PAIRS - 1 and j == 1),
                        skip_group_check=True,
                    )

        att_tiles = [
            work.tile([128, AC, WV], f32r, tag="att", bufs=2, name=f"att{i}")
            for i in range(2)
        ]

        # Pipeline: loads 1 section ahead (SWDGE); per section:
        #   transposes(p) ; alphaT+ctx(p-1) ; att1(p) ; scores+softmax(p)
        # nat(p) lives load(p-1)..ctx(p+1) -> bufs=3. encT written and read
        # within the section (PE in-order) -> bufs=1.
        # Pipeline: loads one section ahead; transposes of pair p+1 are
        # interleaved between the att1 matmuls of pair p (2:1) so the PE
        # stream never has a transpose-only phase (HAM stays warm).
        nats = {0: emit_loads(0)}
        als = {}
        encTs = {0: emit_transposes(0, nats[0])}
        for p in range(PAIRS):
            if p + 1 < PAIRS:
                nats[p + 1] = emit_loads(p + 1)
            emit_att1(p, encTs[p], (0, 1))
            if p >= 1:
                emit_alpha_ctx(p - 1, als.pop(p - 1), nats.pop(p - 1))
            if p + 1 < PAIRS:
                encTs[p + 1] = work.tile(
                    [128, EC, WV], f32r, tag="encT", bufs=2, name="encT"
                )
                jobs = transpose_jobs(p + 1, nats[p + 1], encTs[p + 1])
            else:
                jobs = None
            emit_att1(p, encTs.pop(p), (2, 3), jobs=jobs)
            als[p] = emit_scores_softmax(p)
        emit_alpha_ctx(PAIRS - 1, als.pop(PAIRS - 1), nats.pop(PAIRS - 1))

        for e4 in range(E4):
            ctx_e = work.tile([BL, 512], f32, tag="ctxe", bufs=2, name="ctxe")
            nc.vector.tensor_copy(ctx_e[:], ctx_ps[:, e4, :])
            nc.gpsimd.dma_start(ctx_d.ap()[:, ts(e4, 512)], ctx_e[:])

    return nc


def _get_nc():
    if "nc" not in _CACHE:
        _CACHE["nc"] = build()
    return _CACHE["nc"]


def _run(in_maps, trace=False, tmpdir=None):
    from concourse.bass_utils import run_bass_kernel_spmd

    nc = _get_nc()
    return run_bass_kernel_spmd(
        nc, in_maps, core_ids=list(range(NCORES)), trace=trace, tmpdir=tmpdir
    )


def make_in_maps(encoder_out, decoder_hidden, We, be, Wd, bd, Wf, bf=None):
    enc = np.ascontiguousarray(np.asarray(encoder_out, dtype=np.float32))
    dec = np.ascontiguousarray(np.asarray(decoder_hidden, dtype=np.float32))
    We = np.ascontiguousarray(np.asarray(We, dtype=np.float32))
    be = np.ascontiguousarray(np.asarray(be, dtype=np.float32))
    Wd = np.ascontiguousarray(np.asarray(Wd, dtype=np.float32))
    bd = np.ascontiguousarray(np.asarray(bd, dtype=np.float32))
    Wf = np.ascontiguousarray(np.asarray(Wf, dtype=np.float32))
    in_maps = []
    for i in range(NCORES):
        sl = slice(i * BL, (i + 1) * BL)
        in_maps.append(
            dict(
                encoder_out=enc[sl],
                decoder_hidden=dec[sl],
                We=We,
                be=be,
                Wd=Wd,
                bd=bd,
                Wf=Wf,
            )
        )
    return in_maps


def gather(results):
    context = np.concatenate([results[i]["context"] for i in range(NCORES)], axis=0)
    alpha = np.concatenate([results[i]["alpha"] for i in range(NCORES)], axis=0)
    return context, alpha


def kernel(encoder_out, decoder_hidden, We, be, Wd, bd, Wf, bf):
    in_maps = make_in_maps(encoder_out, decoder_hidden, We, be, Wd, bd, Wf, bf)
    res = _run(in_maps, trace=False)
    _CACHE["last_results"] = res
    return gather(res.results)


# revision 48
# speedup vs baseline: 1.7215x; 1.0301x over previous
"""Bass/Trainium2 kernel for Bahdanau (additive) attention, 8-core data-parallel.

Reference computation (per batch b):
    att1 = enc[b] @ We                    # [N, A]
    att2 = dec[b] @ Wd + bd               # [A]
    att  = tanh(att1 + att2 + be)         # [N, A]
    s    = att @ Wf (+ bf)                # [N]   (bf dropped: softmax-invariant)
    alpha= softmax(s)                     # [N]
    ctx  = sum_n alpha[n] * enc[b, n]     # [E]

Sharding: batch 128 -> 8 cores x 16. Weights replicated. No collectives.

Per-core design (B_loc=16, processed in 8 pairs):
  - enc is loaded f32 (split DMAs across queues), converted to bf16 on
    DVE/ACT, and transposed by the DMA xbar (dma_start transpose=True,
    bf16) straight into SBUF - the PE does no data transposes.
  - att1 computed transposed ([A_chunk=128, n-cols]) in bf16:
    lhsT = We blocks (natural), moving = encT [128, 416] per pair.
  - att2+be add fused into tanh on ScalarE as a per-partition bias.
  - scores via PE: lhsT = Wf chunk [128,1], moving = tanh output.
  - softmax on one partition per pair (reduce_max neg / exp+accum / recip).
  - context via PE with a zero-padded block-diagonal alphaT (each batch's
    alpha column at 256-aligned row offsets) against the bf16 enc tiles,
    all 16 batches accumulated into 4 persistent PSUM banks [16, 512].
  - 3-deep software pipeline: loads(p+2) / convert+xbar(p+1) /
    att1+scores(p) with alpha-placement+context of p-1 emitted mid-att1
    so the PE instruction stream never waits on the softmax chain.
"""

import numpy as np

B, N, E = 128, 196, 2048
D, A = 512, 512
NCORES = 8
BL = B // NCORES            # 16 batches per core
PAIRS = BL // 2             # 8
N0 = 128
N1 = N - N0                 # 68
NP = 80                     # N1 padded to a multiple of XBAR_TILE_SRC_ROWS (16)
PW = N0 + NP                # 208 cols per batch in the transposed layout
W = 2 * PW                  # 416 cols per pair
EC = E // 128               # 16
AC = A // 128               # 4
E4 = E // 512               # 4
DC = D // 128               # 4

_CACHE = {}


def _patch_tile_tail_drain(tile):
    """This walrus build rejects >1 sem-wait per instruction. Split extra
    waits onto single-wait NOPs committed just before the instruction, and
    do the same for the TileContext tail drain."""
    import concourse.mybir as mybir
    from concourse.vector_clock import ScopedClock

    if getattr(tile.TileContext, "_tail_drain_patched", False):
        return

    orig_commit = tile.TileContext._commit_instruction

    def _commit_instruction(self, inst, lazy_reg_writes=True):
        si = getattr(inst, "sync_info", None)
        engine = getattr(inst, "engine", None)
        if (
            si is not None
            and si.on_wait
            and len(si.on_wait) > 1
            and engine is not None
            and engine != mybir.EngineType.Unassigned
            and type(inst).__name__.startswith("Inst")
        ):
            waits = list(si.on_wait)
            for w in waits[:-1]:
                noop = mybir.InstNoOp(
                    name=self.nc.get_next_instruction_name(),
                    sync_info=mybir.SyncInfo(on_wait=[w], on_update=[]),
                    bass_nofuse=True,
                    engine=engine,
                )
                orig_commit(self, noop, lazy_reg_writes=False)
            inst.sync_info = mybir.SyncInfo(
                on_wait=[waits[-1]], on_update=list(si.on_update or [])
            )
        return orig_commit(self, inst, lazy_reg_writes)

    tile.TileContext._commit_instruction = _commit_instruction

    def _drain_and_barrier(self, tick_clock, wait_clock):
        nc = self.nc
        drain_inst = nc.sync.drain()
        wait_clock.add_sem_waits(
            drain_inst.ins, ScopedClock({None: tick_clock.global_clock})
        )
        si = drain_inst.ins.sync_info
        waits = list(si.on_wait or []) if si is not None else []
        if len(waits) > 1:
            drain_inst.ins.sync_info = mybir.SyncInfo(
                on_wait=waits[:1], on_update=list(si.on_update or [])
            )
            for w in waits[1:]:
                d = nc.sync.drain()
                d.ins.sync_info = mybir.SyncInfo(on_wait=[w], on_update=[])
        nc.all_engine_barrier()
        assert self.sems is not None
        popped = nc._tile_sem_poison_stack.pop()
        assert popped is self._sem_poison
        nc.clear_and_free_semaphores(list(self.sems.allocated().values()))
        nc.all_engine_barrier()

    tile.TileContext._drain_and_barrier = _drain_and_barrier
    tile.TileContext._tail_drain_patched = True


def build():
    from contextlib import ExitStack

    import concourse.bass as bass
    import concourse.tile as tile
    from concourse import mybir
    from concourse.bass import ts
    from concourse.masks import make_identity

    _patch_tile_tail_drain(tile)

    f32 = mybir.dt.float32
    f32r = mybir.dt.float32r
    TANH = mybir.ActivationFunctionType.Tanh
    EXP = mybir.ActivationFunctionType.Exp

    nc = bass.Bass(trn_type="TRN2", target_bir_lowering=False, debug=False)
    enc_d = nc.dram_tensor("encoder_out", [BL, N, E], f32, kind="ExternalInput")
    dec_d = nc.dram_tensor("decoder_hidden", [BL, D], f32, kind="ExternalInput")
    we_d = nc.dram_tensor("We", [E, A], f32, kind="ExternalInput")
    be_d = nc.dram_tensor("be", [A], f32, kind="ExternalInput")
    wd_d = nc.dram_tensor("Wd", [D, A], f32, kind="ExternalInput")
    bd_d = nc.dram_tensor("bd", [A], f32, kind="ExternalInput")
    wf_d = nc.dram_tensor("Wf", [A, 1], f32, kind="ExternalInput")
    ctx_d = nc.dram_tensor("context", [BL, E], f32, kind="ExternalOutput")
    alp_d = nc.dram_tensor("alpha", [BL, N, 1], f32, kind="ExternalOutput")

    WV = 392  # pair width in the transposed layout (2 * N)

    with tile.TileContext(nc) as tc, ExitStack() as ctx:
        consts = ctx.enter_context(tc.tile_pool(name="consts", bufs=1))

        ident = consts.tile([128, 128], f32)
        make_identity(nc, ident[:])
        ident_r = consts.tile([128, 128], f32r)
        nc.vector.tensor_copy(ident_r[:], ident[:])

        att2T_sb = consts.tile([128, AC, BL], f32)
        alphaT_pad = consts.tile([128, 2 * BL, BL], f32r)

        # open the main pools early so pair-0's enc loads are the first DMAs
        work = ctx.enter_context(tc.tile_pool(name="work", bufs=2))
        psum = ctx.enter_context(tc.tile_pool(name="psum", bufs=1, space="PSUM"))

        def emit_loads(p):
            # SWDGE (gpsimd) spreads each transfer across all 16 SDMA
            # engines; HWDGE rings would serialize on ~4 queues.
            b0, b1 = 2 * p, 2 * p + 1
            nat0 = work.tile([128, 2, E], f32r, tag="nat0", bufs=3, name="nat0")
            nat1 = work.tile([N1, 2, E], f32r, tag="nat1", bufs=3, name="nat1")
            for j, b in enumerate((b0, b1)):
                for k in range(4):
                    nc.gpsimd.dma_start(
                        nat0[:, j, ts(k, 512)],
                        enc_d.ap()[b, 0:N0, ts(k, 512)].bitcast(f32r),
                    )
                    nc.gpsimd.dma_start(
                        nat1[:, j, ts(k, 512)],
                        enc_d.ap()[b, N0:N, ts(k, 512)].bitcast(f32r),
                    )
            return nat0, nat1

        first_nats = emit_loads(0)

        we_sb = consts.tile([128, EC, A], f32r)
        for c in range(EC):
            nc.gpsimd.dma_start(we_sb[:, c, :], we_d.ap()[ts(c, 128), :].bitcast(f32r))
        wf_sb = consts.tile([128, AC], f32r)
        nc.sync.dma_start(
            wf_sb[:], wf_d.ap().rearrange("(c p) o -> p (c o)", p=128).bitcast(f32r)
        )

        with tc.tile_pool(name="pre", bufs=1) as pre:
            wd_sb = pre.tile([128, DC, A], f32)
            for c in range(DC):
                nc.sync.dma_start(wd_sb[:, c, :], wd_d.ap()[ts(c, 128), :])
            be_sb = pre.tile([128, AC], f32)
            bd_sb = pre.tile([128, AC], f32)
            nc.sync.dma_start(be_sb[:], be_d.ap().rearrange("(c p) -> p c", p=128))
            nc.sync.dma_start(bd_sb[:], bd_d.ap().rearrange("(c p) -> p c", p=128))
            bdbe = pre.tile([128, AC], f32)
            nc.vector.tensor_add(bdbe[:], be_sb[:], bd_sb[:])

            dec_sb = pre.tile([BL, D], f32)
            nc.sync.dma_start(dec_sb[:], dec_d.ap())
            decT_sb = pre.tile([128, DC, BL], f32)
            for c in range(DC):
                ps_tp = psum.tile([128, WV], f32, tag="tp", bufs=2, name="ps_tp")
                nc.tensor.transpose(
                    ps_tp[:, 0:BL], dec_sb[:, ts(c, 128)], ident[0:BL, 0:BL]
                )
                nc.vector.tensor_copy(decT_sb[:, c, :], ps_tp[:, 0:BL])

            for ca in range(AC):
                ps_a2 = psum.tile([128, WV], f32, tag="tp", bufs=2, name="ps_a2")
                for cd in range(DC):
                    nc.tensor.matmul(
                        ps_a2[:, 0:BL],
                        wd_sb[:, cd, ts(ca, 128)],
                        decT_sb[:, cd, :],
                        start=(cd == 0),
                        stop=(cd == DC - 1),
                    )
                nc.scalar.add(att2T_sb[:, ca, :], ps_a2[:, 0:BL], bdbe[:, ca : ca + 1])

            nc.gpsimd.memset(alphaT_pad[:].bitcast(f32), 0.0)

        ctx_ps = psum.tile([BL, E4, 512], f32, tag="ctx", bufs=1)

        def r(ap):
            return ap.bitcast(f32r)

        def copy_op(i, out, in_):
            if i % 2 == 0:
                nc.vector.tensor_copy(out, in_)
            else:
                nc.scalar.copy(out, in_)

        def transpose_jobs(p, nats, encT):
            """Yield closures: 4 PE transposes + 1 evacuation copy per e-chunk.
            Interleaved between att1 matmuls so the PE always has REGULAR
            matmuls in flight (transpose-mode doesn't count as HAM-busy)."""
            nat0, nat1 = nats
            state = {}

            def t_job(ce, j, half):
                def go():
                    if ce not in state:
                        state[ce] = psum.tile(
                            [128, WV], f32r, tag="tp", bufs=2, name="ps_t"
                        )
                    ps_t = state[ce]
                    off = N * j
                    if half == 0:
                        nc.tensor.transpose(
                            ps_t[:, off : off + N0],
                            nat0[:, j, ts(ce, 128)],
                            ident_r[:, :],
                        )
                    else:
                        nc.tensor.transpose(
                            ps_t[:, off + N0 : off + N],
                            nat1[:, j, ts(ce, 128)],
                            ident_r[0:N1, 0:N1],
                        )
                return go

            def c_job(ce):
                def go():
                    copy_op(ce, encT[:, ce, :], state.pop(ce)[:])
                return go

            jobs = []
            for ce in range(EC):
                for j in range(2):
                    for half in range(2):
                        jobs.append(t_job(ce, j, half))
                jobs.append(c_job(ce))
            return jobs

        def emit_transposes(p, nats):
            encT = work.tile([128, EC, WV], f32r, tag="encT", bufs=2, name="encT")
            for job in transpose_jobs(p, nats, encT):
                job()
            return encT

        def emit_att1(p, encT, ca_list, jobs=None):
            b0, b1 = 2 * p, 2 * p + 1
            att = att_tiles[p % 2]
            nmm = len(ca_list) * EC
            k = 0
            jpos = 0
            for ca in ca_list:
                ps_a = psum.tile([128, WV], f32, tag="att1", bufs=2, name="ps_a")
                for ce in range(EC):
                    nc.tensor.matmul(
                        ps_a[:],
                        we_sb[:, ce, ts(ca, 128)],
                        encT[:, ce, :],
                        start=(ce == 0),
                        stop=(ce == EC - 1),
                    )
                    if jobs:
                        k += 1
                        jend = len(jobs) * k // nmm
                        while jpos < jend:
                            jobs[jpos]()
                            jpos += 1
                nc.scalar.activation(
                    att[:, ca, 0:N], ps_a[:, 0:N], TANH,
                    bias=att2T_sb[:, ca, b0 : b0 + 1],
                )
                nc.scalar.activation(
                    att[:, ca, N:WV], ps_a[:, N:WV], TANH,
                    bias=att2T_sb[:, ca, b1 : b1 + 1],
                )
            if jobs:
                while jpos < len(jobs):
                    jobs[jpos]()
                    jpos += 1

        def emit_scores_softmax(p):
            b0 = 2 * p
            att = att_tiles[p % 2]
            ps_s = psum.tile([128, WV], f32, tag="att1", bufs=2, name="ps_s")
            for ca in range(AC):
                nc.tensor.matmul(
                    ps_s[0:1, :],
                    wf_sb[:, ca : ca + 1],
                    att[:, ca, :],
                    start=(ca == 0),
                    stop=(ca == AC - 1),
                )
            sc_sb = work.tile([1, WV], f32, tag="sc", bufs=2, name="sc")
            nc.vector.tensor_copy(sc_sb[:], ps_s[0:1, :])

            mx = work.tile([1, 2], f32, tag="mx", bufs=2, name="mx")
            sm = work.tile([1, 2], f32, tag="sm", bufs=2, name="sm")
            rs = work.tile([1, 2], f32, tag="rs", bufs=2, name="rs")
            nc.vector.reduce_max(
                mx[:],
                sc_sb[:].rearrange("p (h w) -> p h w", h=2),
                axis=mybir.AxisListType.X,
                negate=True,
            )
            al = work.tile([1, WV], f32, tag="al", bufs=2, name="al")
            for h in range(2):
                nc.scalar.activation(
                    al[0:1, ts(h, N)], sc_sb[0:1, ts(h, N)], EXP,
                    bias=mx[0:1, h : h + 1],
                    accum_out=sm[0:1, h : h + 1],
                )
            nc.vector.reciprocal(rs[:], sm[:])
            for h in range(2):
                nc.vector.tensor_scalar_mul(
                    al[0:1, ts(h, N)], al[0:1, ts(h, N)], rs[0:1, h : h + 1]
                )
            nc.gpsimd.dma_start(
                alp_d.ap()[b0 : b0 + 2].rearrange("b n o -> o (b n)"), al[:]
            )
            return al

        def emit_alpha_ctx(p, al, nats):
            nat0, nat1 = nats
            b0, b1 = 2 * p, 2 * p + 1
            ps_al = psum.tile([128, WV], f32, tag="att1", bufs=2, name="ps_al")
            for j, b in enumerate((b0, b1)):
                off = N * j
                nc.tensor.transpose(
                    ps_al[:, 2 * j : 2 * j + 1],
                    al[0:1, off : off + N0],
                    ident[0:1, 0:1],
                )
                nc.tensor.transpose(
                    ps_al[0:N1, 2 * j + 1 : 2 * j + 2],
                    al[0:1, off + N0 : off + N],
                    ident[0:1, 0:1],
                )
                nc.vector.tensor_copy(
                    alphaT_pad[:, 2 * b, b : b + 1], ps_al[:, 2 * j : 2 * j + 1]
                )
                nc.vector.tensor_copy(
                    alphaT_pad[0:N1, 2 * b + 1, b : b + 1],
                    ps_al[0:N1, 2 * j + 1 : 2 * j + 2],
                )
            for j, b in enumerate((b0, b1)):
                for e4 in range(E4):
                    nc.tensor.matmul(
                        ctx_ps[:, e4, :],
                        r(alphaT_pad[:, 2 * b, :]),
                        nat0[:, j, ts(e4, 512)],
                        start=(p == 0 and j == 0),
                        stop=False,
                        skip_group_check=True,
                    )
                    nc.tensor.matmul(
                        ctx_ps[:, e4, :],
                        r(alphaT_pad[0:N1, 2 * b + 1, :]),
                        nat1[:, j, ts(e4, 512)],
                        start=False,
                        stop=(p == PAIRS - 1 and j == 1),
                        skip_group_check=True,
                    )

        att_one = work.tile([128, AC, WV], f32r, tag="att", bufs=1, name="att_one")
        att_tiles = [att_one, att_one]

        # Pipeline: loads 1 section ahead (SWDGE); per section:
        #   transposes(p) ; alphaT+ctx(p-1) ; att1(p) ; scores+softmax(p)
        # nat(p) lives load(p-1)..ctx(p+1) -> bufs=3. encT written and read
        # within the section (PE in-order) -> bufs=1.
        # Pipeline: loads one section ahead; transposes of pair p+1 are
        # interleaved between the att1 matmuls of pair p (2:1) so the PE
        # stream never has a transpose-only phase (HAM stays warm).
        nats = {0: first_nats}
        als = {}
        encTs = {0: emit_transposes(0, nats[0])}
        for p in range(PAIRS):
            if p + 1 < PAIRS:
                nats[p + 1] = emit_loads(p + 1)
            emit_att1(p, encTs[p], (0, 1))
            if p >= 1:
                emit_alpha_ctx(p - 1, als.pop(p - 1), nats.pop(p - 1))
            if p + 1 < PAIRS:
                encTs[p + 1] = work.tile(
                    [128, EC, WV], f32r, tag="encT", bufs=2, name="encT"
                )
                jobs = transpose_jobs(p + 1, nats[p + 1], encTs[p + 1])
            else:
                jobs = None
            emit_att1(p, encTs.pop(p), (2, 3), jobs=jobs)
            als[p] = emit_scores_softmax(p)
        emit_alpha_ctx(PAIRS - 1, als.pop(PAIRS - 1), nats.pop(PAIRS - 1))

        for e4 in range(E4):
            ctx_e = work.tile([BL, 512], f32, tag="ctxe", bufs=2, name="ctxe")
            nc.vector.tensor_copy(ctx_e[:], ctx_ps[:, e4, :])
            nc.gpsimd.dma_start(ctx_d.ap()[:, ts(e4, 512)], ctx_e[:])

    return nc


def _get_nc():
    if "nc" not in _CACHE:
        _CACHE["nc"] = build()
    return _CACHE["nc"]


def _run(in_maps, trace=False, tmpdir=None):
    from concourse.bass_utils import run_bass_kernel_spmd

    nc = _get_nc()
    return run_bass_kernel_spmd(
        nc, in_maps, core_ids=list(range(NCORES)), trace=trace, tmpdir=tmpdir
    )


def make_in_maps(encoder_out, decoder_hidden, We, be, Wd, bd, Wf, bf=None):
    enc = np.ascontiguousarray(np.asarray(encoder_out, dtype=np.float32))
    dec = np.ascontiguousarray(np.asarray(decoder_hidden, dtype=np.float32))
    We = np.ascontiguousarray(np.asarray(We, dtype=np.float32))
    be = np.ascontiguousarray(np.asarray(be, dtype=np.float32))
    Wd = np.ascontiguousarray(np.asarray(Wd, dtype=np.float32))
    bd = np.ascontiguousarray(np.asarray(bd, dtype=np.float32))
    Wf = np.ascontiguousarray(np.asarray(Wf, dtype=np.float32))
    in_maps = []
    for i in range(NCORES):
        sl = slice(i * BL, (i + 1) * BL)
        in_maps.append(
            dict(
                encoder_out=enc[sl],
                decoder_hidden=dec[sl],
                We=We,
                be=be,
                Wd=Wd,
                bd=bd,
                Wf=Wf,
            )
        )
    return in_maps


def gather(results):
    context = np.concatenate([results[i]["context"] for i in range(NCORES)], axis=0)
    alpha = np.concatenate([results[i]["alpha"] for i in range(NCORES)], axis=0)
    return context, alpha


def kernel(encoder_out, decoder_hidden, We, be, Wd, bd, Wf, bf):
    in_maps = make_in_maps(encoder_out, decoder_hidden, We, be, Wd, bd, Wf, bf)
    res = _run(in_maps, trace=False)
    _CACHE["last_results"] = res
    return gather(res.results)


# revision 50
# speedup vs baseline: 1.8085x; 1.0505x over previous
"""Bass/Trainium2 kernel for Bahdanau (additive) attention, 8-core data-parallel.

Reference computation (per batch b):
    att1 = enc[b] @ We                    # [N, A]
    att2 = dec[b] @ Wd + bd               # [A]
    att  = tanh(att1 + att2 + be)         # [N, A]
    s    = att @ Wf (+ bf)                # [N]   (bf dropped: softmax-invariant)
    alpha= softmax(s)                     # [N]
    ctx  = sum_n alpha[n] * enc[b, n]     # [E]

Sharding: batch 128 -> 8 cores x 16. Weights replicated. No collectives.

Per-core design (B_loc=16, processed in 8 pairs):
  - enc is loaded f32 (split DMAs across queues), converted to bf16 on
    DVE/ACT, and transposed by the DMA xbar (dma_start transpose=True,
    bf16) straight into SBUF - the PE does no data transposes.
  - att1 computed transposed ([A_chunk=128, n-cols]) in bf16:
    lhsT = We blocks (natural), moving = encT [128, 416] per pair.
  - att2+be add fused into tanh on ScalarE as a per-partition bias.
  - scores via PE: lhsT = Wf chunk [128,1], moving = tanh output.
  - softmax on one partition per pair (reduce_max neg / exp+accum / recip).
  - context via PE with a zero-padded block-diagonal alphaT (each batch's
    alpha column at 256-aligned row offsets) against the bf16 enc tiles,
    all 16 batches accumulated into 4 persistent PSUM banks [16, 512].
  - 3-deep software pipeline: loads(p+2) / convert+xbar(p+1) /
    att1+scores(p) with alpha-placement+context of p-1 emitted mid-att1
    so the PE instruction stream never waits on the softmax chain.
"""

import numpy as np

B, N, E = 128, 196, 2048
D, A = 512, 512
NCORES = 8
BL = B // NCORES            # 16 batches per core
PAIRS = BL // 2             # 8
N0 = 128
N1 = N - N0                 # 68
NP = 80                     # N1 padded to a multiple of XBAR_TILE_SRC_ROWS (16)
PW = N0 + NP                # 208 cols per batch in the transposed layout
W = 2 * PW                  # 416 cols per pair
EC = E // 128               # 16
AC = A // 128               # 4
E4 = E // 512               # 4
DC = D // 128               # 4

_CACHE = {}


def _patch_tile_tail_drain(tile):
    """This walrus build rejects >1 sem-wait per instruction. Split extra
    waits onto single-wait NOPs committed just before the instruction, and
    do the same for the TileContext tail drain."""
    import concourse.mybir as mybir
    from concourse.vector_clock import ScopedClock

    if getattr(tile.TileContext, "_tail_drain_patched", False):
        return

    orig_commit = tile.TileContext._commit_instruction

    def _commit_instruction(self, inst, lazy_reg_writes=True):
        si = getattr(inst, "sync_info", None)
        engine = getattr(inst, "engine", None)
        if (
            si is not None
            and si.on_wait
            and len(si.on_wait) > 1
            and engine is not None
            and engine != mybir.EngineType.Unassigned
            and type(inst).__name__.startswith("Inst")
        ):
            waits = list(si.on_wait)
            for w in waits[:-1]:
                noop = mybir.InstNoOp(
                    name=self.nc.get_next_instruction_name(),
                    sync_info=mybir.SyncInfo(on_wait=[w], on_update=[]),
                    bass_nofuse=True,
                    engine=engine,
                )
                orig_commit(self, noop, lazy_reg_writes=False)
            inst.sync_info = mybir.SyncInfo(
                on_wait=[waits[-1]], on_update=list(si.on_update or [])
            )
        return orig_commit(self, inst, lazy_reg_writes)

    tile.TileContext._commit_instruction = _commit_instruction

    def _drain_and_barrier(self, tick_clock, wait_clock):
        nc = self.nc
        drain_inst = nc.sync.drain()
        wait_clock.add_sem_waits(
            drain_inst.ins, ScopedClock({None: tick_clock.global_clock})
        )
        si = drain_inst.ins.sync_info
        waits = list(si.on_wait or []) if si is not None else []
        if len(waits) > 1:
            drain_inst.ins.sync_info = mybir.SyncInfo(
                on_wait=waits[:1], on_update=list(si.on_update or [])
            )
            for w in waits[1:]:
                d = nc.sync.drain()
                d.ins.sync_info = mybir.SyncInfo(on_wait=[w], on_update=[])
        nc.all_engine_barrier()
        assert self.sems is not None
        popped = nc._tile_sem_poison_stack.pop()
        assert popped is self._sem_poison
        nc.clear_and_free_semaphores(list(self.sems.allocated().values()))
        nc.all_engine_barrier()

    tile.TileContext._drain_and_barrier = _drain_and_barrier
    tile.TileContext._tail_drain_patched = True


def build():
    from contextlib import ExitStack

    import concourse.bass as bass
    import concourse.tile as tile
    from concourse import mybir
    from concourse.bass import ts
    from concourse.masks import make_identity

    _patch_tile_tail_drain(tile)

    f32 = mybir.dt.float32
    f32r = mybir.dt.float32r
    TANH = mybir.ActivationFunctionType.Tanh
    EXP = mybir.ActivationFunctionType.Exp

    nc = bass.Bass(trn_type="TRN2", target_bir_lowering=False, debug=False)
    enc_d = nc.dram_tensor("encoder_out", [BL, N, E], f32, kind="ExternalInput")
    dec_d = nc.dram_tensor("decoder_hidden", [BL, D], f32, kind="ExternalInput")
    we_d = nc.dram_tensor("We", [E, A], f32, kind="ExternalInput")
    be_d = nc.dram_tensor("be", [A], f32, kind="ExternalInput")
    wd_d = nc.dram_tensor("Wd", [D, A], f32, kind="ExternalInput")
    bd_d = nc.dram_tensor("bd", [A], f32, kind="ExternalInput")
    wf_d = nc.dram_tensor("Wf", [A, 1], f32, kind="ExternalInput")
    ctx_d = nc.dram_tensor("context", [BL, E], f32, kind="ExternalOutput")
    alp_d = nc.dram_tensor("alpha", [BL, N, 1], f32, kind="ExternalOutput")

    WV = 392  # pair width in the transposed layout (2 * N)

    with tile.TileContext(nc) as tc, ExitStack() as ctx:
        consts = ctx.enter_context(tc.tile_pool(name="consts", bufs=1))

        ident = consts.tile([128, 128], f32)
        make_identity(nc, ident[:])
        ident_r = consts.tile([128, 128], f32r)
        nc.vector.tensor_copy(ident_r[:], ident[:])

        att2T_sb = consts.tile([128, AC, BL], f32)
        alphaT_pad = consts.tile([128, 2 * BL, BL], f32r)

        # open the main pools early so pair-0's enc loads are the first DMAs
        work = ctx.enter_context(tc.tile_pool(name="work", bufs=2))
        psum = ctx.enter_context(tc.tile_pool(name="psum", bufs=1, space="PSUM"))

        def emit_loads(p):
            # SWDGE (gpsimd) spreads each transfer across all 16 SDMA
            # engines; HWDGE rings would serialize on ~4 queues.
            b0, b1 = 2 * p, 2 * p + 1
            nat0 = work.tile([128, 2, E], f32r, tag="nat0", bufs=3, name="nat0")
            nat1 = work.tile([N1, 2, E], f32r, tag="nat1", bufs=3, name="nat1")
            # pair 0's SWDGE emission (~1us per dma_start on the Q7) is on
            # the startup critical path: use fewer, bigger transfers there
            nk, w = (2, 1024) if p == 0 else (4, 512)
            for j, b in enumerate((b0, b1)):
                for k in range(nk):
                    nc.gpsimd.dma_start(
                        nat0[:, j, ts(k, w)],
                        enc_d.ap()[b, 0:N0, ts(k, w)].bitcast(f32r),
                    )
                    nc.gpsimd.dma_start(
                        nat1[:, j, ts(k, w)],
                        enc_d.ap()[b, N0:N, ts(k, w)].bitcast(f32r),
                    )
            return nat0, nat1

        first_nats = emit_loads(0)

        we_sb = consts.tile([128, EC, A], f32r)
        for c in range(EC):
            nc.gpsimd.dma_start(we_sb[:, c, :], we_d.ap()[ts(c, 128), :].bitcast(f32r))
        wf_sb = consts.tile([128, AC], f32r)
        nc.sync.dma_start(
            wf_sb[:], wf_d.ap().rearrange("(c p) o -> p (c o)", p=128).bitcast(f32r)
        )

        with tc.tile_pool(name="pre", bufs=1) as pre:
            wd_sb = pre.tile([128, DC, A], f32)
            for c in range(DC):
                nc.sync.dma_start(wd_sb[:, c, :], wd_d.ap()[ts(c, 128), :])
            be_sb = pre.tile([128, AC], f32)
            bd_sb = pre.tile([128, AC], f32)
            nc.sync.dma_start(be_sb[:], be_d.ap().rearrange("(c p) -> p c", p=128))
            nc.sync.dma_start(bd_sb[:], bd_d.ap().rearrange("(c p) -> p c", p=128))
            bdbe = pre.tile([128, AC], f32)
            nc.vector.tensor_add(bdbe[:], be_sb[:], bd_sb[:])

            dec_sb = pre.tile([BL, D], f32)
            nc.sync.dma_start(dec_sb[:], dec_d.ap())
            decT_sb = pre.tile([128, DC, BL], f32)
            for c in range(DC):
                ps_tp = psum.tile([128, WV], f32, tag="tp", bufs=2, name="ps_tp")
                nc.tensor.transpose(
                    ps_tp[:, 0:BL], dec_sb[:, ts(c, 128)], ident[0:BL, 0:BL]
                )
                nc.vector.tensor_copy(decT_sb[:, c, :], ps_tp[:, 0:BL])

            for ca in range(AC):
                ps_a2 = psum.tile([128, WV], f32, tag="tp", bufs=2, name="ps_a2")
                for cd in range(DC):
                    nc.tensor.matmul(
                        ps_a2[:, 0:BL],
                        wd_sb[:, cd, ts(ca, 128)],
                        decT_sb[:, cd, :],
                        start=(cd == 0),
                        stop=(cd == DC - 1),
                    )
                nc.scalar.add(att2T_sb[:, ca, :], ps_a2[:, 0:BL], bdbe[:, ca : ca + 1])

            nc.gpsimd.memset(alphaT_pad[:].bitcast(f32), 0.0)

        ctx_ps = psum.tile([BL, E4, 512], f32, tag="ctx", bufs=1)

        def r(ap):
            return ap.bitcast(f32r)

        def copy_op(i, out, in_):
            if i % 2 == 0:
                nc.vector.tensor_copy(out, in_)
            else:
                nc.scalar.copy(out, in_)

        def transpose_jobs(p, nats, encT):
            """Yield closures: 4 PE transposes + 1 evacuation copy per e-chunk.
            Interleaved between att1 matmuls so the PE always has REGULAR
            matmuls in flight (transpose-mode doesn't count as HAM-busy)."""
            nat0, nat1 = nats
            state = {}

            def t_job(ce, j, half):
                def go():
                    if ce not in state:
                        state[ce] = psum.tile(
                            [128, WV], f32r, tag="tp", bufs=2, name="ps_t"
                        )
                    ps_t = state[ce]
                    off = N * j
                    if half == 0:
                        nc.tensor.transpose(
                            ps_t[:, off : off + N0],
                            nat0[:, j, ts(ce, 128)],
                            ident_r[:, :],
                        )
                    else:
                        nc.tensor.transpose(
                            ps_t[:, off + N0 : off + N],
                            nat1[:, j, ts(ce, 128)],
                            ident_r[0:N1, 0:N1],
                        )
                return go

            def c_job(ce):
                def go():
                    copy_op(ce, encT[:, ce, :], state.pop(ce)[:])
                return go

            jobs = []
            for ce in range(EC):
                for j in range(2):
                    for half in range(2):
                        jobs.append(t_job(ce, j, half))
                jobs.append(c_job(ce))
            return jobs

        def emit_transposes(p, nats):
            encT = work.tile([128, EC, WV], f32r, tag="encT", bufs=2, name="encT")
            for job in transpose_jobs(p, nats, encT):
                job()
            return encT

        def emit_att1(p, encT, ca_list, jobs=None):
            b0, b1 = 2 * p, 2 * p + 1
            att = att_tiles[p % 2]
            nmm = len(ca_list) * EC
            k = 0
            jpos = 0
            for ca in ca_list:
                ps_a = psum.tile([128, WV], f32, tag="att1", bufs=2, name="ps_a")
                for ce in range(EC):
                    nc.tensor.matmul(
                        ps_a[:],
                        we_sb[:, ce, ts(ca, 128)],
                        encT[:, ce, :],
                        start=(ce == 0),
                        stop=(ce == EC - 1),
                    )
                    if jobs:
                        k += 1
                        if k % 4 == 0:
                            jend = len(jobs) * k // nmm
                            while jpos < jend:
                                jobs[jpos]()
                                jpos += 1
                nc.scalar.activation(
                    att[:, ca, 0:N], ps_a[:, 0:N], TANH,
                    bias=att2T_sb[:, ca, b0 : b0 + 1],
                )
                nc.scalar.activation(
                    att[:, ca, N:WV], ps_a[:, N:WV], TANH,
                    bias=att2T_sb[:, ca, b1 : b1 + 1],
                )
            if jobs:
                while jpos < len(jobs):
                    jobs[jpos]()
                    jpos += 1

        def emit_scores_softmax(p):
            b0 = 2 * p
            att = att_tiles[p % 2]
            ps_s = psum.tile([128, WV], f32, tag="att1", bufs=2, name="ps_s")
            for ca in range(AC):
                nc.tensor.matmul(
                    ps_s[0:1, :],
                    wf_sb[:, ca : ca + 1],
                    att[:, ca, :],
                    start=(ca == 0),
                    stop=(ca == AC - 1),
                )
            sc_sb = work.tile([1, WV], f32, tag="sc", bufs=2, name="sc")
            nc.vector.tensor_copy(sc_sb[:], ps_s[0:1, :])

            mx = work.tile([1, 2], f32, tag="mx", bufs=2, name="mx")
            sm = work.tile([1, 2], f32, tag="sm", bufs=2, name="sm")
            rs = work.tile([1, 2], f32, tag="rs", bufs=2, name="rs")
            nc.vector.reduce_max(
                mx[:],
                sc_sb[:].rearrange("p (h w) -> p h w", h=2),
                axis=mybir.AxisListType.X,
                negate=True,
            )
            al = work.tile([1, WV], f32, tag="al", bufs=2, name="al")
            for h in range(2):
                nc.scalar.activation(
                    al[0:1, ts(h, N)], sc_sb[0:1, ts(h, N)], EXP,
                    bias=mx[0:1, h : h + 1],
                    accum_out=sm[0:1, h : h + 1],
                )
            nc.vector.reciprocal(rs[:], sm[:])
            for h in range(2):
                nc.vector.tensor_scalar_mul(
                    al[0:1, ts(h, N)], al[0:1, ts(h, N)], rs[0:1, h : h + 1]
                )
            nc.gpsimd.dma_start(
                alp_d.ap()[b0 : b0 + 2].rearrange("b n o -> o (b n)"), al[:]
            )
            return al

        def emit_alpha_ctx(p, al, nats):
            nat0, nat1 = nats
            b0, b1 = 2 * p, 2 * p + 1
            ps_al = psum.tile([128, WV], f32, tag="att1", bufs=2, name="ps_al")
            for j, b in enumerate((b0, b1)):
                off = N * j
                nc.tensor.transpose(
                    ps_al[:, 2 * j : 2 * j + 1],
                    al[0:1, off : off + N0],
                    ident[0:1, 0:1],
                )
                nc.tensor.transpose(
                    ps_al[0:N1, 2 * j + 1 : 2 * j + 2],
                    al[0:1, off + N0 : off + N],
                    ident[0:1, 0:1],
                )
                nc.vector.tensor_copy(
                    alphaT_pad[:, 2 * b, b : b + 1], ps_al[:, 2 * j : 2 * j + 1]
                )
                nc.vector.tensor_copy(
                    alphaT_pad[0:N1, 2 * b + 1, b : b + 1],
                    ps_al[0:N1, 2 * j + 1 : 2 * j + 2],
                )
            for j, b in enumerate((b0, b1)):
                for e4 in range(E4):
                    nc.tensor.matmul(
                        ctx_ps[:, e4, :],
                        r(alphaT_pad[:, 2 * b, :]),
                        nat0[:, j, ts(e4, 512)],
                        start=(p == 0 and j == 0),
                        stop=False,
                        skip_group_check=True,
                    )
                    nc.tensor.matmul(
                        ctx_ps[:, e4, :],
                        r(alphaT_pad[0:N1, 2 * b + 1, :]),
                        nat1[:, j, ts(e4, 512)],
                        start=False,
                        stop=(p == PAIRS - 1 and j == 1),
                        skip_group_check=True,
                    )

        att_one = work.tile([128, AC, WV], f32r, tag="att", bufs=1, name="att_one")
        att_tiles = [att_one, att_one]

        # Pipeline: loads 1 section ahead (SWDGE); per section:
        #   transposes(p) ; alphaT+ctx(p-1) ; att1(p) ; scores+softmax(p)
        # nat(p) lives load(p-1)..ctx(p+1) -> bufs=3. encT written and read
        # within the section (PE in-order) -> bufs=1.
        # Pipeline: loads one section ahead; transposes of pair p+1 are
        # interleaved between the att1 matmuls of pair p (2:1) so the PE
        # stream never has a transpose-only phase (HAM stays warm).
        nats = {0: first_nats}
        als = {}
        encTs = {0: emit_transposes(0, nats[0])}
        for p in range(PAIRS):
            if p + 1 < PAIRS:
                nats[p + 1] = emit_loads(p + 1)
            emit_att1(p, encTs[p], (0, 1))
            if p >= 1:
                emit_alpha_ctx(p - 1, als.pop(p - 1), nats.pop(p - 1))
            if p + 1 < PAIRS:
                encTs[p + 1] = work.tile(
                    [128, EC, WV], f32r, tag="encT", bufs=2, name="encT"
                )
                jobs = transpose_jobs(p + 1, nats[p + 1], encTs[p + 1])
            else:
                jobs = None
            emit_att1(p, encTs.pop(p), (2, 3), jobs=jobs)
            als[p] = emit_scores_softmax(p)
        emit_alpha_ctx(PAIRS - 1, als.pop(PAIRS - 1), nats.pop(PAIRS - 1))

        for e4 in range(E4):
            ctx_e = work.tile([BL, 512], f32, tag="ctxe", bufs=2, name="ctxe")
            nc.vector.tensor_copy(ctx_e[:], ctx_ps[:, e4, :])
            nc.gpsimd.dma_start(ctx_d.ap()[:, ts(e4, 512)], ctx_e[:])

    return nc


def _get_nc():
    if "nc" not in _CACHE:
        _CACHE["nc"] = build()
    return _CACHE["nc"]


def _run(in_maps, trace=False, tmpdir=None):
    from concourse.bass_utils import run_bass_kernel_spmd

    nc = _get_nc()
    return run_bass_kernel_spmd(
        nc, in_maps, core_ids=list(range(NCORES)), trace=trace, tmpdir=tmpdir
    )


def make_in_maps(encoder_out, decoder_hidden, We, be, Wd, bd, Wf, bf=None):
    enc = np.ascontiguousarray(np.asarray(encoder_out, dtype=np.float32))
    dec = np.ascontiguousarray(np.asarray(decoder_hidden, dtype=np.float32))
    We = np.ascontiguousarray(np.asarray(We, dtype=np.float32))
    be = np.ascontiguousarray(np.asarray(be, dtype=np.float32))
    Wd = np.ascontiguousarray(np.asarray(Wd, dtype=np.float32))
    bd = np.ascontiguousarray(np.asarray(bd, dtype=np.float32))
    Wf = np.ascontiguousarray(np.asarray(Wf, dtype=np.float32))
    in_maps = []
    for i in range(NCORES):
        sl = slice(i * BL, (i + 1) * BL)
        in_maps.append(
            dict(
                encoder_out=enc[sl],
                decoder_hidden=dec[sl],
                We=We,
                be=be,
                Wd=Wd,
                bd=bd,
                Wf=Wf,
            )
        )
    return in_maps


def gather(results):
    context = np.concatenate([results[i]["context"] for i in range(NCORES)], axis=0)
    alpha = np.concatenate([results[i]["alpha"] for i in range(NCORES)], axis=0)
    return context, alpha


def kernel(encoder_out, decoder_hidden, We, be, Wd, bd, Wf, bf):
    in_maps = make_in_maps(encoder_out, decoder_hidden, We, be, Wd, bd, Wf, bf)
    res = _run(in_maps, trace=False)
    _CACHE["last_results"] = res
    return gather(res.results)


# revision 51
# speedup vs baseline: 1.8202x; 1.0065x over previous
"""Bass/Trainium2 kernel for Bahdanau (additive) attention, 8-core data-parallel.

Reference computation (per batch b):
    att1 = enc[b] @ We                    # [N, A]
    att2 = dec[b] @ Wd + bd               # [A]
    att  = tanh(att1 + att2 + be)         # [N, A]
    s    = att @ Wf (+ bf)                # [N]   (bf dropped: softmax-invariant)
    alpha= softmax(s)                     # [N]
    ctx  = sum_n alpha[n] * enc[b, n]     # [E]

Sharding: batch 128 -> 8 cores x 16. Weights replicated. No collectives.

Per-core design (B_loc=16, processed in 8 pairs):
  - enc is loaded f32 (split DMAs across queues), converted to bf16 on
    DVE/ACT, and transposed by the DMA xbar (dma_start transpose=True,
    bf16) straight into SBUF - the PE does no data transposes.
  - att1 computed transposed ([A_chunk=128, n-cols]) in bf16:
    lhsT = We blocks (natural), moving = encT [128, 416] per pair.
  - att2+be add fused into tanh on ScalarE as a per-partition bias.
  - scores via PE: lhsT = Wf chunk [128,1], moving = tanh output.
  - softmax on one partition per pair (reduce_max neg / exp+accum / recip).
  - context via PE with a zero-padded block-diagonal alphaT (each batch's
    alpha column at 256-aligned row offsets) against the bf16 enc tiles,
    all 16 batches accumulated into 4 persistent PSUM banks [16, 512].
  - 3-deep software pipeline: loads(p+2) / convert+xbar(p+1) /
    att1+scores(p) with alpha-placement+context of p-1 emitted mid-att1
    so the PE instruction stream never waits on the softmax chain.
"""

import numpy as np

B, N, E = 128, 196, 2048
D, A = 512, 512
NCORES = 8
BL = B // NCORES            # 16 batches per core
PAIRS = BL // 2             # 8
N0 = 128
N1 = N - N0                 # 68
NP = 80                     # N1 padded to a multiple of XBAR_TILE_SRC_ROWS (16)
PW = N0 + NP                # 208 cols per batch in the transposed layout
W = 2 * PW                  # 416 cols per pair
EC = E // 128               # 16
AC = A // 128               # 4
E4 = E // 512               # 4
DC = D // 128               # 4

_CACHE = {}


def _patch_tile_tail_drain(tile):
    """This walrus build rejects >1 sem-wait per instruction. Split extra
    waits onto single-wait NOPs committed just before the instruction, and
    do the same for the TileContext tail drain."""
    import concourse.mybir as mybir
    from concourse.vector_clock import ScopedClock

    if getattr(tile.TileContext, "_tail_drain_patched", False):
        return

    orig_commit = tile.TileContext._commit_instruction

    def _commit_instruction(self, inst, lazy_reg_writes=True):
        si = getattr(inst, "sync_info", None)
        engine = getattr(inst, "engine", None)
        if (
            si is not None
            and si.on_wait
            and len(si.on_wait) > 1
            and engine is not None
            and engine != mybir.EngineType.Unassigned
            and type(inst).__name__.startswith("Inst")
        ):
            waits = list(si.on_wait)
            for w in waits[:-1]:
                noop = mybir.InstNoOp(
                    name=self.nc.get_next_instruction_name(),
                    sync_info=mybir.SyncInfo(on_wait=[w], on_update=[]),
                    bass_nofuse=True,
                    engine=engine,
                )
                orig_commit(self, noop, lazy_reg_writes=False)
            inst.sync_info = mybir.SyncInfo(
                on_wait=[waits[-1]], on_update=list(si.on_update or [])
            )
        return orig_commit(self, inst, lazy_reg_writes)

    tile.TileContext._commit_instruction = _commit_instruction

    def _drain_and_barrier(self, tick_clock, wait_clock):
        nc = self.nc
        drain_inst = nc.sync.drain()
        wait_clock.add_sem_waits(
            drain_inst.ins, ScopedClock({None: tick_clock.global_clock})
        )
        si = drain_inst.ins.sync_info
        waits = list(si.on_wait or []) if si is not None else []
        if len(waits) > 1:
            drain_inst.ins.sync_info = mybir.SyncInfo(
                on_wait=waits[:1], on_update=list(si.on_update or [])
            )
            for w in waits[1:]:
                d = nc.sync.drain()
                d.ins.sync_info = mybir.SyncInfo(on_wait=[w], on_update=[])
        nc.all_engine_barrier()
        assert self.sems is not None
        popped = nc._tile_sem_poison_stack.pop()
        assert popped is self._sem_poison
        nc.clear_and_free_semaphores(list(self.sems.allocated().values()))
        nc.all_engine_barrier()

    tile.TileContext._drain_and_barrier = _drain_and_barrier
    tile.TileContext._tail_drain_patched = True


def build():
    from contextlib import ExitStack

    import concourse.bass as bass
    import concourse.tile as tile
    from concourse import mybir
    from concourse.bass import ts
    from concourse.masks import make_identity

    _patch_tile_tail_drain(tile)

    f32 = mybir.dt.float32
    f32r = mybir.dt.float32r
    TANH = mybir.ActivationFunctionType.Tanh
    EXP = mybir.ActivationFunctionType.Exp

    nc = bass.Bass(trn_type="TRN2", target_bir_lowering=False, debug=False)
    enc_d = nc.dram_tensor("encoder_out", [BL, N, E], f32, kind="ExternalInput")
    dec_d = nc.dram_tensor("decoder_hidden", [BL, D], f32, kind="ExternalInput")
    we_d = nc.dram_tensor("We", [E, A], f32, kind="ExternalInput")
    be_d = nc.dram_tensor("be", [A], f32, kind="ExternalInput")
    wd_d = nc.dram_tensor("Wd", [D, A], f32, kind="ExternalInput")
    bd_d = nc.dram_tensor("bd", [A], f32, kind="ExternalInput")
    wf_d = nc.dram_tensor("Wf", [A, 1], f32, kind="ExternalInput")
    ctx_d = nc.dram_tensor("context", [BL, E], f32, kind="ExternalOutput")
    alp_d = nc.dram_tensor("alpha", [BL, N, 1], f32, kind="ExternalOutput")

    WV = 392  # pair width in the transposed layout (2 * N)

    with tile.TileContext(nc) as tc, ExitStack() as ctx:
        consts = ctx.enter_context(tc.tile_pool(name="consts", bufs=1))

        ident = consts.tile([128, 128], f32)
        make_identity(nc, ident[:])
        ident_r = consts.tile([128, 128], f32r)
        nc.vector.tensor_copy(ident_r[:], ident[:])

        att2T_sb = consts.tile([128, AC, BL], f32)
        alphaT_pad = consts.tile([128, 2 * BL, BL], f32r)

        # open the main pools early so pair-0's enc loads are the first DMAs
        work = ctx.enter_context(tc.tile_pool(name="work", bufs=2))
        psum = ctx.enter_context(tc.tile_pool(name="psum", bufs=1, space="PSUM"))

        def emit_loads(p):
            # SWDGE (gpsimd) spreads each transfer across all 16 SDMA
            # engines; HWDGE rings would serialize on ~4 queues.
            b0, b1 = 2 * p, 2 * p + 1
            nat0 = work.tile([128, 2, E], f32r, tag="nat0", bufs=3, name="nat0")
            nat1 = work.tile([N1, 2, E], f32r, tag="nat1", bufs=3, name="nat1")
            # pair 0's SWDGE emission (~1us per dma_start on the Q7) is on
            # the startup critical path: use fewer, bigger transfers there
            nk, w = (2, 1024) if p == 0 else (4, 512)
            for j, b in enumerate((b0, b1)):
                for k in range(nk):
                    nc.gpsimd.dma_start(
                        nat0[:, j, ts(k, w)],
                        enc_d.ap()[b, 0:N0, ts(k, w)].bitcast(f32r),
                    )
                    nc.gpsimd.dma_start(
                        nat1[:, j, ts(k, w)],
                        enc_d.ap()[b, N0:N, ts(k, w)].bitcast(f32r),
                    )
            return nat0, nat1

        first_nats = emit_loads(0)

        we_sb = consts.tile([128, EC, A], f32r)
        for c in range(EC):
            nc.gpsimd.dma_start(we_sb[:, c, :], we_d.ap()[ts(c, 128), :].bitcast(f32r))
        wf_sb = consts.tile([128, AC], f32r)
        nc.sync.dma_start(
            wf_sb[:], wf_d.ap().rearrange("(c p) o -> p (c o)", p=128).bitcast(f32r)
        )

        with tc.tile_pool(name="pre", bufs=1) as pre:
            wd_sb = pre.tile([128, DC, A], f32)
            for c in range(DC):
                nc.sync.dma_start(wd_sb[:, c, :], wd_d.ap()[ts(c, 128), :])
            be_sb = pre.tile([128, AC], f32)
            bd_sb = pre.tile([128, AC], f32)
            nc.sync.dma_start(be_sb[:], be_d.ap().rearrange("(c p) -> p c", p=128))
            nc.sync.dma_start(bd_sb[:], bd_d.ap().rearrange("(c p) -> p c", p=128))
            bdbe = pre.tile([128, AC], f32)
            nc.vector.tensor_add(bdbe[:], be_sb[:], bd_sb[:])

            dec_sb = pre.tile([BL, D], f32)
            nc.sync.dma_start(dec_sb[:], dec_d.ap())
            decT_sb = pre.tile([128, DC, BL], f32)
            for c in range(DC):
                ps_tp = psum.tile([128, WV], f32, tag="tp", bufs=2, name="ps_tp")
                nc.tensor.transpose(
                    ps_tp[:, 0:BL], dec_sb[:, ts(c, 128)], ident[0:BL, 0:BL]
                )
                nc.vector.tensor_copy(decT_sb[:, c, :], ps_tp[:, 0:BL])

            for ca in range(AC):
                ps_a2 = psum.tile([128, WV], f32, tag="tp", bufs=2, name="ps_a2")
                for cd in range(DC):
                    nc.tensor.matmul(
                        ps_a2[:, 0:BL],
                        wd_sb[:, cd, ts(ca, 128)],
                        decT_sb[:, cd, :],
                        start=(cd == 0),
                        stop=(cd == DC - 1),
                    )
                nc.scalar.add(att2T_sb[:, ca, :], ps_a2[:, 0:BL], bdbe[:, ca : ca + 1])

            nc.gpsimd.memset(alphaT_pad[:].bitcast(f32), 0.0)

        ctx_ps = psum.tile([BL, E4, 512], f32, tag="ctx", bufs=1)

        def r(ap):
            return ap.bitcast(f32r)

        def copy_op(i, out, in_):
            if i % 2 == 0:
                nc.vector.tensor_copy(out, in_)
            else:
                nc.scalar.copy(out, in_)

        def transpose_jobs(p, nats, encT):
            """Yield closures: 4 PE transposes + 1 evacuation copy per e-chunk.
            Interleaved between att1 matmuls so the PE always has REGULAR
            matmuls in flight (transpose-mode doesn't count as HAM-busy)."""
            nat0, nat1 = nats
            state = {}

            def t_job(ce, j, half):
                def go():
                    if ce not in state:
                        state[ce] = psum.tile(
                            [128, WV], f32r, tag="tp", bufs=2, name="ps_t"
                        )
                    ps_t = state[ce]
                    off = N * j
                    if half == 0:
                        nc.tensor.transpose(
                            ps_t[:, off : off + N0],
                            nat0[:, j, ts(ce, 128)],
                            ident_r[:, :],
                        )
                    else:
                        nc.tensor.transpose(
                            ps_t[:, off + N0 : off + N],
                            nat1[:, j, ts(ce, 128)],
                            ident_r[0:N1, 0:N1],
                        )
                return go

            def c_job(ce):
                def go():
                    copy_op(ce, encT[:, ce, :], state.pop(ce)[:])
                return go

            jobs = []
            for ce in range(EC):
                for j in range(2):
                    for half in range(2):
                        jobs.append(t_job(ce, j, half))
                jobs.append(c_job(ce))
            return jobs

        def emit_transposes(p, nats):
            encT = work.tile([128, EC, WV], f32r, tag="encT", bufs=2, name="encT")
            for job in transpose_jobs(p, nats, encT):
                job()
            return encT

        def emit_att1(p, encT, ca_list, jobs=None):
            b0, b1 = 2 * p, 2 * p + 1
            att = att_tiles[p % 2]
            nmm = len(ca_list) * EC
            k = 0
            jpos = 0
            for ca in ca_list:
                ps_a = psum.tile([128, WV], f32, tag="att1", bufs=2, name="ps_a")
                for ce in range(EC):
                    nc.tensor.matmul(
                        ps_a[:],
                        we_sb[:, ce, ts(ca, 128)],
                        encT[:, ce, :],
                        start=(ce == 0),
                        stop=(ce == EC - 1),
                    )
                    if jobs:
                        k += 1
                        if k % 8 == 0:
                            jend = len(jobs) * k // nmm
                            while jpos < jend:
                                jobs[jpos]()
                                jpos += 1
                nc.scalar.activation(
                    att[:, ca, 0:N], ps_a[:, 0:N], TANH,
                    bias=att2T_sb[:, ca, b0 : b0 + 1],
                )
                nc.scalar.activation(
                    att[:, ca, N:WV], ps_a[:, N:WV], TANH,
                    bias=att2T_sb[:, ca, b1 : b1 + 1],
                )
            if jobs:
                while jpos < len(jobs):
                    jobs[jpos]()
                    jpos += 1

        def emit_scores_softmax(p):
            b0 = 2 * p
            att = att_tiles[p % 2]
            ps_s = psum.tile([128, WV], f32, tag="att1", bufs=2, name="ps_s")
            for ca in range(AC):
                nc.tensor.matmul(
                    ps_s[0:1, :],
                    wf_sb[:, ca : ca + 1],
                    att[:, ca, :],
                    start=(ca == 0),
                    stop=(ca == AC - 1),
                )
            sc_sb = work.tile([1, WV], f32, tag="sc", bufs=2, name="sc")
            nc.vector.tensor_copy(sc_sb[:], ps_s[0:1, :])

            mx = work.tile([1, 2], f32, tag="mx", bufs=2, name="mx")
            sm = work.tile([1, 2], f32, tag="sm", bufs=2, name="sm")
            rs = work.tile([1, 2], f32, tag="rs", bufs=2, name="rs")
            nc.vector.reduce_max(
                mx[:],
                sc_sb[:].rearrange("p (h w) -> p h w", h=2),
                axis=mybir.AxisListType.X,
                negate=True,
            )
            al = work.tile([1, WV], f32, tag="al", bufs=2, name="al")
            for h in range(2):
                nc.scalar.activation(
                    al[0:1, ts(h, N)], sc_sb[0:1, ts(h, N)], EXP,
                    bias=mx[0:1, h : h + 1],
                    accum_out=sm[0:1, h : h + 1],
                )
            nc.vector.reciprocal(rs[:], sm[:])
            for h in range(2):
                nc.vector.tensor_scalar_mul(
                    al[0:1, ts(h, N)], al[0:1, ts(h, N)], rs[0:1, h : h + 1]
                )
            nc.gpsimd.dma_start(
                alp_d.ap()[b0 : b0 + 2].rearrange("b n o -> o (b n)"), al[:]
            )
            return al

        def emit_alpha_ctx(p, al, nats):
            nat0, nat1 = nats
            b0, b1 = 2 * p, 2 * p + 1
            ps_al = psum.tile([128, WV], f32, tag="att1", bufs=2, name="ps_al")
            for j, b in enumerate((b0, b1)):
                off = N * j
                nc.tensor.transpose(
                    ps_al[:, 2 * j : 2 * j + 1],
                    al[0:1, off : off + N0],
                    ident[0:1, 0:1],
                )
                nc.tensor.transpose(
                    ps_al[0:N1, 2 * j + 1 : 2 * j + 2],
                    al[0:1, off + N0 : off + N],
                    ident[0:1, 0:1],
                )
                nc.vector.tensor_copy(
                    alphaT_pad[:, 2 * b, b : b + 1], ps_al[:, 2 * j : 2 * j + 1]
                )
                nc.vector.tensor_copy(
                    alphaT_pad[0:N1, 2 * b + 1, b : b + 1],
                    ps_al[0:N1, 2 * j + 1 : 2 * j + 2],
                )
            for j, b in enumerate((b0, b1)):
                for e4 in range(E4):
                    nc.tensor.matmul(
                        ctx_ps[:, e4, :],
                        r(alphaT_pad[:, 2 * b, :]),
                        nat0[:, j, ts(e4, 512)],
                        start=(p == 0 and j == 0),
                        stop=False,
                        skip_group_check=True,
                    )
                    nc.tensor.matmul(
                        ctx_ps[:, e4, :],
                        r(alphaT_pad[0:N1, 2 * b + 1, :]),
                        nat1[:, j, ts(e4, 512)],
                        start=False,
                        stop=(p == PAIRS - 1 and j == 1),
                        skip_group_check=True,
                    )

        att_one = work.tile([128, AC, WV], f32r, tag="att", bufs=1, name="att_one")
        att_tiles = [att_one, att_one]

        # Pipeline: loads 1 section ahead (SWDGE); per section:
        #   transposes(p) ; alphaT+ctx(p-1) ; att1(p) ; scores+softmax(p)
        # nat(p) lives load(p-1)..ctx(p+1) -> bufs=3. encT written and read
        # within the section (PE in-order) -> bufs=1.
        # Pipeline: loads one section ahead; transposes of pair p+1 are
        # interleaved between the att1 matmuls of pair p (2:1) so the PE
        # stream never has a transpose-only phase (HAM stays warm).
        nats = {0: first_nats}
        als = {}
        encTs = {0: emit_transposes(0, nats[0])}
        for p in range(PAIRS):
            if p + 1 < PAIRS:
                nats[p + 1] = emit_loads(p + 1)
            emit_att1(p, encTs[p], (0, 1))
            if p >= 1:
                emit_alpha_ctx(p - 1, als.pop(p - 1), nats.pop(p - 1))
            if p + 1 < PAIRS:
                encTs[p + 1] = work.tile(
                    [128, EC, WV], f32r, tag="encT", bufs=2, name="encT"
                )
                jobs = transpose_jobs(p + 1, nats[p + 1], encTs[p + 1])
            else:
                jobs = None
            emit_att1(p, encTs.pop(p), (2, 3), jobs=jobs)
            als[p] = emit_scores_softmax(p)
        emit_alpha_ctx(PAIRS - 1, als.pop(PAIRS - 1), nats.pop(PAIRS - 1))

        for e4 in range(E4):
            ctx_e = work.tile([BL, 512], f32, tag="ctxe", bufs=2, name="ctxe")
            nc.vector.tensor_copy(ctx_e[:], ctx_ps[:, e4, :])
            nc.gpsimd.dma_start(ctx_d.ap()[:, ts(e4, 512)], ctx_e[:])

    return nc


def _get_nc():
    if "nc" not in _CACHE:
        _CACHE["nc"] = build()
    return _CACHE["nc"]


def _run(in_maps, trace=False, tmpdir=None):
    from concourse.bass_utils import run_bass_kernel_spmd

    nc = _get_nc()
    return run_bass_kernel_spmd(
        nc, in_maps, core_ids=list(range(NCORES)), trace=trace, tmpdir=tmpdir
    )


def make_in_maps(encoder_out, decoder_hidden, We, be, Wd, bd, Wf, bf=None):
    enc = np.ascontiguousarray(np.asarray(encoder_out, dtype=np.float32))
    dec = np.ascontiguousarray(np.asarray(decoder_hidden, dtype=np.float32))
    We = np.ascontiguousarray(np.asarray(We, dtype=np.float32))
    be = np.ascontiguousarray(np.asarray(be, dtype=np.float32))
    Wd = np.ascontiguousarray(np.asarray(Wd, dtype=np.float32))
    bd = np.ascontiguousarray(np.asarray(bd, dtype=np.float32))
    Wf = np.ascontiguousarray(np.asarray(Wf, dtype=np.float32))
    in_maps = []
    for i in range(NCORES):
        sl = slice(i * BL, (i + 1) * BL)
        in_maps.append(
            dict(
                encoder_out=enc[sl],
                decoder_hidden=dec[sl],
                We=We,
                be=be,
                Wd=Wd,
                bd=bd,
                Wf=Wf,
            )
        )
    return in_maps


def gather(results):
    context = np.concatenate([results[i]["context"] for i in range(NCORES)], axis=0)
    alpha = np.concatenate([results[i]["alpha"] for i in range(NCORES)], axis=0)
    return context, alpha


def kernel(encoder_out, decoder_hidden, We, be, Wd, bd, Wf, bf):
    in_maps = make_in_maps(encoder_out, decoder_hidden, We, be, Wd, bd, Wf, bf)
    res = _run(in_maps, trace=False)
    _CACHE["last_results"] = res
    return gather(res.results)


# revision 52
# speedup vs baseline: 1.8344x; 1.0078x over previous
"""Bass/Trainium2 kernel for Bahdanau (additive) attention, 8-core data-parallel.

Reference computation (per batch b):
    att1 = enc[b] @ We                    # [N, A]
    att2 = dec[b] @ Wd + bd               # [A]
    att  = tanh(att1 + att2 + be)         # [N, A]
    s    = att @ Wf (+ bf)                # [N]   (bf dropped: softmax-invariant)
    alpha= softmax(s)                     # [N]
    ctx  = sum_n alpha[n] * enc[b, n]     # [E]

Sharding: batch 128 -> 8 cores x 16. Weights replicated. No collectives.

Per-core design (B_loc=16, processed in 8 pairs):
  - enc is loaded f32 (split DMAs across queues), converted to bf16 on
    DVE/ACT, and transposed by the DMA xbar (dma_start transpose=True,
    bf16) straight into SBUF - the PE does no data transposes.
  - att1 computed transposed ([A_chunk=128, n-cols]) in bf16:
    lhsT = We blocks (natural), moving = encT [128, 416] per pair.
  - att2+be add fused into tanh on ScalarE as a per-partition bias.
  - scores via PE: lhsT = Wf chunk [128,1], moving = tanh output.
  - softmax on one partition per pair (reduce_max neg / exp+accum / recip).
  - context via PE with a zero-padded block-diagonal alphaT (each batch's
    alpha column at 256-aligned row offsets) against the bf16 enc tiles,
    all 16 batches accumulated into 4 persistent PSUM banks [16, 512].
  - 3-deep software pipeline: loads(p+2) / convert+xbar(p+1) /
    att1+scores(p) with alpha-placement+context of p-1 emitted mid-att1
    so the PE instruction stream never waits on the softmax chain.
"""

import numpy as np

B, N, E = 128, 196, 2048
D, A = 512, 512
NCORES = 8
BL = B // NCORES            # 16 batches per core
PAIRS = BL // 2             # 8
N0 = 128
N1 = N - N0                 # 68
NP = 80                     # N1 padded to a multiple of XBAR_TILE_SRC_ROWS (16)
PW = N0 + NP                # 208 cols per batch in the transposed layout
W = 2 * PW                  # 416 cols per pair
EC = E // 128               # 16
AC = A // 128               # 4
E4 = E // 512               # 4
DC = D // 128               # 4

_CACHE = {}


def _patch_tile_tail_drain(tile):
    """This walrus build rejects >1 sem-wait per instruction. Split extra
    waits onto single-wait NOPs committed just before the instruction, and
    do the same for the TileContext tail drain."""
    import concourse.mybir as mybir
    from concourse.vector_clock import ScopedClock

    if getattr(tile.TileContext, "_tail_drain_patched", False):
        return

    orig_commit = tile.TileContext._commit_instruction

    def _commit_instruction(self, inst, lazy_reg_writes=True):
        si = getattr(inst, "sync_info", None)
        engine = getattr(inst, "engine", None)
        if (
            si is not None
            and si.on_wait
            and len(si.on_wait) > 1
            and engine is not None
            and engine != mybir.EngineType.Unassigned
            and type(inst).__name__.startswith("Inst")
        ):
            waits = list(si.on_wait)
            for w in waits[:-1]:
                noop = mybir.InstNoOp(
                    name=self.nc.get_next_instruction_name(),
                    sync_info=mybir.SyncInfo(on_wait=[w], on_update=[]),
                    bass_nofuse=True,
                    engine=engine,
                )
                orig_commit(self, noop, lazy_reg_writes=False)
            inst.sync_info = mybir.SyncInfo(
                on_wait=[waits[-1]], on_update=list(si.on_update or [])
            )
        return orig_commit(self, inst, lazy_reg_writes)

    tile.TileContext._commit_instruction = _commit_instruction

    def _drain_and_barrier(self, tick_clock, wait_clock):
        nc = self.nc
        drain_inst = nc.sync.drain()
        wait_clock.add_sem_waits(
            drain_inst.ins, ScopedClock({None: tick_clock.global_clock})
        )
        si = drain_inst.ins.sync_info
        waits = list(si.on_wait or []) if si is not None else []
        if len(waits) > 1:
            drain_inst.ins.sync_info = mybir.SyncInfo(
                on_wait=waits[:1], on_update=list(si.on_update or [])
            )
            for w in waits[1:]:
                d = nc.sync.drain()
                d.ins.sync_info = mybir.SyncInfo(on_wait=[w], on_update=[])
        nc.all_engine_barrier()
        assert self.sems is not None
        popped = nc._tile_sem_poison_stack.pop()
        assert popped is self._sem_poison
        nc.clear_and_free_semaphores(list(self.sems.allocated().values()))
        nc.all_engine_barrier()

    tile.TileContext._drain_and_barrier = _drain_and_barrier
    tile.TileContext._tail_drain_patched = True


def build():
    from contextlib import ExitStack

    import concourse.bass as bass
    import concourse.tile as tile
    from concourse import mybir
    from concourse.bass import ts
    from concourse.masks import make_identity

    _patch_tile_tail_drain(tile)

    f32 = mybir.dt.float32
    f32r = mybir.dt.float32r
    TANH = mybir.ActivationFunctionType.Tanh
    EXP = mybir.ActivationFunctionType.Exp

    nc = bass.Bass(trn_type="TRN2", target_bir_lowering=False, debug=False)
    enc_d = nc.dram_tensor("encoder_out", [BL, N, E], f32, kind="ExternalInput")
    dec_d = nc.dram_tensor("decoder_hidden", [BL, D], f32, kind="ExternalInput")
    we_d = nc.dram_tensor("We", [E, A], f32, kind="ExternalInput")
    be_d = nc.dram_tensor("be", [A], f32, kind="ExternalInput")
    wd_d = nc.dram_tensor("Wd", [D, A], f32, kind="ExternalInput")
    bd_d = nc.dram_tensor("bd", [A], f32, kind="ExternalInput")
    wf_d = nc.dram_tensor("Wf", [A, 1], f32, kind="ExternalInput")
    ctx_d = nc.dram_tensor("context", [BL, E], f32, kind="ExternalOutput")
    alp_d = nc.dram_tensor("alpha", [BL, N, 1], f32, kind="ExternalOutput")

    WV = 392  # pair width in the transposed layout (2 * N)

    with tile.TileContext(nc) as tc, ExitStack() as ctx:
        consts = ctx.enter_context(tc.tile_pool(name="consts", bufs=1))

        ident = consts.tile([128, 128], f32)
        make_identity(nc, ident[:])
        ident_r = consts.tile([128, 128], f32r)
        nc.vector.tensor_copy(ident_r[:], ident[:])

        att2T_sb = consts.tile([128, AC, BL], f32)
        alphaT_pad = consts.tile([128, 2 * BL, BL], f32r)

        # open the main pools early so pair-0's enc loads are the first DMAs
        work = ctx.enter_context(tc.tile_pool(name="work", bufs=2))
        psum = ctx.enter_context(tc.tile_pool(name="psum", bufs=1, space="PSUM"))

        def emit_loads(p):
            # SWDGE (gpsimd) spreads each transfer across all 16 SDMA
            # engines; HWDGE rings would serialize on ~4 queues.
            b0, b1 = 2 * p, 2 * p + 1
            nat0 = work.tile([128, 2, E], f32r, tag="nat0", bufs=3, name="nat0")
            nat1 = work.tile([N1, 2, E], f32r, tag="nat1", bufs=3, name="nat1")
            # pair 0's SWDGE emission (~1us per dma_start on the Q7) is on
            # the startup critical path: use fewer, bigger transfers there
            nk, w = (2, 1024) if p == 0 else (4, 512)
            for j, b in enumerate((b0, b1)):
                for k in range(nk):
                    nc.gpsimd.dma_start(
                        nat0[:, j, ts(k, w)],
                        enc_d.ap()[b, 0:N0, ts(k, w)].bitcast(f32r),
                    )
                    nc.gpsimd.dma_start(
                        nat1[:, j, ts(k, w)],
                        enc_d.ap()[b, N0:N, ts(k, w)].bitcast(f32r),
                    )
            return nat0, nat1

        with tc.tile_pool(name="pre", bufs=1) as pre:
            wd_sb = pre.tile([128, DC, A], f32)
            for c in range(DC):
                nc.sync.dma_start(wd_sb[:, c, :], wd_d.ap()[ts(c, 128), :])
            be_sb = pre.tile([128, AC], f32)
            bd_sb = pre.tile([128, AC], f32)
            nc.sync.dma_start(be_sb[:], be_d.ap().rearrange("(c p) -> p c", p=128))
            nc.sync.dma_start(bd_sb[:], bd_d.ap().rearrange("(c p) -> p c", p=128))
            bdbe = pre.tile([128, AC], f32)
            nc.vector.tensor_add(bdbe[:], be_sb[:], bd_sb[:])

            dec_sb = pre.tile([BL, D], f32)
            nc.sync.dma_start(dec_sb[:], dec_d.ap())
            decT_sb = pre.tile([128, DC, BL], f32)
            for c in range(DC):
                ps_tp = psum.tile([128, WV], f32, tag="tp", bufs=2, name="ps_tp")
                nc.tensor.transpose(
                    ps_tp[:, 0:BL], dec_sb[:, ts(c, 128)], ident[0:BL, 0:BL]
                )
                nc.vector.tensor_copy(decT_sb[:, c, :], ps_tp[:, 0:BL])

            for ca in range(AC):
                ps_a2 = psum.tile([128, WV], f32, tag="tp", bufs=2, name="ps_a2")
                for cd in range(DC):
                    nc.tensor.matmul(
                        ps_a2[:, 0:BL],
                        wd_sb[:, cd, ts(ca, 128)],
                        decT_sb[:, cd, :],
                        start=(cd == 0),
                        stop=(cd == DC - 1),
                    )
                nc.scalar.add(att2T_sb[:, ca, :], ps_a2[:, 0:BL], bdbe[:, ca : ca + 1])

            nc.gpsimd.memset(alphaT_pad[:].bitcast(f32), 0.0)

        first_nats = emit_loads(0)

        we_sb = consts.tile([128, EC, A], f32r)
        for c in range(EC):
            nc.gpsimd.dma_start(we_sb[:, c, :], we_d.ap()[ts(c, 128), :].bitcast(f32r))
        wf_sb = consts.tile([128, AC], f32r)
        nc.sync.dma_start(
            wf_sb[:], wf_d.ap().rearrange("(c p) o -> p (c o)", p=128).bitcast(f32r)
        )

        ctx_ps = psum.tile([BL, E4, 512], f32, tag="ctx", bufs=1)

        def r(ap):
            return ap.bitcast(f32r)

        def copy_op(i, out, in_):
            if i % 2 == 0:
                nc.vector.tensor_copy(out, in_)
            else:
                nc.scalar.copy(out, in_)

        def transpose_jobs(p, nats, encT):
            """Yield closures: 4 PE transposes + 1 evacuation copy per e-chunk.
            Interleaved between att1 matmuls so the PE always has REGULAR
            matmuls in flight (transpose-mode doesn't count as HAM-busy)."""
            nat0, nat1 = nats
            state = {}

            def t_job(ce, j, half):
                def go():
                    if ce not in state:
                        state[ce] = psum.tile(
                            [128, WV], f32r, tag="tp", bufs=2, name="ps_t"
                        )
                    ps_t = state[ce]
                    off = N * j
                    if half == 0:
                        nc.tensor.transpose(
                            ps_t[:, off : off + N0],
                            nat0[:, j, ts(ce, 128)],
                            ident_r[:, :],
                        )
                    else:
                        nc.tensor.transpose(
                            ps_t[:, off + N0 : off + N],
                            nat1[:, j, ts(ce, 128)],
                            ident_r[0:N1, 0:N1],
                        )
                return go

            def c_job(ce):
                def go():
                    copy_op(ce, encT[:, ce, :], state.pop(ce)[:])
                return go

            jobs = []
            for ce in range(EC):
                for j in range(2):
                    for half in range(2):
                        jobs.append(t_job(ce, j, half))
                jobs.append(c_job(ce))
            return jobs

        def emit_transposes(p, nats):
            encT = work.tile([128, EC, WV], f32r, tag="encT", bufs=2, name="encT")
            for job in transpose_jobs(p, nats, encT):
                job()
            return encT

        def emit_att1(p, encT, ca_list, jobs=None):
            b0, b1 = 2 * p, 2 * p + 1
            att = att_tiles[p % 2]
            nmm = len(ca_list) * EC
            k = 0
            jpos = 0
            for ca in ca_list:
                ps_a = psum.tile([128, WV], f32, tag="att1", bufs=2, name="ps_a")
                for ce in range(EC):
                    nc.tensor.matmul(
                        ps_a[:],
                        we_sb[:, ce, ts(ca, 128)],
                        encT[:, ce, :],
                        start=(ce == 0),
                        stop=(ce == EC - 1),
                    )
                    if jobs:
                        k += 1
                        if k % 8 == 0:
                            jend = len(jobs) * k // nmm
                            while jpos < jend:
                                jobs[jpos]()
                                jpos += 1
                nc.scalar.activation(
                    att[:, ca, 0:N], ps_a[:, 0:N], TANH,
                    bias=att2T_sb[:, ca, b0 : b0 + 1],
                )
                nc.scalar.activation(
                    att[:, ca, N:WV], ps_a[:, N:WV], TANH,
                    bias=att2T_sb[:, ca, b1 : b1 + 1],
                )
            if jobs:
                while jpos < len(jobs):
                    jobs[jpos]()
                    jpos += 1

        def emit_scores_softmax(p):
            b0 = 2 * p
            att = att_tiles[p % 2]
            ps_s = psum.tile([128, WV], f32, tag="att1", bufs=2, name="ps_s")
            for ca in range(AC):
                nc.tensor.matmul(
                    ps_s[0:1, :],
                    wf_sb[:, ca : ca + 1],
                    att[:, ca, :],
                    start=(ca == 0),
                    stop=(ca == AC - 1),
                )
            sc_sb = work.tile([1, WV], f32, tag="sc", bufs=2, name="sc")
            nc.vector.tensor_copy(sc_sb[:], ps_s[0:1, :])

            mx = work.tile([1, 2], f32, tag="mx", bufs=2, name="mx")
            sm = work.tile([1, 2], f32, tag="sm", bufs=2, name="sm")
            rs = work.tile([1, 2], f32, tag="rs", bufs=2, name="rs")
            nc.vector.reduce_max(
                mx[:],
                sc_sb[:].rearrange("p (h w) -> p h w", h=2),
                axis=mybir.AxisListType.X,
                negate=True,
            )
            al = work.tile([1, WV], f32, tag="al", bufs=2, name="al")
            for h in range(2):
                nc.scalar.activation(
                    al[0:1, ts(h, N)], sc_sb[0:1, ts(h, N)], EXP,
                    bias=mx[0:1, h : h + 1],
                    accum_out=sm[0:1, h : h + 1],
                )
            nc.vector.reciprocal(rs[:], sm[:])
            for h in range(2):
                nc.vector.tensor_scalar_mul(
                    al[0:1, ts(h, N)], al[0:1, ts(h, N)], rs[0:1, h : h + 1]
                )
            nc.gpsimd.dma_start(
                alp_d.ap()[b0 : b0 + 2].rearrange("b n o -> o (b n)"), al[:]
            )
            return al

        def emit_alpha_ctx(p, al, nats):
            nat0, nat1 = nats
            b0, b1 = 2 * p, 2 * p + 1
            ps_al = psum.tile([128, WV], f32, tag="att1", bufs=2, name="ps_al")
            for j, b in enumerate((b0, b1)):
                off = N * j
                nc.tensor.transpose(
                    ps_al[:, 2 * j : 2 * j + 1],
                    al[0:1, off : off + N0],
                    ident[0:1, 0:1],
                )
                nc.tensor.transpose(
                    ps_al[0:N1, 2 * j + 1 : 2 * j + 2],
                    al[0:1, off + N0 : off + N],
                    ident[0:1, 0:1],
                )
                nc.vector.tensor_copy(
                    alphaT_pad[:, 2 * b, b : b + 1], ps_al[:, 2 * j : 2 * j + 1]
                )
                nc.vector.tensor_copy(
                    alphaT_pad[0:N1, 2 * b + 1, b : b + 1],
                    ps_al[0:N1, 2 * j + 1 : 2 * j + 2],
                )
            for j, b in enumerate((b0, b1)):
                for e4 in range(E4):
                    nc.tensor.matmul(
                        ctx_ps[:, e4, :],
                        r(alphaT_pad[:, 2 * b, :]),
                        nat0[:, j, ts(e4, 512)],
                        start=(p == 0 and j == 0),
                        stop=False,
                        skip_group_check=True,
                    )
                    nc.tensor.matmul(
                        ctx_ps[:, e4, :],
                        r(alphaT_pad[0:N1, 2 * b + 1, :]),
                        nat1[:, j, ts(e4, 512)],
                        start=False,
                        stop=(p == PAIRS - 1 and j == 1),
                        skip_group_check=True,
                    )

        att_one = work.tile([128, AC, WV], f32r, tag="att", bufs=1, name="att_one")
        att_tiles = [att_one, att_one]

        # Pipeline: loads 1 section ahead (SWDGE); per section:
        #   transposes(p) ; alphaT+ctx(p-1) ; att1(p) ; scores+softmax(p)
        # nat(p) lives load(p-1)..ctx(p+1) -> bufs=3. encT written and read
        # within the section (PE in-order) -> bufs=1.
        # Pipeline: loads one section ahead; transposes of pair p+1 are
        # interleaved between the att1 matmuls of pair p (2:1) so the PE
        # stream never has a transpose-only phase (HAM stays warm).
        nats = {0: first_nats}
        als = {}
        encTs = {0: emit_transposes(0, nats[0])}
        for p in range(PAIRS):
            if p + 1 < PAIRS:
                nats[p + 1] = emit_loads(p + 1)
            emit_att1(p, encTs[p], (0, 1))
            if p >= 1:
                emit_alpha_ctx(p - 1, als.pop(p - 1), nats.pop(p - 1))
            if p + 1 < PAIRS:
                encTs[p + 1] = work.tile(
                    [128, EC, WV], f32r, tag="encT", bufs=2, name="encT"
                )
                jobs = transpose_jobs(p + 1, nats[p + 1], encTs[p + 1])
            else:
                jobs = None
            emit_att1(p, encTs.pop(p), (2, 3), jobs=jobs)
            als[p] = emit_scores_softmax(p)
        emit_alpha_ctx(PAIRS - 1, als.pop(PAIRS - 1), nats.pop(PAIRS - 1))

        for e4 in range(E4):
            ctx_e = work.tile([BL, 512], f32, tag="ctxe", bufs=2, name="ctxe")
            nc.vector.tensor_copy(ctx_e[:], ctx_ps[:, e4, :])
            nc.gpsimd.dma_start(ctx_d.ap()[:, ts(e4, 512)], ctx_e[:])

    return nc


def _get_nc():
    if "nc" not in _CACHE:
        _CACHE["nc"] = build()
    return _CACHE["nc"]


def _run(in_maps, trace=False, tmpdir=None):
    from concourse.bass_utils import run_bass_kernel_spmd

    nc = _get_nc()
    return run_bass_kernel_spmd(
        nc, in_maps, core_ids=list(range(NCORES)), trace=trace, tmpdir=tmpdir
    )


def make_in_maps(encoder_out, decoder_hidden, We, be, Wd, bd, Wf, bf=None):
    enc = np.ascontiguousarray(np.asarray(encoder_out, dtype=np.float32))
    dec = np.ascontiguousarray(np.asarray(decoder_hidden, dtype=np.float32))
    We = np.ascontiguousarray(np.asarray(We, dtype=np.float32))
    be = np.ascontiguousarray(np.asarray(be, dtype=np.float32))
    Wd = np.ascontiguousarray(np.asarray(Wd, dtype=np.float32))
    bd = np.ascontiguousarray(np.asarray(bd, dtype=np.float32))
    Wf = np.ascontiguousarray(np.asarray(Wf, dtype=np.float32))
    in_maps = []
    for i in range(NCORES):
        sl = slice(i * BL, (i + 1) * BL)
        in_maps.append(
            dict(
                encoder_out=enc[sl],
                decoder_hidden=dec[sl],
                We=We,
                be=be,
                Wd=Wd,
                bd=bd,
                Wf=Wf,
            )
        )
    return in_maps


def gather(results):
    context = np.concatenate([results[i]["context"] for i in range(NCORES)], axis=0)
    alpha = np.concatenate([results[i]["alpha"] for i in range(NCORES)], axis=0)
    return context, alpha


def kernel(encoder_out, decoder_hidden, We, be, Wd, bd, Wf, bf):
    in_maps = make_in_maps(encoder_out, decoder_hidden, We, be, Wd, bd, Wf, bf)
    res = _run(in_maps, trace=False)
    _CACHE["last_results"] = res
    return gather(res.results)


# revision 53
# speedup vs baseline: 1.8652x; 1.0168x over previous
"""Bass/Trainium2 kernel for Bahdanau (additive) attention, 8-core data-parallel.

Reference computation (per batch b):
    att1 = enc[b] @ We                    # [N, A]
    att2 = dec[b] @ Wd + bd               # [A]
    att  = tanh(att1 + att2 + be)         # [N, A]
    s    = att @ Wf (+ bf)                # [N]   (bf dropped: softmax-invariant)
    alpha= softmax(s)                     # [N]
    ctx  = sum_n alpha[n] * enc[b, n]     # [E]

Sharding: batch 128 -> 8 cores x 16. Weights replicated. No collectives.

Per-core design (B_loc=16, processed in 8 pairs), all matmuls in float32r
(fp32 bits through the PE fast path, full rate at moving-dim >= 256):
  - enc loaded f32 via SWDGE (gpsimd) chunk DMAs - SWDGE spreads each
    transfer across all 16 SDMA engines (HWDGE rings serialize on ~4).
  - enc transposed on the PE (identity matmuls) into encT [e, pair-cols];
    the transposes of pair p+1 are interleaved in 8-matmul bursts between
    the att1 matmuls of pair p so the PE never has a transpose-only phase
    (transpose-mode doesn't count as HAM-busy and would re-throttle the
    2.4 GHz clock).
  - att1 computed transposed ([A_chunk=128, 392 pair-cols]): lhsT = We
    blocks (natural layout), moving = encT -> one PSUM bank per A-chunk.
  - att2+be fused into tanh on ScalarE as a per-partition bias
    (att2T precomputed once via PE from decT).
  - scores via PE: lhsT = Wf chunk [128,1], moving = tanh output.
  - softmax on one partition per pair (reduce_max negate / exp with fused
    accum_out / reciprocal), bf dropped (softmax-invariant).
  - context via PE with a zero-padded block-diagonal alphaT (each batch's
    alpha column at 256-aligned row offsets), all 16 batches accumulated
    into 4 persistent PSUM banks [16, 512]; emitted one pair late so the
    PE stream never waits on the softmax chain.
  - PSUM->SBUF evacuations alternate between DVE and ACT.

Measured on TRN2: ~250 us HW exec, rel err ~1.2e-4 vs the f32 reference.
"""

import numpy as np

B, N, E = 128, 196, 2048
D, A = 512, 512
NCORES = 8
BL = B // NCORES            # 16 batches per core
PAIRS = BL // 2             # 8
N0 = 128
N1 = N - N0                 # 68
NP = 80                     # N1 padded to a multiple of XBAR_TILE_SRC_ROWS (16)
PW = N0 + NP                # 208 cols per batch in the transposed layout
W = 2 * PW                  # 416 cols per pair
EC = E // 128               # 16
AC = A // 128               # 4
E4 = E // 512               # 4
DC = D // 128               # 4

_CACHE = {}


def _patch_tile_tail_drain(tile):
    """This walrus build rejects >1 sem-wait per instruction. Split extra
    waits onto single-wait NOPs committed just before the instruction, and
    do the same for the TileContext tail drain."""
    import concourse.mybir as mybir
    from concourse.vector_clock import ScopedClock

    if getattr(tile.TileContext, "_tail_drain_patched", False):
        return

    orig_commit = tile.TileContext._commit_instruction

    def _commit_instruction(self, inst, lazy_reg_writes=True):
        si = getattr(inst, "sync_info", None)
        engine = getattr(inst, "engine", None)
        if (
            si is not None
            and si.on_wait
            and len(si.on_wait) > 1
            and engine is not None
            and engine != mybir.EngineType.Unassigned
            and type(inst).__name__.startswith("Inst")
        ):
            waits = list(si.on_wait)
            for w in waits[:-1]:
                noop = mybir.InstNoOp(
                    name=self.nc.get_next_instruction_name(),
                    sync_info=mybir.SyncInfo(on_wait=[w], on_update=[]),
                    bass_nofuse=True,
                    engine=engine,
                )
                orig_commit(self, noop, lazy_reg_writes=False)
            inst.sync_info = mybir.SyncInfo(
                on_wait=[waits[-1]], on_update=list(si.on_update or [])
            )
        return orig_commit(self, inst, lazy_reg_writes)

    tile.TileContext._commit_instruction = _commit_instruction

    def _drain_and_barrier(self, tick_clock, wait_clock):
        nc = self.nc
        drain_inst = nc.sync.drain()
        wait_clock.add_sem_waits(
            drain_inst.ins, ScopedClock({None: tick_clock.global_clock})
        )
        si = drain_inst.ins.sync_info
        waits = list(si.on_wait or []) if si is not None else []
        if len(waits) > 1:
            drain_inst.ins.sync_info = mybir.SyncInfo(
                on_wait=waits[:1], on_update=list(si.on_update or [])
            )
            for w in waits[1:]:
                d = nc.sync.drain()
                d.ins.sync_info = mybir.SyncInfo(on_wait=[w], on_update=[])
        nc.all_engine_barrier()
        assert self.sems is not None
        popped = nc._tile_sem_poison_stack.pop()
        assert popped is self._sem_poison
        nc.clear_and_free_semaphores(list(self.sems.allocated().values()))
        nc.all_engine_barrier()

    tile.TileContext._drain_and_barrier = _drain_and_barrier
    tile.TileContext._tail_drain_patched = True


def build():
    from contextlib import ExitStack

    import concourse.bass as bass
    import concourse.tile as tile
    from concourse import mybir
    from concourse.bass import ts
    from concourse.masks import make_identity

    _patch_tile_tail_drain(tile)

    f32 = mybir.dt.float32
    f32r = mybir.dt.float32r
    TANH = mybir.ActivationFunctionType.Tanh
    EXP = mybir.ActivationFunctionType.Exp

    nc = bass.Bass(trn_type="TRN2", target_bir_lowering=False, debug=False)
    enc_d = nc.dram_tensor("encoder_out", [BL, N, E], f32, kind="ExternalInput")
    dec_d = nc.dram_tensor("decoder_hidden", [BL, D], f32, kind="ExternalInput")
    we_d = nc.dram_tensor("We", [E, A], f32, kind="ExternalInput")
    be_d = nc.dram_tensor("be", [A], f32, kind="ExternalInput")
    wd_d = nc.dram_tensor("Wd", [D, A], f32, kind="ExternalInput")
    bd_d = nc.dram_tensor("bd", [A], f32, kind="ExternalInput")
    wf_d = nc.dram_tensor("Wf", [A, 1], f32, kind="ExternalInput")
    ctx_d = nc.dram_tensor("context", [BL, E], f32, kind="ExternalOutput")
    alp_d = nc.dram_tensor("alpha", [BL, N, 1], f32, kind="ExternalOutput")

    WV = 392  # pair width in the transposed layout (2 * N)

    with tile.TileContext(nc) as tc, ExitStack() as ctx:
        consts = ctx.enter_context(tc.tile_pool(name="consts", bufs=1))

        ident = consts.tile([128, 128], f32)
        make_identity(nc, ident[:])
        ident_r = consts.tile([128, 128], f32r)
        nc.vector.tensor_copy(ident_r[:], ident[:])

        att2T_sb = consts.tile([128, AC, BL], f32)
        alphaT_pad = consts.tile([128, 2 * BL, BL], f32r)

        # open the main pools early so pair-0's enc loads are the first DMAs
        work = ctx.enter_context(tc.tile_pool(name="work", bufs=2))
        psum = ctx.enter_context(tc.tile_pool(name="psum", bufs=1, space="PSUM"))

        def emit_loads(p):
            # SWDGE (gpsimd) spreads each transfer across all 16 SDMA
            # engines; HWDGE rings would serialize on ~4 queues.
            b0, b1 = 2 * p, 2 * p + 1
            nat0 = work.tile([128, 2, E], f32r, tag="nat0", bufs=3, name="nat0")
            nat1 = work.tile([N1, 2, E], f32r, tag="nat1", bufs=3, name="nat1")
            # pair 0's SWDGE emission (~1us per dma_start on the Q7) is on
            # the startup critical path: use fewer, bigger transfers there
            nk, w = (2, 1024) if p == 0 else (4, 512)
            for j, b in enumerate((b0, b1)):
                for k in range(nk):
                    nc.gpsimd.dma_start(
                        nat0[:, j, ts(k, w)],
                        enc_d.ap()[b, 0:N0, ts(k, w)].bitcast(f32r),
                    )
                    nc.gpsimd.dma_start(
                        nat1[:, j, ts(k, w)],
                        enc_d.ap()[b, N0:N, ts(k, w)].bitcast(f32r),
                    )
            return nat0, nat1

        with tc.tile_pool(name="pre", bufs=1) as pre:
            wd_sb = pre.tile([128, DC, A], f32)
            for c in range(DC):
                nc.sync.dma_start(wd_sb[:, c, :], wd_d.ap()[ts(c, 128), :])
            be_sb = pre.tile([128, AC], f32)
            bd_sb = pre.tile([128, AC], f32)
            nc.sync.dma_start(be_sb[:], be_d.ap().rearrange("(c p) -> p c", p=128))
            nc.sync.dma_start(bd_sb[:], bd_d.ap().rearrange("(c p) -> p c", p=128))
            bdbe = pre.tile([128, AC], f32)
            nc.vector.tensor_add(bdbe[:], be_sb[:], bd_sb[:])

            dec_sb = pre.tile([BL, D], f32)
            nc.sync.dma_start(dec_sb[:], dec_d.ap())
            decT_sb = pre.tile([128, DC, BL], f32)
            for c in range(DC):
                ps_tp = psum.tile([128, WV], f32, tag="tp", bufs=2, name="ps_tp")
                nc.tensor.transpose(
                    ps_tp[:, 0:BL], dec_sb[:, ts(c, 128)], ident[0:BL, 0:BL]
                )
                nc.vector.tensor_copy(decT_sb[:, c, :], ps_tp[:, 0:BL])

            for ca in range(AC):
                ps_a2 = psum.tile([128, WV], f32, tag="tp", bufs=2, name="ps_a2")
                for cd in range(DC):
                    nc.tensor.matmul(
                        ps_a2[:, 0:BL],
                        wd_sb[:, cd, ts(ca, 128)],
                        decT_sb[:, cd, :],
                        start=(cd == 0),
                        stop=(cd == DC - 1),
                    )
                nc.scalar.add(att2T_sb[:, ca, :], ps_a2[:, 0:BL], bdbe[:, ca : ca + 1])

            nc.gpsimd.memset(alphaT_pad[:].bitcast(f32), 0.0)

        first_nats = emit_loads(0)

        we_sb = consts.tile([128, EC, A], f32r)
        for c in range(EC):
            nc.gpsimd.dma_start(we_sb[:, c, :], we_d.ap()[ts(c, 128), :].bitcast(f32r))
        wf_sb = consts.tile([128, AC], f32r)
        nc.sync.dma_start(
            wf_sb[:], wf_d.ap().rearrange("(c p) o -> p (c o)", p=128).bitcast(f32r)
        )

        ctx_ps = psum.tile([BL, E4, 512], f32, tag="ctx", bufs=1)

        def r(ap):
            return ap.bitcast(f32r)

        def copy_op(i, out, in_):
            if i % 2 == 0:
                nc.vector.tensor_copy(out, in_)
            else:
                nc.scalar.copy(out, in_)

        def transpose_jobs(p, nats, encT):
            """Yield closures: 4 PE transposes + 1 evacuation copy per e-chunk.
            Interleaved between att1 matmuls so the PE always has REGULAR
            matmuls in flight (transpose-mode doesn't count as HAM-busy)."""
            nat0, nat1 = nats
            state = {}

            def t_job(ce, j, half):
                def go():
                    if ce not in state:
                        state[ce] = psum.tile(
                            [128, WV], f32r, tag="tp", bufs=2, name="ps_t"
                        )
                    ps_t = state[ce]
                    off = N * j
                    if half == 0:
                        nc.tensor.transpose(
                            ps_t[:, off : off + N0],
                            nat0[:, j, ts(ce, 128)],
                            ident_r[:, :],
                        )
                    else:
                        nc.tensor.transpose(
                            ps_t[:, off + N0 : off + N],
                            nat1[:, j, ts(ce, 128)],
                            ident_r[0:N1, 0:N1],
                        )
                return go

            def c_job(ce):
                def go():
                    copy_op(ce, encT[:, ce, :], state.pop(ce)[:])
                return go

            jobs = []
            for ce in range(EC):
                for j in range(2):
                    for half in range(2):
                        jobs.append(t_job(ce, j, half))
                jobs.append(c_job(ce))
            return jobs

        def emit_transposes(p, nats):
            encT = work.tile([128, EC, WV], f32r, tag="encT", bufs=2, name="encT")
            for job in transpose_jobs(p, nats, encT):
                job()
            return encT

        def emit_att1(p, encT, ca_list, jobs=None):
            b0, b1 = 2 * p, 2 * p + 1
            att = att_tiles[p % 2]
            nmm = len(ca_list) * EC
            k = 0
            jpos = 0
            for ca in ca_list:
                ps_a = psum.tile([128, WV], f32, tag="att1", bufs=2, name="ps_a")
                for ce in range(EC):
                    nc.tensor.matmul(
                        ps_a[:],
                        we_sb[:, ce, ts(ca, 128)],
                        encT[:, ce, :],
                        start=(ce == 0),
                        stop=(ce == EC - 1),
                    )
                    if jobs:
                        k += 1
                        if k % 8 == 0:
                            jend = len(jobs) * k // nmm
                            while jpos < jend:
                                jobs[jpos]()
                                jpos += 1
                nc.scalar.activation(
                    att[:, ca, 0:N], ps_a[:, 0:N], TANH,
                    bias=att2T_sb[:, ca, b0 : b0 + 1],
                )
                nc.scalar.activation(
                    att[:, ca, N:WV], ps_a[:, N:WV], TANH,
                    bias=att2T_sb[:, ca, b1 : b1 + 1],
                )
            if jobs:
                while jpos < len(jobs):
                    jobs[jpos]()
                    jpos += 1

        def emit_scores_softmax(p):
            b0 = 2 * p
            att = att_tiles[p % 2]
            ps_s = psum.tile([128, WV], f32, tag="att1", bufs=2, name="ps_s")
            for ca in range(AC):
                nc.tensor.matmul(
                    ps_s[0:1, :],
                    wf_sb[:, ca : ca + 1],
                    att[:, ca, :],
                    start=(ca == 0),
                    stop=(ca == AC - 1),
                )
            sc_sb = work.tile([1, WV], f32, tag="sc", bufs=2, name="sc")
            nc.vector.tensor_copy(sc_sb[:], ps_s[0:1, :])

            mx = work.tile([1, 2], f32, tag="mx", bufs=2, name="mx")
            sm = work.tile([1, 2], f32, tag="sm", bufs=2, name="sm")
            rs = work.tile([1, 2], f32, tag="rs", bufs=2, name="rs")
            nc.vector.reduce_max(
                mx[:],
                sc_sb[:].rearrange("p (h w) -> p h w", h=2),
                axis=mybir.AxisListType.X,
                negate=True,
            )
            al = work.tile([1, WV], f32, tag="al", bufs=2, name="al")
            for h in range(2):
                nc.scalar.activation(
                    al[0:1, ts(h, N)], sc_sb[0:1, ts(h, N)], EXP,
                    bias=mx[0:1, h : h + 1],
                    accum_out=sm[0:1, h : h + 1],
                )
            nc.vector.reciprocal(rs[:], sm[:])
            for h in range(2):
                nc.vector.tensor_scalar_mul(
                    al[0:1, ts(h, N)], al[0:1, ts(h, N)], rs[0:1, h : h + 1]
                )
            nc.gpsimd.dma_start(
                alp_d.ap()[b0 : b0 + 2].rearrange("b n o -> o (b n)"), al[:]
            )
            return al

        def emit_alpha_ctx(p, al, nats):
            nat0, nat1 = nats
            b0, b1 = 2 * p, 2 * p + 1
            ps_al = psum.tile([128, WV], f32, tag="att1", bufs=2, name="ps_al")
            for j, b in enumerate((b0, b1)):
                off = N * j
                nc.tensor.transpose(
                    ps_al[:, 2 * j : 2 * j + 1],
                    al[0:1, off : off + N0],
                    ident[0:1, 0:1],
                )
                nc.tensor.transpose(
                    ps_al[0:N1, 2 * j + 1 : 2 * j + 2],
                    al[0:1, off + N0 : off + N],
                    ident[0:1, 0:1],
                )
                nc.vector.tensor_copy(
                    alphaT_pad[:, 2 * b, b : b + 1], ps_al[:, 2 * j : 2 * j + 1]
                )
                nc.vector.tensor_copy(
                    alphaT_pad[0:N1, 2 * b + 1, b : b + 1],
                    ps_al[0:N1, 2 * j + 1 : 2 * j + 2],
                )
            for j, b in enumerate((b0, b1)):
                for e4 in range(E4):
                    nc.tensor.matmul(
                        ctx_ps[:, e4, :],
                        r(alphaT_pad[:, 2 * b, :]),
                        nat0[:, j, ts(e4, 512)],
                        start=(p == 0 and j == 0),
                        stop=False,
                        skip_group_check=True,
                    )
                    nc.tensor.matmul(
                        ctx_ps[:, e4, :],
                        r(alphaT_pad[0:N1, 2 * b + 1, :]),
                        nat1[:, j, ts(e4, 512)],
                        start=False,
                        stop=(p == PAIRS - 1 and j == 1),
                        skip_group_check=True,
                    )

        att_one = work.tile([128, AC, WV], f32r, tag="att", bufs=1, name="att_one")
        att_tiles = [att_one, att_one]

        # Pipeline: loads 1 section ahead (SWDGE); per section:
        #   transposes(p) ; alphaT+ctx(p-1) ; att1(p) ; scores+softmax(p)
        # nat(p) lives load(p-1)..ctx(p+1) -> bufs=3. encT written and read
        # within the section (PE in-order) -> bufs=1.
        # Pipeline: loads one section ahead; transposes of pair p+1 are
        # interleaved between the att1 matmuls of pair p (2:1) so the PE
        # stream never has a transpose-only phase (HAM stays warm).
        nats = {0: first_nats}
        als = {}
        encTs = {0: emit_transposes(0, nats[0])}
        for p in range(PAIRS):
            if p + 1 < PAIRS:
                nats[p + 1] = emit_loads(p + 1)
            emit_att1(p, encTs[p], (0, 1))
            if p >= 1:
                emit_alpha_ctx(p - 1, als.pop(p - 1), nats.pop(p - 1))
            if p + 1 < PAIRS:
                encTs[p + 1] = work.tile(
                    [128, EC, WV], f32r, tag="encT", bufs=2, name="encT"
                )
                jobs = transpose_jobs(p + 1, nats[p + 1], encTs[p + 1])
            else:
                jobs = None
            emit_att1(p, encTs.pop(p), (2, 3), jobs=jobs)
            als[p] = emit_scores_softmax(p)
        emit_alpha_ctx(PAIRS - 1, als.pop(PAIRS - 1), nats.pop(PAIRS - 1))

        for e4 in range(E4):
            ctx_e = work.tile([BL, 512], f32, tag="ctxe", bufs=2, name="ctxe")
            nc.vector.tensor_copy(ctx_e[:], ctx_ps[:, e4, :])
            nc.gpsimd.dma_start(ctx_d.ap()[:, ts(e4, 512)], ctx_e[:])

    return nc


def _get_nc():
    if "nc" not in _CACHE:
        _CACHE["nc"] = build()
    return _CACHE["nc"]


def _run(in_maps, trace=False, tmpdir=None):
    from concourse.bass_utils import run_bass_kernel_spmd

    nc = _get_nc()
    return run_bass_kernel_spmd(
        nc, in_maps, core_ids=list(range(NCORES)), trace=trace, tmpdir=tmpdir
    )


def make_in_maps(encoder_out, decoder_hidden, We, be, Wd, bd, Wf, bf=None):
    enc = np.ascontiguousarray(np.asarray(encoder_out, dtype=np.float32))
    dec = np.ascontiguousarray(np.asarray(decoder_hidden, dtype=np.float32))
    We = np.ascontiguousarray(np.asarray(We, dtype=np.float32))
    be = np.ascontiguousarray(np.asarray(be, dtype=np.float32))
    Wd = np.ascontiguousarray(np.asarray(Wd, dtype=np.float32))
    bd = np.ascontiguousarray(np.asarray(bd, dtype=np.float32))
    Wf = np.ascontiguousarray(np.asarray(Wf, dtype=np.float32))
    in_maps = []
    for i in range(NCORES):
        sl = slice(i * BL, (i + 1) * BL)
        in_maps.append(
            dict(
                encoder_out=enc[sl],
                decoder_hidden=dec[sl],
                We=We,
                be=be,
                Wd=Wd,
                bd=bd,
                Wf=Wf,
            )
        )
    return in_maps


def gather(results):
    context = np.concatenate([results[i]["context"] for i in range(NCORES)], axis=0)
    alpha = np.concatenate([results[i]["alpha"] for i in range(NCORES)], axis=0)
    return context, alpha


def kernel(encoder_out, decoder_hidden, We, be, Wd, bd, Wf, bf):
    in_maps = make_in_maps(encoder_out, decoder_hidden, We, be, Wd, bd, Wf, bf)
    res = _run(in_maps, trace=False)
    _CACHE["last_results"] = res
    return gather(res.results)


# revision 54
# speedup vs baseline: 1.8978x; 1.0175x over previous
"""Bass/Trainium2 kernel for Bahdanau (additive) attention, 8-core data-parallel.

Reference computation (per batch b):
    att1 = enc[b] @ We                    # [N, A]
    att2 = dec[b] @ Wd + bd               # [A]
    att  = tanh(att1 + att2 + be)         # [N, A]
    s    = att @ Wf (+ bf)                # [N]   (bf dropped: softmax-invariant)
    alpha= softmax(s)                     # [N]
    ctx  = sum_n alpha[n] * enc[b, n]     # [E]

Sharding: batch 128 -> 8 cores x 16. Weights replicated. No collectives.

Per-core design (B_loc=16, processed in 8 pairs), all matmuls in float32r
(fp32 bits through the PE fast path, full rate at moving-dim >= 256):
  - enc loaded f32 via SWDGE (gpsimd) chunk DMAs - SWDGE spreads each
    transfer across all 16 SDMA engines (HWDGE rings serialize on ~4).
  - enc transposed on the PE (identity matmuls) into encT [e, pair-cols];
    the transposes of pair p+1 are interleaved in 8-matmul bursts between
    the att1 matmuls of pair p so the PE never has a transpose-only phase
    (transpose-mode doesn't count as HAM-busy and would re-throttle the
    2.4 GHz clock).
  - att1 computed transposed ([A_chunk=128, 392 pair-cols]): lhsT = We
    blocks (natural layout), moving = encT -> one PSUM bank per A-chunk.
  - att2+be fused into tanh on ScalarE as a per-partition bias
    (att2T precomputed once via PE from decT).
  - scores via PE: lhsT = Wf chunk [128,1], moving = tanh output.
  - softmax on one partition per pair (reduce_max negate / exp with fused
    accum_out / reciprocal), bf dropped (softmax-invariant).
  - context via PE with a zero-padded block-diagonal alphaT (each batch's
    alpha column at 256-aligned row offsets), all 16 batches accumulated
    into 4 persistent PSUM banks [16, 512]; emitted one pair late so the
    PE stream never waits on the softmax chain.
  - PSUM->SBUF evacuations alternate between DVE and ACT.

Measured on TRN2: ~250 us HW exec, rel err ~1.2e-4 vs the f32 reference.
"""

import numpy as np

B, N, E = 128, 196, 2048
D, A = 512, 512
NCORES = 8
BL = B // NCORES            # 16 batches per core
PAIRS = BL // 2             # 8
N0 = 128
N1 = N - N0                 # 68
NP = 80                     # N1 padded to a multiple of XBAR_TILE_SRC_ROWS (16)
PW = N0 + NP                # 208 cols per batch in the transposed layout
W = 2 * PW                  # 416 cols per pair
EC = E // 128               # 16
AC = A // 128               # 4
E4 = E // 512               # 4
DC = D // 128               # 4

_CACHE = {}


def _patch_tile_tail_drain(tile):
    """This walrus build rejects >1 sem-wait per instruction. Split extra
    waits onto single-wait NOPs committed just before the instruction, and
    do the same for the TileContext tail drain."""
    import concourse.mybir as mybir
    from concourse.vector_clock import ScopedClock

    if getattr(tile.TileContext, "_tail_drain_patched", False):
        return

    orig_commit = tile.TileContext._commit_instruction

    def _commit_instruction(self, inst, lazy_reg_writes=True):
        si = getattr(inst, "sync_info", None)
        engine = getattr(inst, "engine", None)
        if (
            si is not None
            and si.on_wait
            and len(si.on_wait) > 1
            and engine is not None
            and engine != mybir.EngineType.Unassigned
            and type(inst).__name__.startswith("Inst")
        ):
            waits = list(si.on_wait)
            for w in waits[:-1]:
                noop = mybir.InstNoOp(
                    name=self.nc.get_next_instruction_name(),
                    sync_info=mybir.SyncInfo(on_wait=[w], on_update=[]),
                    bass_nofuse=True,
                    engine=engine,
                )
                orig_commit(self, noop, lazy_reg_writes=False)
            inst.sync_info = mybir.SyncInfo(
                on_wait=[waits[-1]], on_update=list(si.on_update or [])
            )
        return orig_commit(self, inst, lazy_reg_writes)

    tile.TileContext._commit_instruction = _commit_instruction

    def _drain_and_barrier(self, tick_clock, wait_clock):
        nc = self.nc
        drain_inst = nc.sync.drain()
        wait_clock.add_sem_waits(
            drain_inst.ins, ScopedClock({None: tick_clock.global_clock})
        )
        si = drain_inst.ins.sync_info
        waits = list(si.on_wait or []) if si is not None else []
        if len(waits) > 1:
            drain_inst.ins.sync_info = mybir.SyncInfo(
                on_wait=waits[:1], on_update=list(si.on_update or [])
            )
            for w in waits[1:]:
                d = nc.sync.drain()
                d.ins.sync_info = mybir.SyncInfo(on_wait=[w], on_update=[])
        nc.all_engine_barrier()
        assert self.sems is not None
        popped = nc._tile_sem_poison_stack.pop()
        assert popped is self._sem_poison
        nc.clear_and_free_semaphores(list(self.sems.allocated().values()))
        nc.all_engine_barrier()

    tile.TileContext._drain_and_barrier = _drain_and_barrier
    tile.TileContext._tail_drain_patched = True


def build():
    from contextlib import ExitStack

    import concourse.bass as bass
    import concourse.tile as tile
    from concourse import mybir
    from concourse.bass import ts
    from concourse.masks import make_identity

    _patch_tile_tail_drain(tile)

    f32 = mybir.dt.float32
    f32r = mybir.dt.float32r
    TANH = mybir.ActivationFunctionType.Tanh
    EXP = mybir.ActivationFunctionType.Exp

    nc = bass.Bass(trn_type="TRN2", target_bir_lowering=False, debug=False)
    enc_d = nc.dram_tensor("encoder_out", [BL, N, E], f32, kind="ExternalInput")
    dec_d = nc.dram_tensor("decoder_hidden", [BL, D], f32, kind="ExternalInput")
    we_d = nc.dram_tensor("We", [E, A], f32, kind="ExternalInput")
    be_d = nc.dram_tensor("be", [A], f32, kind="ExternalInput")
    wd_d = nc.dram_tensor("Wd", [D, A], f32, kind="ExternalInput")
    bd_d = nc.dram_tensor("bd", [A], f32, kind="ExternalInput")
    wf_d = nc.dram_tensor("Wf", [A, 1], f32, kind="ExternalInput")
    ctx_d = nc.dram_tensor("context", [BL, E], f32, kind="ExternalOutput")
    alp_d = nc.dram_tensor("alpha", [BL, N, 1], f32, kind="ExternalOutput")

    WV = 392  # pair width in the transposed layout (2 * N)

    with tile.TileContext(nc) as tc, ExitStack() as ctx:
        consts = ctx.enter_context(tc.tile_pool(name="consts", bufs=1))

        ident = consts.tile([128, 128], f32)
        make_identity(nc, ident[:])
        ident_r = consts.tile([128, 128], f32r)
        nc.vector.tensor_copy(ident_r[:], ident[:])

        att2T_sb = consts.tile([128, AC, BL], f32)
        alphaT_pad = consts.tile([128, 2 * BL, BL], f32r)

        # open the main pools early so pair-0's enc loads are the first DMAs
        work = ctx.enter_context(tc.tile_pool(name="work", bufs=2))
        psum = ctx.enter_context(tc.tile_pool(name="psum", bufs=1, space="PSUM"))

        def emit_loads(p):
            # SWDGE (gpsimd) spreads each transfer across all 16 SDMA
            # engines; HWDGE rings would serialize on ~4 queues.
            b0, b1 = 2 * p, 2 * p + 1
            nat0 = work.tile([128, 2, E], f32r, tag="nat0", bufs=3, name="nat0")
            nat1 = work.tile([N1, 2, E], f32r, tag="nat1", bufs=3, name="nat1")
            # pair 0's SWDGE emission (~1us per dma_start on the Q7) is on
            # the startup critical path: use fewer, bigger transfers there
            nk, w = (2, 1024) if p == 0 else (4, 512)
            for j, b in enumerate((b0, b1)):
                for k in range(nk):
                    nc.gpsimd.dma_start(
                        nat0[:, j, ts(k, w)],
                        enc_d.ap()[b, 0:N0, ts(k, w)].bitcast(f32r),
                    )
                    nc.gpsimd.dma_start(
                        nat1[:, j, ts(k, w)],
                        enc_d.ap()[b, N0:N, ts(k, w)].bitcast(f32r),
                    )
            return nat0, nat1

        with tc.tile_pool(name="pre", bufs=1) as pre:
            wd_sb = pre.tile([128, DC, A], f32)
            for c in range(DC):
                nc.gpsimd.dma_start(wd_sb[:, c, :], wd_d.ap()[ts(c, 128), :])
            be_sb = pre.tile([128, AC], f32)
            bd_sb = pre.tile([128, AC], f32)
            nc.sync.dma_start(be_sb[:], be_d.ap().rearrange("(c p) -> p c", p=128))
            nc.sync.dma_start(bd_sb[:], bd_d.ap().rearrange("(c p) -> p c", p=128))
            bdbe = pre.tile([128, AC], f32)
            nc.vector.tensor_add(bdbe[:], be_sb[:], bd_sb[:])

            dec_sb = pre.tile([BL, D], f32)
            nc.gpsimd.dma_start(dec_sb[:], dec_d.ap())
            decT_sb = pre.tile([128, DC, BL], f32)
            for c in range(DC):
                ps_tp = psum.tile([128, WV], f32, tag="tp", bufs=2, name="ps_tp")
                nc.tensor.transpose(
                    ps_tp[:, 0:BL], dec_sb[:, ts(c, 128)], ident[0:BL, 0:BL]
                )
                nc.vector.tensor_copy(decT_sb[:, c, :], ps_tp[:, 0:BL])

            for ca in range(AC):
                ps_a2 = psum.tile([128, WV], f32, tag="tp", bufs=2, name="ps_a2")
                for cd in range(DC):
                    nc.tensor.matmul(
                        ps_a2[:, 0:BL],
                        wd_sb[:, cd, ts(ca, 128)],
                        decT_sb[:, cd, :],
                        start=(cd == 0),
                        stop=(cd == DC - 1),
                    )
                nc.scalar.add(att2T_sb[:, ca, :], ps_a2[:, 0:BL], bdbe[:, ca : ca + 1])

            nc.gpsimd.memset(alphaT_pad[:].bitcast(f32), 0.0)

        first_nats = emit_loads(0)

        we_sb = consts.tile([128, EC, A], f32r)
        for c in range(EC):
            nc.gpsimd.dma_start(we_sb[:, c, :], we_d.ap()[ts(c, 128), :].bitcast(f32r))
        wf_sb = consts.tile([128, AC], f32r)
        nc.sync.dma_start(
            wf_sb[:], wf_d.ap().rearrange("(c p) o -> p (c o)", p=128).bitcast(f32r)
        )

        ctx_ps = psum.tile([BL, E4, 512], f32, tag="ctx", bufs=1)

        def r(ap):
            return ap.bitcast(f32r)

        def copy_op(i, out, in_):
            if i % 2 == 0:
                nc.vector.tensor_copy(out, in_)
            else:
                nc.scalar.copy(out, in_)

        def transpose_jobs(p, nats, encT):
            """Yield closures: 4 PE transposes + 1 evacuation copy per e-chunk.
            Interleaved between att1 matmuls so the PE always has REGULAR
            matmuls in flight (transpose-mode doesn't count as HAM-busy)."""
            nat0, nat1 = nats
            state = {}

            def t_job(ce, j, half):
                def go():
                    if ce not in state:
                        state[ce] = psum.tile(
                            [128, WV], f32r, tag="tp", bufs=2, name="ps_t"
                        )
                    ps_t = state[ce]
                    off = N * j
                    if half == 0:
                        nc.tensor.transpose(
                            ps_t[:, off : off + N0],
                            nat0[:, j, ts(ce, 128)],
                            ident_r[:, :],
                        )
                    else:
                        nc.tensor.transpose(
                            ps_t[:, off + N0 : off + N],
                            nat1[:, j, ts(ce, 128)],
                            ident_r[0:N1, 0:N1],
                        )
                return go

            def c_job(ce):
                def go():
                    copy_op(ce, encT[:, ce, :], state.pop(ce)[:])
                return go

            jobs = []
            for ce in range(EC):
                for j in range(2):
                    for half in range(2):
                        jobs.append(t_job(ce, j, half))
                jobs.append(c_job(ce))
            return jobs

        def emit_transposes(p, nats):
            encT = work.tile([128, EC, WV], f32r, tag="encT", bufs=2, name="encT")
            for job in transpose_jobs(p, nats, encT):
                job()
            return encT

        def emit_att1(p, encT, ca_list, jobs=None):
            b0, b1 = 2 * p, 2 * p + 1
            att = att_tiles[p % 2]
            nmm = len(ca_list) * EC
            k = 0
            jpos = 0
            for ca in ca_list:
                ps_a = psum.tile([128, WV], f32, tag="att1", bufs=2, name="ps_a")
                for ce in range(EC):
                    nc.tensor.matmul(
                        ps_a[:],
                        we_sb[:, ce, ts(ca, 128)],
                        encT[:, ce, :],
                        start=(ce == 0),
                        stop=(ce == EC - 1),
                    )
                    if jobs:
                        k += 1
                        if k % 8 == 0:
                            jend = len(jobs) * k // nmm
                            while jpos < jend:
                                jobs[jpos]()
                                jpos += 1
                nc.scalar.activation(
                    att[:, ca, 0:N], ps_a[:, 0:N], TANH,
                    bias=att2T_sb[:, ca, b0 : b0 + 1],
                )
                nc.scalar.activation(
                    att[:, ca, N:WV], ps_a[:, N:WV], TANH,
                    bias=att2T_sb[:, ca, b1 : b1 + 1],
                )
            if jobs:
                while jpos < len(jobs):
                    jobs[jpos]()
                    jpos += 1

        def emit_scores_softmax(p):
            b0 = 2 * p
            att = att_tiles[p % 2]
            ps_s = psum.tile([128, WV], f32, tag="att1", bufs=2, name="ps_s")
            for ca in range(AC):
                nc.tensor.matmul(
                    ps_s[0:1, :],
                    wf_sb[:, ca : ca + 1],
                    att[:, ca, :],
                    start=(ca == 0),
                    stop=(ca == AC - 1),
                )
            sc_sb = work.tile([1, WV], f32, tag="sc", bufs=2, name="sc")
            nc.vector.tensor_copy(sc_sb[:], ps_s[0:1, :])

            mx = work.tile([1, 2], f32, tag="mx", bufs=2, name="mx")
            sm = work.tile([1, 2], f32, tag="sm", bufs=2, name="sm")
            rs = work.tile([1, 2], f32, tag="rs", bufs=2, name="rs")
            nc.vector.reduce_max(
                mx[:],
                sc_sb[:].rearrange("p (h w) -> p h w", h=2),
                axis=mybir.AxisListType.X,
                negate=True,
            )
            al = work.tile([1, WV], f32, tag="al", bufs=2, name="al")
            for h in range(2):
                nc.scalar.activation(
                    al[0:1, ts(h, N)], sc_sb[0:1, ts(h, N)], EXP,
                    bias=mx[0:1, h : h + 1],
                    accum_out=sm[0:1, h : h + 1],
                )
            nc.vector.reciprocal(rs[:], sm[:])
            for h in range(2):
                nc.vector.tensor_scalar_mul(
                    al[0:1, ts(h, N)], al[0:1, ts(h, N)], rs[0:1, h : h + 1]
                )
            nc.gpsimd.dma_start(
                alp_d.ap()[b0 : b0 + 2].rearrange("b n o -> o (b n)"), al[:]
            )
            return al

        def emit_alpha_ctx(p, al, nats):
            nat0, nat1 = nats
            b0, b1 = 2 * p, 2 * p + 1
            ps_al = psum.tile([128, WV], f32, tag="att1", bufs=2, name="ps_al")
            for j, b in enumerate((b0, b1)):
                off = N * j
                nc.tensor.transpose(
                    ps_al[:, 2 * j : 2 * j + 1],
                    al[0:1, off : off + N0],
                    ident[0:1, 0:1],
                )
                nc.tensor.transpose(
                    ps_al[0:N1, 2 * j + 1 : 2 * j + 2],
                    al[0:1, off + N0 : off + N],
                    ident[0:1, 0:1],
                )
                nc.vector.tensor_copy(
                    alphaT_pad[:, 2 * b, b : b + 1], ps_al[:, 2 * j : 2 * j + 1]
                )
                nc.vector.tensor_copy(
                    alphaT_pad[0:N1, 2 * b + 1, b : b + 1],
                    ps_al[0:N1, 2 * j + 1 : 2 * j + 2],
                )
            for j, b in enumerate((b0, b1)):
                for e4 in range(E4):
                    nc.tensor.matmul(
                        ctx_ps[:, e4, :],
                        r(alphaT_pad[:, 2 * b, :]),
                        nat0[:, j, ts(e4, 512)],
                        start=(p == 0 and j == 0),
                        stop=False,
                        skip_group_check=True,
                    )
                    nc.tensor.matmul(
                        ctx_ps[:, e4, :],
                        r(alphaT_pad[0:N1, 2 * b + 1, :]),
                        nat1[:, j, ts(e4, 512)],
                        start=False,
                        stop=(p == PAIRS - 1 and j == 1),
                        skip_group_check=True,
                    )

        att_one = work.tile([128, AC, WV], f32r, tag="att", bufs=1, name="att_one")
        att_tiles = [att_one, att_one]

        # Pipeline: loads 1 section ahead (SWDGE); per section:
        #   transposes(p) ; alphaT+ctx(p-1) ; att1(p) ; scores+softmax(p)
        # nat(p) lives load(p-1)..ctx(p+1) -> bufs=3. encT written and read
        # within the section (PE in-order) -> bufs=1.
        # Pipeline: loads one section ahead; transposes of pair p+1 are
        # interleaved between the att1 matmuls of pair p (2:1) so the PE
        # stream never has a transpose-only phase (HAM stays warm).
        nats = {0: first_nats}
        als = {}
        encTs = {0: emit_transposes(0, nats[0])}
        for p in range(PAIRS):
            if p + 1 < PAIRS:
                nats[p + 1] = emit_loads(p + 1)
            emit_att1(p, encTs[p], (0, 1))
            if p >= 1:
                emit_alpha_ctx(p - 1, als.pop(p - 1), nats.pop(p - 1))
            if p + 1 < PAIRS:
                encTs[p + 1] = work.tile(
                    [128, EC, WV], f32r, tag="encT", bufs=2, name="encT"
                )
                jobs = transpose_jobs(p + 1, nats[p + 1], encTs[p + 1])
            else:
                jobs = None
            emit_att1(p, encTs.pop(p), (2, 3), jobs=jobs)
            als[p] = emit_scores_softmax(p)
        emit_alpha_ctx(PAIRS - 1, als.pop(PAIRS - 1), nats.pop(PAIRS - 1))

        for e4 in range(E4):
            ctx_e = work.tile([BL, 512], f32, tag="ctxe", bufs=2, name="ctxe")
            copy_op(e4, ctx_e[:], ctx_ps[:, e4, :])
            nc.gpsimd.dma_start(ctx_d.ap()[:, ts(e4, 512)], ctx_e[:])

    return nc


def _get_nc():
    if "nc" not in _CACHE:
        _CACHE["nc"] = build()
    return _CACHE["nc"]


def _run(in_maps, trace=False, tmpdir=None):
    from concourse.bass_utils import run_bass_kernel_spmd

    nc = _get_nc()
    return run_bass_kernel_spmd(
        nc, in_maps, core_ids=list(range(NCORES)), trace=trace, tmpdir=tmpdir
    )


def make_in_maps(encoder_out, decoder_hidden, We, be, Wd, bd, Wf, bf=None):
    enc = np.ascontiguousarray(np.asarray(encoder_out, dtype=np.float32))
    dec = np.ascontiguousarray(np.asarray(decoder_hidden, dtype=np.float32))
    We = np.ascontiguousarray(np.asarray(We, dtype=np.float32))
    be = np.ascontiguousarray(np.asarray(be, dtype=np.float32))
    Wd = np.ascontiguousarray(np.asarray(Wd, dtype=np.float32))
    bd = np.ascontiguousarray(np.asarray(bd, dtype=np.float32))
    Wf = np.ascontiguousarray(np.asarray(Wf, dtype=np.float32))
    in_maps = []
    for i in range(NCORES):
        sl = slice(i * BL, (i + 1) * BL)
        in_maps.append(
            dict(
                encoder_out=enc[sl],
                decoder_hidden=dec[sl],
                We=We,
                be=be,
                Wd=Wd,
                bd=bd,
                Wf=Wf,
            )
        )
    return in_maps


def gather(results):
    context = np.concatenate([results[i]["context"] for i in range(NCORES)], axis=0)
    alpha = np.concatenate([results[i]["alpha"] for i in range(NCORES)], axis=0)
    return context, alpha


def kernel(encoder_out, decoder_hidden, We, be, Wd, bd, Wf, bf):
    in_maps = make_in_maps(encoder_out, decoder_hidden, We, be, Wd, bd, Wf, bf)
    res = _run(in_maps, trace=False)
    _CACHE["last_results"] = res
    return gather(res.results)


# revision 55
# speedup vs baseline: 1.9291x; 1.0165x over previous
"""Bass/Trainium2 kernel for Bahdanau (additive) attention, 8-core data-parallel.

Reference computation (per batch b):
    att1 = enc[b] @ We                    # [N, A]
    att2 = dec[b] @ Wd + bd               # [A]
    att  = tanh(att1 + att2 + be)         # [N, A]
    s    = att @ Wf (+ bf)                # [N]   (bf dropped: softmax-invariant)
    alpha= softmax(s)                     # [N]
    ctx  = sum_n alpha[n] * enc[b, n]     # [E]

Sharding: batch 128 -> 8 cores x 16. Weights replicated. No collectives.

Per-core design (B_loc=16, processed in 8 pairs), all matmuls in float32r
(fp32 bits through the PE fast path, full rate at moving-dim >= 256):
  - enc loaded f32 via SWDGE (gpsimd) chunk DMAs - SWDGE spreads each
    transfer across all 16 SDMA engines (HWDGE rings serialize on ~4).
  - enc transposed on the PE (identity matmuls) into encT [e, pair-cols];
    the transposes of pair p+1 are interleaved in 8-matmul bursts between
    the att1 matmuls of pair p so the PE never has a transpose-only phase
    (transpose-mode doesn't count as HAM-busy and would re-throttle the
    2.4 GHz clock).
  - att1 computed transposed ([A_chunk=128, 392 pair-cols]): lhsT = We
    blocks (natural layout), moving = encT -> one PSUM bank per A-chunk.
  - att2+be fused into tanh on ScalarE as a per-partition bias
    (att2T precomputed once via PE from decT).
  - scores via PE: lhsT = Wf chunk [128,1], moving = tanh output.
  - softmax on one partition per pair (reduce_max negate / exp with fused
    accum_out / reciprocal), bf dropped (softmax-invariant).
  - context via PE with a zero-padded block-diagonal alphaT (each batch's
    alpha column at 256-aligned row offsets), all 16 batches accumulated
    into 4 persistent PSUM banks [16, 512]; emitted one pair late so the
    PE stream never waits on the softmax chain.
  - PSUM->SBUF evacuations alternate between DVE and ACT.

Measured on TRN2: ~242 us HW exec, rel err ~1.2e-4 vs the f32 reference.
"""

import numpy as np

B, N, E = 128, 196, 2048
D, A = 512, 512
NCORES = 8
BL = B // NCORES            # 16 batches per core
PAIRS = BL // 2             # 8
N0 = 128
N1 = N - N0                 # 68
NP = 80                     # N1 padded to a multiple of XBAR_TILE_SRC_ROWS (16)
PW = N0 + NP                # 208 cols per batch in the transposed layout
W = 2 * PW                  # 416 cols per pair
EC = E // 128               # 16
AC = A // 128               # 4
E4 = E // 512               # 4
DC = D // 128               # 4

_CACHE = {}


def _patch_tile_tail_drain(tile):
    """This walrus build rejects >1 sem-wait per instruction. Split extra
    waits onto single-wait NOPs committed just before the instruction, and
    do the same for the TileContext tail drain."""
    import concourse.mybir as mybir
    from concourse.vector_clock import ScopedClock

    if getattr(tile.TileContext, "_tail_drain_patched", False):
        return

    orig_commit = tile.TileContext._commit_instruction

    def _commit_instruction(self, inst, lazy_reg_writes=True):
        si = getattr(inst, "sync_info", None)
        engine = getattr(inst, "engine", None)
        if (
            si is not None
            and si.on_wait
            and len(si.on_wait) > 1
            and engine is not None
            and engine != mybir.EngineType.Unassigned
            and type(inst).__name__.startswith("Inst")
        ):
            waits = list(si.on_wait)
            for w in waits[:-1]:
                noop = mybir.InstNoOp(
                    name=self.nc.get_next_instruction_name(),
                    sync_info=mybir.SyncInfo(on_wait=[w], on_update=[]),
                    bass_nofuse=True,
                    engine=engine,
                )
                orig_commit(self, noop, lazy_reg_writes=False)
            inst.sync_info = mybir.SyncInfo(
                on_wait=[waits[-1]], on_update=list(si.on_update or [])
            )
        return orig_commit(self, inst, lazy_reg_writes)

    tile.TileContext._commit_instruction = _commit_instruction

    def _drain_and_barrier(self, tick_clock, wait_clock):
        nc = self.nc
        drain_inst = nc.sync.drain()
        wait_clock.add_sem_waits(
            drain_inst.ins, ScopedClock({None: tick_clock.global_clock})
        )
        si = drain_inst.ins.sync_info
        waits = list(si.on_wait or []) if si is not None else []
        if len(waits) > 1:
            drain_inst.ins.sync_info = mybir.SyncInfo(
                on_wait=waits[:1], on_update=list(si.on_update or [])
            )
            for w in waits[1:]:
                d = nc.sync.drain()
                d.ins.sync_info = mybir.SyncInfo(on_wait=[w], on_update=[])
        nc.all_engine_barrier()
        assert self.sems is not None
        popped = nc._tile_sem_poison_stack.pop()
        assert popped is self._sem_poison
        nc.clear_and_free_semaphores(list(self.sems.allocated().values()))
        nc.all_engine_barrier()

    tile.TileContext._drain_and_barrier = _drain_and_barrier
    tile.TileContext._tail_drain_patched = True


def build():
    from contextlib import ExitStack

    import concourse.bass as bass
    import concourse.tile as tile
    from concourse import mybir
    from concourse.bass import ts
    from concourse.masks import make_identity

    _patch_tile_tail_drain(tile)

    f32 = mybir.dt.float32
    f32r = mybir.dt.float32r
    TANH = mybir.ActivationFunctionType.Tanh
    EXP = mybir.ActivationFunctionType.Exp

    nc = bass.Bass(trn_type="TRN2", target_bir_lowering=False, debug=False)
    enc_d = nc.dram_tensor("encoder_out", [BL, N, E], f32, kind="ExternalInput")
    dec_d = nc.dram_tensor("decoder_hidden", [BL, D], f32, kind="ExternalInput")
    we_d = nc.dram_tensor("We", [E, A], f32, kind="ExternalInput")
    be_d = nc.dram_tensor("be", [A], f32, kind="ExternalInput")
    wd_d = nc.dram_tensor("Wd", [D, A], f32, kind="ExternalInput")
    bd_d = nc.dram_tensor("bd", [A], f32, kind="ExternalInput")
    wf_d = nc.dram_tensor("Wf", [A, 1], f32, kind="ExternalInput")
    ctx_d = nc.dram_tensor("context", [BL, E], f32, kind="ExternalOutput")
    alp_d = nc.dram_tensor("alpha", [BL, N, 1], f32, kind="ExternalOutput")

    WV = 392  # pair width in the transposed layout (2 * N)

    with tile.TileContext(nc) as tc, ExitStack() as ctx:
        consts = ctx.enter_context(tc.tile_pool(name="consts", bufs=1))

        ident = consts.tile([128, 128], f32)
        make_identity(nc, ident[:])
        ident_r = consts.tile([128, 128], f32r)
        nc.vector.tensor_copy(ident_r[:], ident[:])

        att2T_sb = consts.tile([128, AC, BL], f32)
        alphaT_pad = consts.tile([128, 2 * BL, BL], f32r)

        # open the main pools early so pair-0's enc loads are the first DMAs
        work = ctx.enter_context(tc.tile_pool(name="work", bufs=2))
        psum = ctx.enter_context(tc.tile_pool(name="psum", bufs=1, space="PSUM"))

        def emit_loads(p):
            # SWDGE (gpsimd) spreads each transfer across all 16 SDMA
            # engines; HWDGE rings would serialize on ~4 queues.
            b0, b1 = 2 * p, 2 * p + 1
            nat0 = work.tile([128, 2, E], f32r, tag="nat0", bufs=3, name="nat0")
            nat1 = work.tile([N1, 2, E], f32r, tag="nat1", bufs=3, name="nat1")
            # pair 0's SWDGE emission (~1us per dma_start on the Q7) is on
            # the startup critical path: use fewer, bigger transfers there
            nk, w = (2, 1024) if p == 0 else (4, 512)
            for j, b in enumerate((b0, b1)):
                for k in range(nk):
                    nc.gpsimd.dma_start(
                        nat0[:, j, ts(k, w)],
                        enc_d.ap()[b, 0:N0, ts(k, w)].bitcast(f32r),
                    )
                    nc.gpsimd.dma_start(
                        nat1[:, j, ts(k, w)],
                        enc_d.ap()[b, N0:N, ts(k, w)].bitcast(f32r),
                    )
            return nat0, nat1

        with tc.tile_pool(name="pre", bufs=1) as pre:
            wd_sb = pre.tile([128, DC, A], f32)
            for c in range(DC):
                nc.gpsimd.dma_start(wd_sb[:, c, :], wd_d.ap()[ts(c, 128), :])
            be_sb = pre.tile([128, AC], f32)
            bd_sb = pre.tile([128, AC], f32)
            nc.sync.dma_start(be_sb[:], be_d.ap().rearrange("(c p) -> p c", p=128))
            nc.sync.dma_start(bd_sb[:], bd_d.ap().rearrange("(c p) -> p c", p=128))
            bdbe = pre.tile([128, AC], f32)
            nc.vector.tensor_add(bdbe[:], be_sb[:], bd_sb[:])

            dec_sb = pre.tile([BL, D], f32)
            nc.gpsimd.dma_start(dec_sb[:], dec_d.ap())
            decT_sb = pre.tile([128, DC, BL], f32)
            for c in range(DC):
                ps_tp = psum.tile([128, WV], f32, tag="tp", bufs=2, name="ps_tp")
                nc.tensor.transpose(
                    ps_tp[:, 0:BL], dec_sb[:, ts(c, 128)], ident[0:BL, 0:BL]
                )
                nc.vector.tensor_copy(decT_sb[:, c, :], ps_tp[:, 0:BL])

            for ca in range(AC):
                ps_a2 = psum.tile([128, WV], f32, tag="tp", bufs=2, name="ps_a2")
                for cd in range(DC):
                    nc.tensor.matmul(
                        ps_a2[:, 0:BL],
                        wd_sb[:, cd, ts(ca, 128)],
                        decT_sb[:, cd, :],
                        start=(cd == 0),
                        stop=(cd == DC - 1),
                    )
                nc.scalar.add(att2T_sb[:, ca, :], ps_a2[:, 0:BL], bdbe[:, ca : ca + 1])

            nc.gpsimd.memset(alphaT_pad[:].bitcast(f32), 0.0)

        first_nats = emit_loads(0)

        we_sb = consts.tile([128, EC, A], f32r)
        for c in range(EC):
            nc.gpsimd.dma_start(we_sb[:, c, :], we_d.ap()[ts(c, 128), :].bitcast(f32r))
        wf_sb = consts.tile([128, AC], f32r)
        nc.sync.dma_start(
            wf_sb[:], wf_d.ap().rearrange("(c p) o -> p (c o)", p=128).bitcast(f32r)
        )

        ctx_ps = psum.tile([BL, E4, 512], f32, tag="ctx", bufs=1)

        def r(ap):
            return ap.bitcast(f32r)

        def copy_op(i, out, in_):
            if i % 2 == 0:
                nc.vector.tensor_copy(out, in_)
            else:
                nc.scalar.copy(out, in_)

        def transpose_jobs(p, nats, encT):
            """Yield closures: 4 PE transposes + 1 evacuation copy per e-chunk.
            Interleaved between att1 matmuls so the PE always has REGULAR
            matmuls in flight (transpose-mode doesn't count as HAM-busy)."""
            nat0, nat1 = nats
            state = {}

            def t_job(ce, j, half):
                def go():
                    if ce not in state:
                        state[ce] = psum.tile(
                            [128, WV], f32r, tag="tp", bufs=2, name="ps_t"
                        )
                    ps_t = state[ce]
                    off = N * j
                    if half == 0:
                        nc.tensor.transpose(
                            ps_t[:, off : off + N0],
                            nat0[:, j, ts(ce, 128)],
                            ident_r[:, :],
                        )
                    else:
                        nc.tensor.transpose(
                            ps_t[:, off + N0 : off + N],
                            nat1[:, j, ts(ce, 128)],
                            ident_r[0:N1, 0:N1],
                        )
                return go

            def c_job(ce):
                def go():
                    copy_op(ce, encT[:, ce, :], state.pop(ce)[:])
                return go

            jobs = []
            for ce in range(EC):
                for j in range(2):
                    for half in range(2):
                        jobs.append(t_job(ce, j, half))
                jobs.append(c_job(ce))
            return jobs

        def emit_transposes(p, nats):
            encT = work.tile([128, EC, WV], f32r, tag="encT", bufs=2, name="encT")
            for job in transpose_jobs(p, nats, encT):
                job()
            return encT

        def emit_att1(p, encT, ca_list, jobs=None):
            b0, b1 = 2 * p, 2 * p + 1
            att = att_tiles[p % 2]
            nmm = len(ca_list) * EC
            k = 0
            jpos = 0
            for ca in ca_list:
                ps_a = psum.tile([128, WV], f32, tag="att1", bufs=2, name="ps_a")
                for ce in range(EC):
                    nc.tensor.matmul(
                        ps_a[:],
                        we_sb[:, ce, ts(ca, 128)],
                        encT[:, ce, :],
                        start=(ce == 0),
                        stop=(ce == EC - 1),
                    )
                    if jobs:
                        k += 1
                        if k % 8 == 0:
                            jend = len(jobs) * k // nmm
                            while jpos < jend:
                                jobs[jpos]()
                                jpos += 1
                nc.scalar.activation(
                    att[:, ca, 0:N], ps_a[:, 0:N], TANH,
                    bias=att2T_sb[:, ca, b0 : b0 + 1],
                )
                nc.scalar.activation(
                    att[:, ca, N:WV], ps_a[:, N:WV], TANH,
                    bias=att2T_sb[:, ca, b1 : b1 + 1],
                )
            if jobs:
                while jpos < len(jobs):
                    jobs[jpos]()
                    jpos += 1

        def emit_scores_softmax(p):
            b0 = 2 * p
            att = att_tiles[p % 2]
            ps_s = psum.tile([128, WV], f32, tag="att1", bufs=2, name="ps_s")
            for ca in range(AC):
                nc.tensor.matmul(
                    ps_s[0:1, :],
                    wf_sb[:, ca : ca + 1],
                    att[:, ca, :],
                    start=(ca == 0),
                    stop=(ca == AC - 1),
                )
            sc_sb = work.tile([1, WV], f32, tag="sc", bufs=2, name="sc")
            nc.vector.tensor_copy(sc_sb[:], ps_s[0:1, :])

            mx = work.tile([1, 2], f32, tag="mx", bufs=2, name="mx")
            sm = work.tile([1, 2], f32, tag="sm", bufs=2, name="sm")
            rs = work.tile([1, 2], f32, tag="rs", bufs=2, name="rs")
            nc.vector.reduce_max(
                mx[:],
                sc_sb[:].rearrange("p (h w) -> p h w", h=2),
                axis=mybir.AxisListType.X,
                negate=True,
            )
            al = work.tile([1, WV], f32, tag="al", bufs=2, name="al")
            for h in range(2):
                nc.scalar.activation(
                    al[0:1, ts(h, N)], sc_sb[0:1, ts(h, N)], EXP,
                    bias=mx[0:1, h : h + 1],
                    accum_out=sm[0:1, h : h + 1],
                )
            nc.vector.reciprocal(rs[:], sm[:])
            for h in range(2):
                nc.vector.tensor_scalar_mul(
                    al[0:1, ts(h, N)], al[0:1, ts(h, N)], rs[0:1, h : h + 1]
                )
            nc.gpsimd.dma_start(
                alp_d.ap()[b0 : b0 + 2].rearrange("b n o -> o (b n)"), al[:]
            )
            return al

        def emit_alpha_ctx(p, al, nats):
            nat0, nat1 = nats
            b0, b1 = 2 * p, 2 * p + 1
            ps_al = psum.tile([128, WV], f32, tag="att1", bufs=2, name="ps_al")
            for j, b in enumerate((b0, b1)):
                off = N * j
                nc.tensor.transpose(
                    ps_al[:, 2 * j : 2 * j + 1],
                    al[0:1, off : off + N0],
                    ident[0:1, 0:1],
                )
                nc.tensor.transpose(
                    ps_al[0:N1, 2 * j + 1 : 2 * j + 2],
                    al[0:1, off + N0 : off + N],
                    ident[0:1, 0:1],
                )
                nc.vector.tensor_copy(
                    alphaT_pad[:, 2 * b, b : b + 1], ps_al[:, 2 * j : 2 * j + 1]
                )
                nc.vector.tensor_copy(
                    alphaT_pad[0:N1, 2 * b + 1, b : b + 1],
                    ps_al[0:N1, 2 * j + 1 : 2 * j + 2],
                )
            for j, b in enumerate((b0, b1)):
                for e4 in range(E4):
                    nc.tensor.matmul(
                        ctx_ps[:, e4, :],
                        r(alphaT_pad[:, 2 * b, :]),
                        nat0[:, j, ts(e4, 512)],
                        start=(p == 0 and j == 0),
                        stop=False,
                        skip_group_check=True,
                    )
                    nc.tensor.matmul(
                        ctx_ps[:, e4, :],
                        r(alphaT_pad[0:N1, 2 * b + 1, :]),
                        nat1[:, j, ts(e4, 512)],
                        start=False,
                        stop=(p == PAIRS - 1 and j == 1),
                        skip_group_check=True,
                    )

        att_one = work.tile([128, AC, WV], f32r, tag="att", bufs=1, name="att_one")
        att_tiles = [att_one, att_one]

        # Pipeline: loads 1 section ahead (SWDGE); per section:
        #   transposes(p) ; alphaT+ctx(p-1) ; att1(p) ; scores+softmax(p)
        # nat(p) lives load(p-1)..ctx(p+1) -> bufs=3. encT written and read
        # within the section (PE in-order) -> bufs=1.
        # Pipeline: loads one section ahead; transposes of pair p+1 are
        # interleaved between the att1 matmuls of pair p (2:1) so the PE
        # stream never has a transpose-only phase (HAM stays warm).
        nats = {0: first_nats}
        als = {}
        encTs = {0: emit_transposes(0, nats[0])}
        for p in range(PAIRS):
            if p + 1 < PAIRS:
                nats[p + 1] = emit_loads(p + 1)
            emit_att1(p, encTs[p], (0, 1))
            if p >= 1:
                emit_alpha_ctx(p - 1, als.pop(p - 1), nats.pop(p - 1))
            if p + 1 < PAIRS:
                encTs[p + 1] = work.tile(
                    [128, EC, WV], f32r, tag="encT", bufs=2, name="encT"
                )
                jobs = transpose_jobs(p + 1, nats[p + 1], encTs[p + 1])
            else:
                jobs = None
            emit_att1(p, encTs.pop(p), (2, 3), jobs=jobs)
            als[p] = emit_scores_softmax(p)
        emit_alpha_ctx(PAIRS - 1, als.pop(PAIRS - 1), nats.pop(PAIRS - 1))

        for e4 in range(E4):
            ctx_e = work.tile([BL, 512], f32, tag="ctxe", bufs=2, name="ctxe")
            copy_op(e4, ctx_e[:], ctx_ps[:, e4, :])
            nc.gpsimd.dma_start(ctx_d.ap()[:, ts(e4, 512)], ctx_e[:])

    return nc


def _get_nc():
    if "nc" not in _CACHE:
        _CACHE["nc"] = build()
    return _CACHE["nc"]


def _run(in_maps, trace=False, tmpdir=None):
    from concourse.bass_utils import run_bass_kernel_spmd

    nc = _get_nc()
    return run_bass_kernel_spmd(
        nc, in_maps, core_ids=list(range(NCORES)), trace=trace, tmpdir=tmpdir
    )


def make_in_maps(encoder_out, decoder_hidden, We, be, Wd, bd, Wf, bf=None):
    enc = np.ascontiguousarray(np.asarray(encoder_out, dtype=np.float32))
    dec = np.ascontiguousarray(np.asarray(decoder_hidden, dtype=np.float32))
    We = np.ascontiguousarray(np.asarray(We, dtype=np.float32))
    be = np.ascontiguousarray(np.asarray(be, dtype=np.float32))
    Wd = np.ascontiguousarray(np.asarray(Wd, dtype=np.float32))
    bd = np.ascontiguousarray(np.asarray(bd, dtype=np.float32))
    Wf = np.ascontiguousarray(np.asarray(Wf, dtype=np.float32))
    in_maps = []
    for i in range(NCORES):
        sl = slice(i * BL, (i + 1) * BL)
        in_maps.append(
            dict(
                encoder_out=enc[sl],
                decoder_hidden=dec[sl],
                We=We,
                be=be,
                Wd=Wd,
                bd=bd,
                Wf=Wf,
            )
        )
    return in_maps


def gather(results):
    context = np.concatenate([results[i]["context"] for i in range(NCORES)], axis=0)
    alpha = np.concatenate([results[i]["alpha"] for i in range(NCORES)], axis=0)
    return context, alpha


def kernel(encoder_out, decoder_hidden, We, be, Wd, bd, Wf, bf):
    in_maps = make_in_maps(encoder_out, decoder_hidden, We, be, Wd, bd, Wf, bf)
    res = _run(in_maps, trace=False)
    _CACHE["last_results"] = res
    return gather(res.results)
